# revision 12
# baseline (speedup 1.0000x reference)
"""Bidirectional Mamba layer on 8 Trainium2 NeuronCores (Bass/Tile).

Sharding: 8 cores = 2 directions x 4 batch samples. Each core runs the full
Mamba block for its (direction, sample) pair; a pairwise AllGather combines
the two directions, and every core (redundantly, SPMD-symmetric) applies the
final residual + LayerNorm.

Layout: channels on partitions, time on the free axis, full L=2048 per op
(monolith — no time chunking, so the selective scan needs no state carry).
All matmuls run bf16 (single PE pass); elementwise work is bf16 for the DVE
2x mode. The per-(group, state) inner loop is:
  Act:  da = exp(dt * A[g,n])          (scalar engine)
  DVE/GPS: bt = p * B_n                (broadcast multiply)
  DVE:  h = tensor_tensor_scan(da, bt) (the irreducible 2 cyc/elem scan)
  DVE:  yn = h * C_n
  PE:   py += I @ yn                   (PSUM accumulation over states)
"""

import ml_dtypes
import numpy as np

import concourse.bass as bass
import concourse.bacc as bacc
import concourse.tile as tile
from concourse import mybir
from concourse.bass_utils import run_bass_kernel_spmd

# ---- problem shapes (hardcoded per contract) ----
B_SZ, L, DM = 4, 2048, 512
D_CONV, DS, DR = 4, 16, 32
DI = 1024                  # d_inner
E2 = 2 * DI                # in_proj rows
NG = DI // 128             # 8 channel blocks
NTT = L // 128             # 16 time tiles of 128
NSL = L // 512             # 4 psum-width slices
EPS = 1e-5
F32 = mybir.dt.float32
BF16 = mybir.dt.bfloat16
AF = mybir.ActivationFunctionType
OP = mybir.AluOpType

N_CORES = 8

# CoreSim does not implement Silu; tests flip this to use sigmoid+mult
USE_SILU = True
REPLICA_GROUPS = [[0, 4], [1, 5], [2, 6], [3, 7]]

# groups whose bt-/yn-multiply runs on GPSIMD instead of DVE
GPS_GS = (1, 3, 5, 7)
GPS_YS = (1, 5)


def _silu(nc, pool, out_tile, psum, bias=None):
    """out = silu(psum [+ bias]); Silu on HW, sigmoid+mult fallback for sim."""
    kw = {} if bias is None else {"bias": bias}
    if USE_SILU:
        nc.scalar.activation(out_tile, psum, AF.Silu, **kw)
    else:
        zb = pool.tile(list(out_tile.shape), F32, tag="_silu_zb", name="zb",
                       bufs=1)
        nc.scalar.activation(zb, psum, AF.Identity, **kw)
        nc.scalar.activation(out_tile, psum, AF.Sigmoid, **kw)
        nc.vector.tensor_mul(out_tile, zb, out_tile)


def build_module():
    nc = bacc.Bacc(
        "TRN2", target_bir_lowering=False, debug=False, num_devices=N_CORES
    )

    # ---------------- I/O ----------------
    x_d = nc.dram_tensor("x_d", [L, DM], F32, kind="ExternalInput")
    x_nat = nc.dram_tensor("x_nat", [L, DM], BF16, kind="ExternalInput")
    w_in = nc.dram_tensor("w_in", [DM, E2], BF16, kind="ExternalInput")
    convd = nc.dram_tensor("convd", [D_CONV, NG, 128, 128], BF16, kind="ExternalInput")
    convb = nc.dram_tensor("convb", [NG, 128], F32, kind="ExternalInput")
    silub = nc.dram_tensor("silub", [NG, 128], F32, kind="ExternalInput")
    w_xp = nc.dram_tensor("w_xp", [DI, DR + 2 * DS], BF16, kind="ExternalInput")
    w_dt = nc.dram_tensor("w_dt", [DR, DI], BF16, kind="ExternalInput")
    dtb = nc.dram_tensor("dtb", [NG, 128], F32, kind="ExternalInput")
    a_sc = nc.dram_tensor("a_sc", [NG, 128, DS], F32, kind="ExternalInput")
    d_vec = nc.dram_tensor("d_vec", [NG, 128], F32, kind="ExternalInput")
    w_out = nc.dram_tensor("w_out", [DI, DM], BF16, kind="ExternalInput")
    eye = nc.dram_tensor("eye", [128, 128], F32, kind="ExternalInput")
    eyen = nc.dram_tensor("eyen", [128, 128], BF16, kind="ExternalInput")
    eyej = nc.dram_tensor("eyej", [128, 128], BF16, kind="ExternalInput")
    ln2w = nc.dram_tensor("ln2w", [1, DM], F32, kind="ExternalInput")
    ln2b = nc.dram_tensor("ln2b", [1, DM], F32, kind="ExternalInput")
    out = nc.dram_tensor("out", [L, DM], F32, kind="ExternalOutput")

    with tile.TileContext(nc) as tc:
        build_program(
            tc, x_d, x_nat, w_in, convd, convb, silub, w_xp, w_dt, dtb,
            a_sc, d_vec, w_out, eye, eyen, eyej, ln2w, ln2b, out,
        )
    nc.compile()
    return nc


def build_program(tc, x_d, x_nat, w_in, convd, convb, silub, w_xp, w_dt, dtb,
                  a_sc, d_vec, w_out, eye, eyen, eyej, ln2w, ln2b, out):
    nc = tc.nc
    from contextlib import ExitStack

    ctx = ExitStack()
    with ctx:
        wp = ctx.enter_context(tc.tile_pool(name="wp", bufs=1))
        lnp = ctx.enter_context(tc.tile_pool(name="lnp", bufs=2))
        big = ctx.enter_context(tc.tile_pool(name="big", bufs=1))
        snp = ctx.enter_context(tc.tile_pool(name="snp", bufs=3))
        repp = ctx.enter_context(tc.tile_pool(name="repp", bufs=3))
        dramp = ctx.enter_context(tc.tile_pool(name="dramp", bufs=2, space="DRAM"))
        pp_ctx = tc.tile_pool(name="pp", space="PSUM", bufs=2)
        pp = pp_ctx.__enter__()

        w_in_sb = wp.tile([128, DM // 128, E2], BF16)
        for k in range(DM // 128):
            nc.sync.dma_start(w_in_sb[:, k, :], w_in[k * 128:(k + 1) * 128, :])
        convd_sb = wp.tile([128, D_CONV, NG, 128], BF16)
        for k in range(D_CONV):
            for g in range(NG):
                nc.sync.dma_start(convd_sb[:, k, g, :], convd[k, g])
        w_xp_sb = wp.tile([128, NG, DR + 2 * DS], BF16)
        for g in range(NG):
            nc.sync.dma_start(w_xp_sb[:, g, :], w_xp[g * 128:(g + 1) * 128, :])
        w_dt_sb = wp.tile([DR, DI], BF16)
        nc.sync.dma_start(w_dt_sb, w_dt[:, :])
        w_out_sb = wp.tile([128, NG, DM], BF16)
        for g in range(NG):
            nc.sync.dma_start(w_out_sb[:, g, :], w_out[g * 128:(g + 1) * 128, :])
        a_sb = wp.tile([128, NG, DS], F32)
        for g in range(NG):
            nc.sync.dma_start(a_sb[:, g, :], a_sc[g])
        eye_sb = wp.tile([128, 128], F32)
        nc.sync.dma_start(eye_sb, eye[:, :])
        eyen_sb = wp.tile([128, 128], BF16)
        nc.sync.dma_start(eyen_sb, eyen[:, :])
        eyej_sb = wp.tile([128, 128], BF16)
        nc.sync.dma_start(eyej_sb, eyej[:, :])

        def col_load(name, src):
            t = wp.tile([128, NG], F32, name=name)
            for g in range(NG):
                nc.sync.dma_start(t[:, g:g + 1], src[g:g + 1, :].rearrange("a b -> b a"))
            return t

        convb_sb = col_load("convb_sb", convb)
        silub_sb = col_load("silub_sb", silub)
        dtb_sb = col_load("dtb_sb", dtb)
        d_sb = col_load("d_sb", d_vec)

        ln2w_sb = wp.tile([128, DM], F32)
        nc.sync.dma_start(ln2w_sb, ln2w[0:1, :].to_broadcast([128, DM]))
        ln2b_sb = wp.tile([128, DM], F32)
        nc.sync.dma_start(ln2b_sb, ln2b[0:1, :].to_broadcast([128, DM]))

        eps_sb = wp.tile([128, 1], F32)
        nc.vector.memset(eps_sb, EPS)

        ypart = dramp.tile([L, DM], BF16, bufs=1)
        sg_d = dramp.tile([NG, 128, L], BF16, bufs=1)
        gbuf = dramp.tile([2, L, DM], BF16, bufs=1)
        bc_d = dramp.tile([2 * DS, L], BF16, bufs=1)

        # ---------------- phase A: LayerNorm1 + transpose ----------------
        # xn_t: [dm-part, k-block, t] bf16, consumed by in_proj
        xn_t = big.tile([128, DM // 128, L], BF16, name="xn_t")
        # batched stats: mean/var for all 16 tiles, then ONE Ln + ONE Exp for
        # rstd (avoids per-tile activation-table ping-pong); x is loaded twice
        # (stats sweep, then normalize sweep)
        mvall = wp.tile([128, 2, NTT], F32, name="mvall")
        nmr = wp.tile([128, NTT], F32, name="nmr")
        for i in range(NTT):
            xt = lnp.tile([128, DM], F32, tag="xt", bufs=4)
            nc.sync.dma_start(xt, x_d[i * 128:(i + 1) * 128, :])
            st = lnp.tile([128, 6], F32, tag="st")
            nc.vector.bn_stats(st, xt)
            nc.vector.bn_aggr(mvall[:, :, i], st)
        nc.scalar.activation(mvall[:, 1, :], mvall[:, 1, :], AF.Ln,
                             bias=eps_sb[:, 0:1])
        nc.scalar.activation(mvall[:, 1, :], mvall[:, 1, :], AF.Exp,
                             scale=-0.5)
        nc.vector.tensor_mul(nmr, mvall[:, 0, :], mvall[:, 1, :])
        nc.vector.tensor_scalar_mul(nmr, nmr, -1.0)
        for i in range(NTT):
            xt = lnp.tile([128, DM], F32, tag="xt", bufs=4)
            nc.sync.dma_start(xt, x_d[i * 128:(i + 1) * 128, :])
            nc.scalar.activation(xt, xt, AF.Identity,
                                 scale=mvall[:, 1, i:i + 1],
                                 bias=nmr[:, i:i + 1])
            for kb in range(DM // 128):
                ps_t = pp.tile([128, 512], F32, tag="ps")
                nc.tensor.transpose(ps_t[:, 0:128], xt[:, kb * 128:(kb + 1) * 128],
                                    eye_sb)
                nc.scalar.copy(xn_t[:, kb, i * 128:(i + 1) * 128],
                               ps_t[:, 0:128])

        # ---------------- phase B: in_proj ----------------
        # xc half into conv staging (3-col zero halo at front)
        xzf = [big.tile([128, L + 3], BF16, name=f"xzf{g}") for g in range(NG)]
        for g in range(NG):
            nc.vector.memset(xzf[g][:, 0:3], 0.0)
            for s in range(NSL):
                pz = pp.tile([128, 512], F32, tag="ps")
                for k in range(DM // 128):
                    nc.tensor.matmul(pz, w_in_sb[:, k, g * 128:(g + 1) * 128],
                                     xn_t[:, k, s * 512:(s + 1) * 512],
                                     start=(k == 0), stop=(k == DM // 128 - 1))
                nc.scalar.copy(xzf[g][:, 3 + s * 512:3 + (s + 1) * 512], pz)

        # z half + SiLU gate -> sg (spilled to DRAM, reloaded at gate)
        for g in range(NG):
            for s in range(NSL):
                pz = pp.tile([128, 512], F32, tag="ps")
                for k in range(DM // 128):
                    nc.tensor.matmul(
                        pz, w_in_sb[:, k, DI + g * 128: DI + (g + 1) * 128],
                        xn_t[:, k, s * 512:(s + 1) * 512],
                        start=(k == 0), stop=(k == DM // 128 - 1))
                sgt = snp.tile([128, 512], BF16, tag="sgt", bufs=2)
                _silu(nc, snp, sgt, pz, silub_sb[:, g:g + 1])
                nc.sync.dma_start(sg_d[g][:, s * 512:(s + 1) * 512], sgt)

        # ---------------- phase C: depthwise causal conv + SiLU ----------
        xc = [big.tile([128, L], BF16, name=f"xc{g}") for g in range(NG)]
        for g in range(NG):
            for s in range(NSL):
                pc = pp.tile([128, 512], F32, tag="ps")
                for k in range(D_CONV):
                    nc.tensor.matmul(pc, convd_sb[:, k, g, :],
                                     xzf[g][:, k + s * 512: k + (s + 1) * 512],
                                     start=(k == 0), stop=(k == D_CONV - 1))
                _silu(nc, snp, xc[g][:, s * 512:(s + 1) * 512], pc,
                      convb_sb[:, g:g + 1])

        # ---------------- phase D: x_proj -> (dt_r, B, C) -----------------
        xdbl = big.tile([64, L], BF16, name="xdbl")
        for s in range(NSL):
            pxd = pp.tile([64, 512], F32, tag="pxd", bufs=1)
            for g in range(NG):
                nc.tensor.matmul(pxd, w_xp_sb[:, g, :],
                                 xc[g][:, s * 512:(s + 1) * 512],
                                 start=(g == 0), stop=(g == NG - 1))
            nc.scalar.copy(xdbl[:, s * 512:(s + 1) * 512], pxd)
        nc.sync.dma_start(bc_d, xdbl[DR:DR + 2 * DS, :])

        # dt = softplus(dt_proj @ dt_r + bias); p = dt * xc
        # dt_c[0..3] reuse xn_t's space, dt_c[4..7] reuse w_in_sb (both dead
        # after in_proj); p_c reuses the conv staging xzf (dead after conv)
        dt_c = [xn_t[:, g, :] if g < 4 else w_in_sb[:, g - 4, :]
                for g in range(NG)]
        p_c = [xzf[g][:, 0:L] for g in range(NG)]
        for g in range(NG):
            for s in range(NSL):
                sl = slice(s * 512, (s + 1) * 512)
                pdt = pp.tile([128, 512], F32, tag="ps")
                nc.tensor.matmul(pdt, w_dt_sb[:, g * 128:(g + 1) * 128],
                                 xdbl[0:DR, sl],
                                 start=True, stop=True)
                nc.scalar.activation(dt_c[g][:, sl], pdt, AF.Exp,
                                     bias=dtb_sb[:, g:g + 1])
        for g in range(NG):
            nc.scalar.activation(dt_c[g], dt_c[g], AF.Ln, bias=1.0)
            nc.gpsimd.tensor_tensor(p_c[g], dt_c[g], xc[g], op=OP.mult)

        # ---------------- phase E: selective scan ------------------------
        # wave-outer (4 groups share each B/C broadcast), time in halves of
        # 1024 so four 2-bank PSUM accumulators fit; h carried across halves
        pp_ctx.__exit__(None, None, None)
        HW = L // 2
        yf = [xc[g] for g in range(NG)]
        with tc.tile_pool(name="ppy", space="PSUM", bufs=1) as ppy:
            for w0 in (0, 4):
                wave = range(w0, w0 + 4)
                hcar = snp.tile([128, 4, DS], BF16, tag="hcar", bufs=2,
                                name=f"hcar{w0}")
                for half in (0, 1):
                    hs = slice(half * HW, (half + 1) * HW)
                    py = [ppy.tile([128, HW], F32, tag=f"py{j}", bufs=1,
                                   name=f"py{j}") for j in range(4)]
                    for j, g in enumerate(wave):
                        yd = snp.tile([128, HW], BF16, tag="yd", bufs=2)
                        nc.vector.tensor_scalar_mul(yd, xc[g][:, hs],
                                                    d_sb[:, g:g + 1])
                        for s in range(2):
                            nc.tensor.matmul(py[j][:, s * 512:(s + 1) * 512],
                                             eyen_sb,
                                             yd[:, s * 512:(s + 1) * 512],
                                             start=True, stop=False)
                    for n in range(DS):
                        brep = repp.tile([128, HW], BF16, tag="brep", bufs=3)
                        nc.sync.dma_start(
                            brep, bc_d[n:n + 1, hs].to_broadcast([128, HW]))
                        crep = repp.tile([128, HW], BF16, tag="crep", bufs=3)
                        nc.sync.dma_start(
                            crep,
                            bc_d[DS + n:DS + n + 1, hs].to_broadcast([128, HW]))
                        for j, g in enumerate(wave):
                            da = snp.tile([128, HW], BF16, tag="da", bufs=3)
                            nc.scalar.activation(da, dt_c[g][:, hs], AF.Exp,
                                                 scale=a_sb[:, g, n:n + 1])
                            if g in GPS_GS:
                                bt = snp.tile([128, HW], BF16, tag="bt",
                                              name="btg", bufs=3)
                                nc.gpsimd.tensor_tensor(bt, p_c[g][:, hs],
                                                        brep, op=OP.mult)
                            else:
                                bt = snp.tile([128, HW], BF16, tag="bt",
                                              name="btd", bufs=3)
                                nc.vector.tensor_mul(bt, p_c[g][:, hs], brep)
                            h = snp.tile([128, HW], BF16, tag="h", bufs=2)
                            init = 0.0 if half == 0 else hcar[:, j, n:n + 1]
                            nc.vector.tensor_tensor_scan(h, da, bt,
                                                         initial=init,
                                                         op0=OP.mult,
                                                         op1=OP.add)
                            if half == 0:
                                nc.scalar.copy(hcar[:, j, n:n + 1],
                                               h[:, HW - 1:HW])
                            yn = snp.tile([128, HW], BF16, tag="yn", bufs=2)
                            if g in GPS_YS:
                                nc.gpsimd.tensor_tensor(yn, h, crep,
                                                        op=OP.mult)
                            else:
                                nc.vector.tensor_mul(yn, h, crep)
                            for s in range(2):
                                nc.tensor.matmul(
                                    py[j][:, s * 512:(s + 1) * 512], eyen_sb,
                                    yn[:, s * 512:(s + 1) * 512],
                                    start=False, stop=(n == DS - 1))
                    # gate with silu(z); write into xc[g]'s tile (xc dead now)
                    for j, g in enumerate(wave):
                        sgl = repp.tile([128, HW], BF16, tag="sgl", bufs=2)
                        nc.sync.dma_start(sgl, sg_d[g][:, hs])
                        nc.vector.tensor_mul(yf[g][:, hs], py[j], sgl)

        # ---------------- phase F: out_proj -> ypart ----------------------
        pp2_ctx = tc.tile_pool(name="pp2", space="PSUM", bufs=2)
        pp = pp2_ctx.__enter__()
        for tb in range(NTT):
            po = pp.tile([128, DM], F32, tag="ps")
            for g in range(NG):
                nc.tensor.matmul(po, yf[g][:, tb * 128:(tb + 1) * 128],
                                 w_out_sb[:, g, :],
                                 start=(g == 0), stop=(g == NG - 1))
            ot = lnp.tile([128, DM], BF16, tag="ot")
            nc.vector.tensor_copy(ot, po)
            nc.sync.dma_start(
                ypart[tb * 128:(tb + 1) * 128, :], ot)

        # ---------------- combine directions + final LayerNorm ------------
        nc.gpsimd.collective_compute(
            "AllGather", OP.bypass, replica_groups=REPLICA_GROUPS,
            ins=[ypart.opt()], outs=[gbuf.opt()],
        )
        mvall2 = wp.tile([128, 2, NTT], F32, name="mvall2")
        nmr2 = wp.tile([128, NTT], F32, name="nmr2")

        def sum3(i):
            s0 = lnp.tile([128, DM], BF16, tag="s0")
            nc.sync.dma_start(s0, gbuf[0, i * 128:(i + 1) * 128, :])
            # direction-1 partial is in flipped time order: load the mirrored
            # block forward, then reverse rows via the exchange matrix
            s1 = lnp.tile([128, DM], BF16, tag="s1")
            nc.sync.dma_start(
                s1, gbuf[1, L - (i + 1) * 128: L - i * 128, :])
            xt2 = lnp.tile([128, DM], BF16, tag="xt2")
            nc.sync.dma_start(xt2, x_nat[i * 128:(i + 1) * 128, :])
            pj = pp.tile([128, DM], F32, tag="ps")
            nc.tensor.matmul(pj, eyej_sb, s1, start=True, stop=False)
            nc.tensor.matmul(pj, eyen_sb, s0, start=False, stop=False)
            nc.tensor.matmul(pj, eyen_sb, xt2, start=False, stop=True)
            return pj

        # sweep 1: stats for every tile (batched rstd afterwards)
        for i in range(NTT):
            pj = sum3(i)
            st = lnp.tile([128, 6], F32, tag="st2")
            nc.vector.bn_stats(st, pj)
            nc.vector.bn_aggr(mvall2[:, :, i], st)
        nc.scalar.activation(mvall2[:, 1, :], mvall2[:, 1, :], AF.Ln,
                             bias=eps_sb[:, 0:1])
        nc.scalar.activation(mvall2[:, 1, :], mvall2[:, 1, :], AF.Exp,
                             scale=-0.5)
        nc.vector.tensor_mul(nmr2, mvall2[:, 0, :], mvall2[:, 1, :])
        nc.vector.tensor_scalar_mul(nmr2, nmr2, -1.0)
        # sweep 2: rebuild the sum (3 cheap matmuls) and normalize from PSUM
        for i in range(NTT):
            pj = sum3(i)
            s = lnp.tile([128, DM], F32, tag="ssum")
            nc.scalar.activation(s, pj, AF.Identity,
                                 scale=mvall2[:, 1, i:i + 1],
                                 bias=nmr2[:, i:i + 1])
            nc.vector.tensor_mul(s, s, ln2w_sb)
            nc.vector.tensor_add(s, s, ln2b_sb)
            nc.sync.dma_start(out[i * 128:(i + 1) * 128, :], s)
        pp2_ctx.__exit__(None, None, None)


# ---------------- host side ----------------

def make_core_inputs(inputs):
    """Build the 8 per-core input dicts from the full problem inputs."""
    x = np.ascontiguousarray(inputs["x"], dtype=np.float32)        # [B, L, DM]
    ln1_w = inputs["ln1_w"].astype(np.float32)
    ln1_b = inputs["ln1_b"].astype(np.float32)
    in_w = inputs["in_proj_w"].astype(np.float32)                  # [2, 2DI, DM]
    conv_w = inputs["conv_w"].astype(np.float32)                   # [2, DI, 4]
    conv_b = inputs["conv_b"].astype(np.float32)                   # [2, DI]
    xp_w = inputs["x_proj_w"].astype(np.float32)                   # [2, 64, DI]
    dt_w = inputs["dt_proj_w"].astype(np.float32)                  # [2, DI, DR]
    dt_b = inputs["dt_proj_b"].astype(np.float32)                  # [2, DI]
    a_log = inputs["A_log"].astype(np.float32)                     # [2, DI, DS]
    d_par = inputs["D_param"].astype(np.float32)                   # [2, DI]
    out_w = inputs["out_proj_w"].astype(np.float32)                # [2, DM, DI]
    ln2_w = inputs["ln2_w"].astype(np.float32)
    ln2_b = inputs["ln2_b"].astype(np.float32)

    bf = ml_dtypes.bfloat16
    eye = np.eye(128, dtype=np.float32)
    per_dir = []
    for d in range(2):
        w = in_w[d]                                   # [2DI, DM]
        w_in_T = np.ascontiguousarray((w * ln1_w[None, :]).T)      # [DM, 2DI]
        v = w @ ln1_b                                  # [2DI]
        csum = conv_w[d].sum(axis=1)                   # [DI]
        convb_adj = conv_b[d] + csum * v[:DI]
        silub_adj = v[DI:]
        convd = np.zeros((D_CONV, NG, 128, 128), np.float32)
        for k in range(D_CONV):
            for g in range(NG):
                np.fill_diagonal(convd[k, g], conv_w[d, g * 128:(g + 1) * 128, k])
        a_neg = -np.exp(a_log[d])                      # [DI, DS]
        per_dir.append(dict(
            w_in=w_in_T.astype(bf),
            convd=convd.astype(bf),
            convb=convb_adj.reshape(NG, 128),
            silub=silub_adj.reshape(NG, 128),
            w_xp=np.ascontiguousarray(xp_w[d].T).astype(bf),       # [DI, 64]
            w_dt=np.ascontiguousarray(dt_w[d].T).astype(bf),       # [DR, DI]
            dtb=dt_b[d].reshape(NG, 128),
            a_sc=np.ascontiguousarray(a_neg.reshape(NG, 128, DS)),
            d_vec=d_par[d].reshape(NG, 128),
            w_out=np.ascontiguousarray(out_w[d].T).astype(bf),     # [DI, DM]
        ))

    in_maps = []
    for core in range(N_CORES):
        d, b = core // 4, core % 4
        xb = x[b]
        m = dict(per_dir[d])
        m["x_d"] = xb if d == 0 else np.ascontiguousarray(xb[::-1])
        m["x_nat"] = xb.astype(bf)
        m["eye"] = eye
        m["eyen"] = eye.astype(bf)
        m["eyej"] = eye[::-1].copy().astype(bf)
        m["ln2w"] = ln2_w.reshape(1, DM)
        m["ln2b"] = ln2_b.reshape(1, DM)
        in_maps.append(m)
    return in_maps


_NC = None


def _get_module():
    global _NC
    if _NC is None:
        _NC = build_module()
    return _NC


def kernel(**inputs) -> np.ndarray:
    nc = _get_module()
    in_maps = make_core_inputs(inputs)
    res = run_bass_kernel_spmd(nc, in_maps, core_ids=list(range(N_CORES)))
    outs = [res.results[b]["out"] for b in range(B_SZ)]
    return np.stack(outs, axis=0)


if __name__ == "__main__":
    nc = build_module()
    print("module built ok")


# revision 15
# speedup vs baseline: 1.2154x; 1.2154x over previous
"""Bidirectional Mamba layer on 8 Trainium2 NeuronCores (Bass/Tile).

Sharding: 8 cores = 2 directions x 4 batch samples. Each core runs the full
Mamba block for its (direction, sample) pair; a pairwise AllGather combines
the two directions, and every core (redundantly, SPMD-symmetric) applies the
final residual + LayerNorm.

Layout: channels on partitions, time on the free axis, full L=2048 per op
(monolith — no time chunking, so the selective scan needs no state carry).
All matmuls run bf16 (single PE pass); elementwise work is bf16 for the DVE
2x mode. The per-(group, state) inner loop is:
  Act:  da = exp(dt * A[g,n])          (scalar engine)
  DVE/GPS: bt = p * B_n                (broadcast multiply)
  DVE:  h = tensor_tensor_scan(da, bt) (the irreducible 2 cyc/elem scan)
  DVE:  yn = h * C_n
  PE:   py += I @ yn                   (PSUM accumulation over states)
"""

import ml_dtypes
import numpy as np

import concourse.bass as bass
import concourse.bacc as bacc
import concourse.tile as tile
from concourse import mybir
from concourse.bass_utils import run_bass_kernel_spmd

# ---- problem shapes (hardcoded per contract) ----
B_SZ, L, DM = 4, 2048, 512
D_CONV, DS, DR = 4, 16, 32
DI = 1024                  # d_inner
E2 = 2 * DI                # in_proj rows
NG = DI // 128             # 8 channel blocks
NTT = L // 128             # 16 time tiles of 128
NSL = L // 512             # 4 psum-width slices
EPS = 1e-5
F32 = mybir.dt.float32
BF16 = mybir.dt.bfloat16
AF = mybir.ActivationFunctionType
OP = mybir.AluOpType

N_CORES = 8

# CoreSim does not implement Silu; tests flip this to use sigmoid+mult
USE_SILU = True
REPLICA_GROUPS = [[0, 4], [1, 5], [2, 6], [3, 7]]

# groups whose bt-/yn-multiply runs on GPSIMD instead of DVE.
# Empirically GPSIMD elementwise is net-negative here: it contends for the
# DVE's SBUF port and slows 2x-mode DVE ops more than it absorbs.
GPS_GS = ()
GPS_YS = ()


def _silu(nc, pool, out_tile, psum, bias=None):
    """out = silu(psum [+ bias]); Silu on HW, sigmoid+mult fallback for sim."""
    kw = {} if bias is None else {"bias": bias}
    if USE_SILU:
        nc.scalar.activation(out_tile, psum, AF.Silu, **kw)
    else:
        zb = pool.tile(list(out_tile.shape), F32, tag="_silu_zb", name="zb",
                       bufs=1)
        nc.scalar.activation(zb, psum, AF.Identity, **kw)
        nc.scalar.activation(out_tile, psum, AF.Sigmoid, **kw)
        nc.vector.tensor_mul(out_tile, zb, out_tile)


def build_module():
    nc = bacc.Bacc(
        "TRN2", target_bir_lowering=False, debug=False, num_devices=N_CORES
    )

    # ---------------- I/O ----------------
    x_d = nc.dram_tensor("x_d", [L, DM], F32, kind="ExternalInput")
    x_nat = nc.dram_tensor("x_nat", [L, DM], BF16, kind="ExternalInput")
    w_in = nc.dram_tensor("w_in", [DM, E2], BF16, kind="ExternalInput")
    convd = nc.dram_tensor("convd", [D_CONV, NG, 128, 128], BF16, kind="ExternalInput")
    convb = nc.dram_tensor("convb", [NG, 128], F32, kind="ExternalInput")
    silub = nc.dram_tensor("silub", [NG, 128], F32, kind="ExternalInput")
    w_xp = nc.dram_tensor("w_xp", [DI, DR + 2 * DS], BF16, kind="ExternalInput")
    w_dt = nc.dram_tensor("w_dt", [DR, DI], BF16, kind="ExternalInput")
    dtb = nc.dram_tensor("dtb", [NG, 128], F32, kind="ExternalInput")
    a_sc = nc.dram_tensor("a_sc", [NG, 128, DS], F32, kind="ExternalInput")
    d_vec = nc.dram_tensor("d_vec", [NG, 128], F32, kind="ExternalInput")
    w_out = nc.dram_tensor("w_out", [DI, DM], BF16, kind="ExternalInput")
    eye = nc.dram_tensor("eye", [128, 128], F32, kind="ExternalInput")
    eyen = nc.dram_tensor("eyen", [128, 128], BF16, kind="ExternalInput")
    eyej = nc.dram_tensor("eyej", [128, 128], BF16, kind="ExternalInput")
    ln2w = nc.dram_tensor("ln2w", [1, DM], F32, kind="ExternalInput")
    ln2b = nc.dram_tensor("ln2b", [1, DM], F32, kind="ExternalInput")
    out = nc.dram_tensor("out", [L, DM], F32, kind="ExternalOutput")

    with tile.TileContext(nc) as tc:
        build_program(
            tc, x_d, x_nat, w_in, convd, convb, silub, w_xp, w_dt, dtb,
            a_sc, d_vec, w_out, eye, eyen, eyej, ln2w, ln2b, out,
        )
    nc.compile()
    return nc


def build_program(tc, x_d, x_nat, w_in, convd, convb, silub, w_xp, w_dt, dtb,
                  a_sc, d_vec, w_out, eye, eyen, eyej, ln2w, ln2b, out):
    nc = tc.nc
    from contextlib import ExitStack

    ctx = ExitStack()
    with ctx:
        wp = ctx.enter_context(tc.tile_pool(name="wp", bufs=1))
        lnp = ctx.enter_context(tc.tile_pool(name="lnp", bufs=2))
        big = ctx.enter_context(tc.tile_pool(name="big", bufs=1))
        snp = ctx.enter_context(tc.tile_pool(name="snp", bufs=3))
        repp = ctx.enter_context(tc.tile_pool(name="repp", bufs=3))
        dramp = ctx.enter_context(tc.tile_pool(name="dramp", bufs=2, space="DRAM"))
        pp_ctx = tc.tile_pool(name="pp", space="PSUM", bufs=2)
        pp = pp_ctx.__enter__()

        w_in_sb = wp.tile([128, DM // 128, E2], BF16)
        for k in range(DM // 128):
            nc.sync.dma_start(w_in_sb[:, k, :], w_in[k * 128:(k + 1) * 128, :])
        convd_sb = wp.tile([128, D_CONV, NG, 128], BF16)
        for k in range(D_CONV):
            for g in range(NG):
                nc.sync.dma_start(convd_sb[:, k, g, :], convd[k, g])
        w_xp_sb = wp.tile([128, NG, DR + 2 * DS], BF16)
        for g in range(NG):
            nc.sync.dma_start(w_xp_sb[:, g, :], w_xp[g * 128:(g + 1) * 128, :])
        w_dt_sb = wp.tile([DR, DI], BF16)
        nc.sync.dma_start(w_dt_sb, w_dt[:, :])
        w_out_sb = wp.tile([128, NG, DM], BF16)
        for g in range(NG):
            nc.sync.dma_start(w_out_sb[:, g, :], w_out[g * 128:(g + 1) * 128, :])
        a_sb = wp.tile([128, NG, DS], F32)
        for g in range(NG):
            nc.sync.dma_start(a_sb[:, g, :], a_sc[g])
        eye_sb = wp.tile([128, 128], F32)
        nc.sync.dma_start(eye_sb, eye[:, :])
        eyen_sb = wp.tile([128, 128], BF16)
        nc.sync.dma_start(eyen_sb, eyen[:, :])
        eyej_sb = wp.tile([128, 128], BF16)
        nc.sync.dma_start(eyej_sb, eyej[:, :])

        def col_load(name, src):
            t = wp.tile([128, NG], F32, name=name)
            for g in range(NG):
                nc.sync.dma_start(t[:, g:g + 1], src[g:g + 1, :].rearrange("a b -> b a"))
            return t

        convb_sb = col_load("convb_sb", convb)
        silub_sb = col_load("silub_sb", silub)
        dtb_sb = col_load("dtb_sb", dtb)
        d_sb = col_load("d_sb", d_vec)

        ln2w_sb = wp.tile([128, DM], F32)
        nc.sync.dma_start(ln2w_sb, ln2w[0:1, :].to_broadcast([128, DM]))
        ln2b_sb = wp.tile([128, DM], F32)
        nc.sync.dma_start(ln2b_sb, ln2b[0:1, :].to_broadcast([128, DM]))

        eps_sb = wp.tile([128, 1], F32)
        nc.vector.memset(eps_sb, EPS)

        sg_d = dramp.tile([NG, 128, L], BF16, bufs=1)
        bc_d = dramp.tile([2 * DS, L], BF16, bufs=1)

        # ---------------- phase A: LayerNorm1 + transpose ----------------
        # xn_t: [dm-part, k-block, t] bf16, consumed by in_proj
        xn_t = big.tile([128, DM // 128, L], BF16, name="xn_t")
        # batched stats: mean/var for all 16 tiles, then ONE Ln + ONE Exp for
        # rstd (avoids per-tile activation-table ping-pong); x is loaded twice
        # (stats sweep, then normalize sweep)
        mvall = wp.tile([128, 2, NTT], F32, name="mvall")
        nmr = wp.tile([128, NTT], F32, name="nmr")
        for i in range(NTT):
            xt = lnp.tile([128, DM], F32, tag="xt", bufs=4)
            nc.sync.dma_start(xt, x_d[i * 128:(i + 1) * 128, :])
            st = lnp.tile([128, 6], F32, tag="st")
            nc.vector.bn_stats(st, xt)
            nc.vector.bn_aggr(mvall[:, :, i], st)
        nc.scalar.activation(mvall[:, 1, :], mvall[:, 1, :], AF.Ln,
                             bias=eps_sb[:, 0:1])
        nc.scalar.activation(mvall[:, 1, :], mvall[:, 1, :], AF.Exp,
                             scale=-0.5)
        nc.vector.tensor_mul(nmr, mvall[:, 0, :], mvall[:, 1, :])
        nc.vector.tensor_scalar_mul(nmr, nmr, -1.0)
        for i in range(NTT):
            xt = lnp.tile([128, DM], F32, tag="xt", bufs=4)
            nc.sync.dma_start(xt, x_d[i * 128:(i + 1) * 128, :])
            nc.scalar.activation(xt, xt, AF.Identity,
                                 scale=mvall[:, 1, i:i + 1],
                                 bias=nmr[:, i:i + 1])
            for kb in range(DM // 128):
                ps_t = pp.tile([128, 512], F32, tag="ps")
                nc.tensor.transpose(ps_t[:, 0:128], xt[:, kb * 128:(kb + 1) * 128],
                                    eye_sb)
                nc.scalar.copy(xn_t[:, kb, i * 128:(i + 1) * 128],
                               ps_t[:, 0:128])

        # ---------------- phase B: in_proj ----------------
        # xc half into conv staging (3-col zero halo at front)
        xzf = [big.tile([128, L + 3], BF16, name=f"xzf{g}") for g in range(NG)]
        for g in range(NG):
            nc.vector.memset(xzf[g][:, 0:3], 0.0)
            for s in range(NSL):
                pz = pp.tile([128, 512], F32, tag="ps")
                for k in range(DM // 128):
                    nc.tensor.matmul(pz, w_in_sb[:, k, g * 128:(g + 1) * 128],
                                     xn_t[:, k, s * 512:(s + 1) * 512],
                                     start=(k == 0), stop=(k == DM // 128 - 1))
                nc.scalar.copy(xzf[g][:, 3 + s * 512:3 + (s + 1) * 512], pz)

        # z half + SiLU gate -> sg (spilled to DRAM, reloaded at gate)
        for g in range(NG):
            for s in range(NSL):
                pz = pp.tile([128, 512], F32, tag="ps")
                for k in range(DM // 128):
                    nc.tensor.matmul(
                        pz, w_in_sb[:, k, DI + g * 128: DI + (g + 1) * 128],
                        xn_t[:, k, s * 512:(s + 1) * 512],
                        start=(k == 0), stop=(k == DM // 128 - 1))
                sgt = snp.tile([128, 512], BF16, tag="sgt", bufs=2)
                _silu(nc, snp, sgt, pz, silub_sb[:, g:g + 1])
                nc.sync.dma_start(sg_d[g][:, s * 512:(s + 1) * 512], sgt)

        # ---------------- phase C: depthwise causal conv + SiLU ----------
        xc = [big.tile([128, L], BF16, name=f"xc{g}") for g in range(NG)]
        for g in range(NG):
            for s in range(NSL):
                pc = pp.tile([128, 512], F32, tag="ps")
                for k in range(D_CONV):
                    nc.tensor.matmul(pc, convd_sb[:, k, g, :],
                                     xzf[g][:, k + s * 512: k + (s + 1) * 512],
                                     start=(k == 0), stop=(k == D_CONV - 1))
                _silu(nc, snp, xc[g][:, s * 512:(s + 1) * 512], pc,
                      convb_sb[:, g:g + 1])

        # ---------------- phase D: x_proj -> (dt_r, B, C) -----------------
        xdbl = big.tile([64, L], BF16, name="xdbl")
        for s in range(NSL):
            pxd = pp.tile([64, 512], F32, tag="pxd", bufs=1)
            for g in range(NG):
                nc.tensor.matmul(pxd, w_xp_sb[:, g, :],
                                 xc[g][:, s * 512:(s + 1) * 512],
                                 start=(g == 0), stop=(g == NG - 1))
            nc.scalar.copy(xdbl[:, s * 512:(s + 1) * 512], pxd)
        nc.sync.dma_start(bc_d, xdbl[DR:DR + 2 * DS, :])

        # dt = softplus(dt_proj @ dt_r + bias); p = dt * xc
        # dt_c[0..3] reuse xn_t's space, dt_c[4..7] reuse w_in_sb (both dead
        # after in_proj); p_c reuses the conv staging xzf (dead after conv)
        dt_c = [xn_t[:, g, :] if g < 4 else w_in_sb[:, g - 4, :]
                for g in range(NG)]
        p_c = [xzf[g][:, 0:L] for g in range(NG)]
        for g in range(NG):
            for s in range(NSL):
                sl = slice(s * 512, (s + 1) * 512)
                pdt = pp.tile([128, 512], F32, tag="ps")
                nc.tensor.matmul(pdt, w_dt_sb[:, g * 128:(g + 1) * 128],
                                 xdbl[0:DR, sl],
                                 start=True, stop=True)
                nc.scalar.activation(dt_c[g][:, sl], pdt, AF.Exp,
                                     bias=dtb_sb[:, g:g + 1])
        for g in range(NG):
            nc.scalar.activation(dt_c[g], dt_c[g], AF.Ln, bias=1.0)
            nc.vector.tensor_mul(p_c[g], dt_c[g], xc[g])

        # ---------------- phase E: selective scan ------------------------
        # waves of 3 groups share each B/C broadcast; time in halves of 1024.
        # py uses 6 PSUM banks, leaving 2 for out_proj so each half's
        # out_proj + collective piece overlaps the next half's compute.
        pp_ctx.__exit__(None, None, None)
        HW = L // 2
        WAVES = [(0, 1, 2), (3, 4, 5), (6, 7)]
        yf = [xc[g] for g in range(NG)]
        ypart_h = [dramp.tile([HW, DM], BF16, bufs=1, name=f"yph{h}")
                   for h in range(2)]
        gbuf_h = [dramp.tile([2, HW, DM], BF16, bufs=1, name=f"gbh{h}")
                  for h in range(2)]
        ppy_ctx = tc.tile_pool(name="ppy", space="PSUM", bufs=1)
        ppy = ppy_ctx.__enter__()
        pp2_ctx = tc.tile_pool(name="pp2", space="PSUM", bufs=2)
        pp = pp2_ctx.__enter__()
        hcars = [snp.tile([128, 4, DS], BF16, tag="hcar", bufs=3,
                          name=f"hcar{w}") for w in range(3)]

        def out_proj_half(half):
            hs = slice(half * HW, (half + 1) * HW)
            for tb in range(8):
                t0 = half * HW + tb * 128
                po = pp.tile([128, DM], F32, tag="ps")
                for g in range(NG):
                    nc.tensor.matmul(po, yf[g][:, t0:t0 + 128],
                                     w_out_sb[:, g, :],
                                     start=(g == 0), stop=(g == NG - 1))
                ot = lnp.tile([128, DM], BF16, tag="ot")
                nc.scalar.copy(ot, po)
                nc.sync.dma_start(ypart_h[half][tb * 128:(tb + 1) * 128, :], ot)
            nc.gpsimd.collective_compute(
                "AllGather", OP.bypass, replica_groups=REPLICA_GROUPS,
                ins=[ypart_h[half].opt()], outs=[gbuf_h[half].opt()],
            )

        for half in (0, 1):
            hs = slice(half * HW, (half + 1) * HW)
            for wv, wave in enumerate(WAVES):
                hcar = hcars[wv]
                py = [ppy.tile([128, HW], F32, tag=f"py{j}", bufs=1,
                               name=f"py{j}") for j in range(len(wave))]
                for j, g in enumerate(wave):
                    yd = snp.tile([128, HW], BF16, tag="yd", bufs=2)
                    nc.vector.tensor_scalar_mul(yd, xc[g][:, hs],
                                                d_sb[:, g:g + 1])
                    for s in range(2):
                        nc.tensor.matmul(py[j][:, s * 512:(s + 1) * 512],
                                         eyen_sb,
                                         yd[:, s * 512:(s + 1) * 512],
                                         start=True, stop=False)
                for n in range(DS):
                    brep = repp.tile([128, HW], BF16, tag="brep", bufs=3)
                    nc.sync.dma_start(
                        brep, bc_d[n:n + 1, hs].to_broadcast([128, HW]))
                    crep = repp.tile([128, HW], BF16, tag="crep", bufs=3)
                    nc.sync.dma_start(
                        crep,
                        bc_d[DS + n:DS + n + 1, hs].to_broadcast([128, HW]))
                    for j, g in enumerate(wave):
                        da = snp.tile([128, HW], BF16, tag="da", bufs=3)
                        nc.scalar.activation(da, dt_c[g][:, hs], AF.Exp,
                                             scale=a_sb[:, g, n:n + 1])
                        bt = snp.tile([128, HW], BF16, tag="bt",
                                      name="btd", bufs=3)
                        nc.vector.tensor_mul(bt, p_c[g][:, hs], brep)
                        h = snp.tile([128, HW], BF16, tag="h", bufs=2)
                        init = 0.0 if half == 0 else hcar[:, j, n:n + 1]
                        nc.vector.tensor_tensor_scan(h, da, bt,
                                                     initial=init,
                                                     op0=OP.mult,
                                                     op1=OP.add)
                        if half == 0:
                            nc.vector.tensor_copy(hcar[:, j, n:n + 1],
                                                  h[:, HW - 1:HW])
                        yn = snp.tile([128, HW], BF16, tag="yn", bufs=2)
                        nc.vector.tensor_mul(yn, h, crep)
                        for s in range(2):
                            nc.tensor.matmul(
                                py[j][:, s * 512:(s + 1) * 512], eyen_sb,
                                yn[:, s * 512:(s + 1) * 512],
                                start=False, stop=(n == DS - 1))
                # gate with silu(z); write into xc[g]'s tile (xc dead now)
                for j, g in enumerate(wave):
                    sgl = repp.tile([128, HW], BF16, tag="sgl", bufs=2)
                    nc.sync.dma_start(sgl, sg_d[g][:, hs])
                    nc.vector.tensor_mul(yf[g][:, hs], py[j], sgl)
            out_proj_half(half)

        # ---------------- final residual + LayerNorm ----------------------
        mvall2 = wp.tile([128, 2, NTT], F32, name="mvall2")
        nmr2 = wp.tile([128, NTT], F32, name="nmr2")

        def sum3(i):
            # natural-time tile i: dir0 rows i*128.. (gbuf piece by row range)
            r0 = i * 128
            ph0 = 0 if r0 < HW else 1
            s0 = lnp.tile([128, DM], BF16, tag="s0")
            nc.sync.dma_start(
                s0, gbuf_h[ph0][0, r0 - ph0 * HW: r0 - ph0 * HW + 128, :])
            # direction-1 partial is in flipped local order
            lr = L - (i + 1) * 128
            ph1 = 0 if lr < HW else 1
            s1 = lnp.tile([128, DM], BF16, tag="s1")
            nc.sync.dma_start(
                s1, gbuf_h[ph1][1, lr - ph1 * HW: lr - ph1 * HW + 128, :])
            xt2 = lnp.tile([128, DM], BF16, tag="xt2")
            nc.sync.dma_start(xt2, x_nat[i * 128:(i + 1) * 128, :])
            pj = pp.tile([128, DM], F32, tag="ps")
            nc.tensor.matmul(pj, eyej_sb, s1, start=True, stop=False)
            nc.tensor.matmul(pj, eyen_sb, s0, start=False, stop=False)
            nc.tensor.matmul(pj, eyen_sb, xt2, start=False, stop=True)
            return pj

        # sweep 1: stats for every tile (batched rstd afterwards)
        for i in range(NTT):
            pj = sum3(i)
            st = lnp.tile([128, 6], F32, tag="st2")
            nc.vector.bn_stats(st, pj)
            nc.vector.bn_aggr(mvall2[:, :, i], st)
        nc.scalar.activation(mvall2[:, 1, :], mvall2[:, 1, :], AF.Ln,
                             bias=eps_sb[:, 0:1])
        nc.scalar.activation(mvall2[:, 1, :], mvall2[:, 1, :], AF.Exp,
                             scale=-0.5)
        nc.vector.tensor_mul(nmr2, mvall2[:, 0, :], mvall2[:, 1, :])
        nc.vector.tensor_scalar_mul(nmr2, nmr2, -1.0)
        # sweep 2: rebuild the sum (3 cheap matmuls) and normalize from PSUM
        for i in range(NTT):
            pj = sum3(i)
            s = lnp.tile([128, DM], F32, tag="ssum")
            nc.scalar.activation(s, pj, AF.Identity,
                                 scale=mvall2[:, 1, i:i + 1],
                                 bias=nmr2[:, i:i + 1])
            nc.vector.tensor_mul(s, s, ln2w_sb)
            nc.vector.tensor_add(s, s, ln2b_sb)
            nc.sync.dma_start(out[i * 128:(i + 1) * 128, :], s)
        pp2_ctx.__exit__(None, None, None)
        ppy_ctx.__exit__(None, None, None)


# ---------------- host side ----------------

def make_core_inputs(inputs):
    """Build the 8 per-core input dicts from the full problem inputs."""
    x = np.ascontiguousarray(inputs["x"], dtype=np.float32)        # [B, L, DM]
    ln1_w = inputs["ln1_w"].astype(np.float32)
    ln1_b = inputs["ln1_b"].astype(np.float32)
    in_w = inputs["in_proj_w"].astype(np.float32)                  # [2, 2DI, DM]
    conv_w = inputs["conv_w"].astype(np.float32)                   # [2, DI, 4]
    conv_b = inputs["conv_b"].astype(np.float32)                   # [2, DI]
    xp_w = inputs["x_proj_w"].astype(np.float32)                   # [2, 64, DI]
    dt_w = inputs["dt_proj_w"].astype(np.float32)                  # [2, DI, DR]
    dt_b = inputs["dt_proj_b"].astype(np.float32)                  # [2, DI]
    a_log = inputs["A_log"].astype(np.float32)                     # [2, DI, DS]
    d_par = inputs["D_param"].astype(np.float32)                   # [2, DI]
    out_w = inputs["out_proj_w"].astype(np.float32)                # [2, DM, DI]
    ln2_w = inputs["ln2_w"].astype(np.float32)
    ln2_b = inputs["ln2_b"].astype(np.float32)

    bf = ml_dtypes.bfloat16
    eye = np.eye(128, dtype=np.float32)
    per_dir = []
    for d in range(2):
        w = in_w[d]                                   # [2DI, DM]
        w_in_T = np.ascontiguousarray((w * ln1_w[None, :]).T)      # [DM, 2DI]
        v = w @ ln1_b                                  # [2DI]
        csum = conv_w[d].sum(axis=1)                   # [DI]
        convb_adj = conv_b[d] + csum * v[:DI]
        silub_adj = v[DI:]
        convd = np.zeros((D_CONV, NG, 128, 128), np.float32)
        for k in range(D_CONV):
            for g in range(NG):
                np.fill_diagonal(convd[k, g], conv_w[d, g * 128:(g + 1) * 128, k])
        a_neg = -np.exp(a_log[d])                      # [DI, DS]
        per_dir.append(dict(
            w_in=w_in_T.astype(bf),
            convd=convd.astype(bf),
            convb=convb_adj.reshape(NG, 128),
            silub=silub_adj.reshape(NG, 128),
            w_xp=np.ascontiguousarray(xp_w[d].T).astype(bf),       # [DI, 64]
            w_dt=np.ascontiguousarray(dt_w[d].T).astype(bf),       # [DR, DI]
            dtb=dt_b[d].reshape(NG, 128),
            a_sc=np.ascontiguousarray(a_neg.reshape(NG, 128, DS)),
            d_vec=d_par[d].reshape(NG, 128),
            w_out=np.ascontiguousarray(out_w[d].T).astype(bf),     # [DI, DM]
        ))

    in_maps = []
    for core in range(N_CORES):
        d, b = core // 4, core % 4
        xb = x[b]
        m = dict(per_dir[d])
        m["x_d"] = xb if d == 0 else np.ascontiguousarray(xb[::-1])
        m["x_nat"] = xb.astype(bf)
        m["eye"] = eye
        m["eyen"] = eye.astype(bf)
        m["eyej"] = eye[::-1].copy().astype(bf)
        m["ln2w"] = ln2_w.reshape(1, DM)
        m["ln2b"] = ln2_b.reshape(1, DM)
        in_maps.append(m)
    return in_maps


_NC = None


def _get_module():
    global _NC
    if _NC is None:
        _NC = build_module()
    return _NC


def kernel(**inputs) -> np.ndarray:
    nc = _get_module()
    in_maps = make_core_inputs(inputs)
    res = run_bass_kernel_spmd(nc, in_maps, core_ids=list(range(N_CORES)))
    outs = [res.results[b]["out"] for b in range(B_SZ)]
    return np.stack(outs, axis=0)


if __name__ == "__main__":
    nc = build_module()
    print("module built ok")


# revision 19
# speedup vs baseline: 1.2398x; 1.0201x over previous
"""Bidirectional Mamba layer on 8 Trainium2 NeuronCores (Bass/Tile).

Sharding: 8 cores = 2 directions x 4 batch samples. Each core runs the full
Mamba block for its (direction, sample) pair; a pairwise AllGather combines
the two directions, and every core (redundantly, SPMD-symmetric) applies the
final residual + LayerNorm.

Layout: channels on partitions, time on the free axis, full L=2048 per op
(monolith — no time chunking, so the selective scan needs no state carry).
All matmuls run bf16 (single PE pass); elementwise work is bf16 for the DVE
2x mode. The per-(group, state) inner loop is:
  Act:  da = exp(dt * A[g,n])          (scalar engine)
  DVE/GPS: bt = p * B_n                (broadcast multiply)
  DVE:  h = tensor_tensor_scan(da, bt) (the irreducible 2 cyc/elem scan)
  DVE:  yn = h * C_n
  PE:   py += I @ yn                   (PSUM accumulation over states)
"""

import ml_dtypes
import numpy as np

import concourse.bass as bass
import concourse.bacc as bacc
import concourse.tile as tile
from concourse import mybir
from concourse.bass_utils import run_bass_kernel_spmd

# ---- problem shapes (hardcoded per contract) ----
B_SZ, L, DM = 4, 2048, 512
D_CONV, DS, DR = 4, 16, 32
DI = 1024                  # d_inner
E2 = 2 * DI                # in_proj rows
NG = DI // 128             # 8 channel blocks
NTT = L // 128             # 16 time tiles of 128
NSL = L // 512             # 4 psum-width slices
EPS = 1e-5
F32 = mybir.dt.float32
BF16 = mybir.dt.bfloat16
AF = mybir.ActivationFunctionType
OP = mybir.AluOpType

N_CORES = 8

# CoreSim does not implement Silu; tests flip this to use sigmoid+mult
USE_SILU = True
REPLICA_GROUPS = [[0, 4], [1, 5], [2, 6], [3, 7]]

# groups whose bt-/yn-multiply runs on GPSIMD instead of DVE.
# Empirically GPSIMD elementwise is net-negative here: it contends for the
# DVE's SBUF port and slows 2x-mode DVE ops more than it absorbs.
GPS_GS = ()
GPS_YS = ()


def _silu(nc, pool, out_tile, psum, bias=None):
    """out = silu(psum [+ bias]); Silu on HW, sigmoid+mult fallback for sim."""
    kw = {} if bias is None else {"bias": bias}
    if USE_SILU:
        nc.scalar.activation(out_tile, psum, AF.Silu, **kw)
    else:
        zb = pool.tile(list(out_tile.shape), F32, tag="_silu_zb", name="zb",
                       bufs=1)
        nc.scalar.activation(zb, psum, AF.Identity, **kw)
        nc.scalar.activation(out_tile, psum, AF.Sigmoid, **kw)
        nc.vector.tensor_mul(out_tile, zb, out_tile)


def build_module():
    nc = bacc.Bacc(
        "TRN2", target_bir_lowering=False, debug=False, num_devices=N_CORES
    )

    # ---------------- I/O ----------------
    x_d = nc.dram_tensor("x_d", [L, DM], F32, kind="ExternalInput")
    x_nat = nc.dram_tensor("x_nat", [L, DM], BF16, kind="ExternalInput")
    w_in = nc.dram_tensor("w_in", [DM, E2], BF16, kind="ExternalInput")
    convd = nc.dram_tensor("convd", [D_CONV, NG, 128, 128], BF16, kind="ExternalInput")
    convb = nc.dram_tensor("convb", [NG, 128], F32, kind="ExternalInput")
    silub = nc.dram_tensor("silub", [NG, 128], F32, kind="ExternalInput")
    w_xp = nc.dram_tensor("w_xp", [DI, DR + 2 * DS], BF16, kind="ExternalInput")
    w_dt = nc.dram_tensor("w_dt", [DR, DI], BF16, kind="ExternalInput")
    dtb = nc.dram_tensor("dtb", [NG, 128], F32, kind="ExternalInput")
    a_sc = nc.dram_tensor("a_sc", [NG, 128, DS], F32, kind="ExternalInput")
    d_vec = nc.dram_tensor("d_vec", [NG, 128], F32, kind="ExternalInput")
    w_out = nc.dram_tensor("w_out", [DI, DM], BF16, kind="ExternalInput")
    eye = nc.dram_tensor("eye", [128, 128], F32, kind="ExternalInput")
    eyen = nc.dram_tensor("eyen", [128, 128], BF16, kind="ExternalInput")
    eyej = nc.dram_tensor("eyej", [128, 128], BF16, kind="ExternalInput")
    ln2w = nc.dram_tensor("ln2w", [1, DM], F32, kind="ExternalInput")
    ln2b = nc.dram_tensor("ln2b", [1, DM], F32, kind="ExternalInput")
    out = nc.dram_tensor("out", [L, DM], F32, kind="ExternalOutput")

    with tile.TileContext(nc) as tc:
        build_program(
            tc, x_d, x_nat, w_in, convd, convb, silub, w_xp, w_dt, dtb,
            a_sc, d_vec, w_out, eye, eyen, eyej, ln2w, ln2b, out,
        )
    nc.compile()
    return nc


def build_program(tc, x_d, x_nat, w_in, convd, convb, silub, w_xp, w_dt, dtb,
                  a_sc, d_vec, w_out, eye, eyen, eyej, ln2w, ln2b, out):
    nc = tc.nc
    from contextlib import ExitStack

    ctx = ExitStack()
    with ctx:
        wp = ctx.enter_context(tc.tile_pool(name="wp", bufs=1))
        lnp = ctx.enter_context(tc.tile_pool(name="lnp", bufs=2))
        big = ctx.enter_context(tc.tile_pool(name="big", bufs=1))
        snp = ctx.enter_context(tc.tile_pool(name="snp", bufs=3))
        repp = ctx.enter_context(tc.tile_pool(name="repp", bufs=3))
        dramp = ctx.enter_context(tc.tile_pool(name="dramp", bufs=2, space="DRAM"))
        pp_ctx = tc.tile_pool(name="pp", space="PSUM", bufs=2)
        pp = pp_ctx.__enter__()

        # x-load + LN1 stats first so its DMAs aren't queued behind weights
        eps_sb = wp.tile([128, 1], F32)
        nc.vector.memset(eps_sb, EPS)
        mvall = wp.tile([128, 2, NTT], F32, name="mvall")
        nmr = wp.tile([128, NTT], F32, name="nmr")
        for i in range(NTT):
            xt = lnp.tile([128, DM], F32, tag="xt", bufs=4)
            nc.sync.dma_start(xt, x_d[i * 128:(i + 1) * 128, :])
            st = lnp.tile([128, 6], F32, tag="st")
            nc.vector.bn_stats(st, xt)
            nc.vector.bn_aggr(mvall[:, :, i], st)
        nc.scalar.activation(mvall[:, 1, :], mvall[:, 1, :], AF.Ln,
                             bias=eps_sb[:, 0:1])
        nc.scalar.activation(mvall[:, 1, :], mvall[:, 1, :], AF.Exp,
                             scale=-0.5)
        nc.vector.tensor_mul(nmr, mvall[:, 0, :], mvall[:, 1, :])
        nc.vector.tensor_scalar_mul(nmr, nmr, -1.0)

        w_in_sb = wp.tile([128, DM // 128, E2], BF16)
        for k in range(DM // 128):
            nc.sync.dma_start(w_in_sb[:, k, :], w_in[k * 128:(k + 1) * 128, :])
        convd_sb = wp.tile([128, D_CONV, NG, 128], BF16)
        for k in range(D_CONV):
            for g in range(NG):
                nc.sync.dma_start(convd_sb[:, k, g, :], convd[k, g])
        w_xp_sb = wp.tile([128, NG, DR + 2 * DS], BF16)
        for g in range(NG):
            nc.sync.dma_start(w_xp_sb[:, g, :], w_xp[g * 128:(g + 1) * 128, :])
        w_dt_sb = wp.tile([DR, DI], BF16)
        nc.sync.dma_start(w_dt_sb, w_dt[:, :])
        w_out_sb = wp.tile([128, NG, DM], BF16)
        for g in range(NG):
            nc.sync.dma_start(w_out_sb[:, g, :], w_out[g * 128:(g + 1) * 128, :])
        a_sb = wp.tile([128, NG, DS], F32)
        for g in range(NG):
            nc.sync.dma_start(a_sb[:, g, :], a_sc[g])
        eye_sb = wp.tile([128, 128], F32)
        nc.sync.dma_start(eye_sb, eye[:, :])
        eyen_sb = wp.tile([128, 128], BF16)
        nc.sync.dma_start(eyen_sb, eyen[:, :])
        eyej_sb = wp.tile([128, 128], BF16)
        nc.sync.dma_start(eyej_sb, eyej[:, :])

        def col_load(name, src):
            t = wp.tile([128, NG], F32, name=name)
            for g in range(NG):
                nc.sync.dma_start(t[:, g:g + 1], src[g:g + 1, :].rearrange("a b -> b a"))
            return t

        convb_sb = col_load("convb_sb", convb)
        silub_sb = col_load("silub_sb", silub)
        dtb_sb = col_load("dtb_sb", dtb)
        d_sb = col_load("d_sb", d_vec)

        ln2w_sb = wp.tile([128, DM], F32)
        nc.sync.dma_start(ln2w_sb, ln2w[0:1, :].to_broadcast([128, DM]))
        ln2b_sb = wp.tile([128, DM], F32)
        nc.sync.dma_start(ln2b_sb, ln2b[0:1, :].to_broadcast([128, DM]))

        sg_d = dramp.tile([NG, 128, L], BF16, bufs=1)
        bc_d = dramp.tile([2 * DS, L], BF16, bufs=1)

        # ---------------- phase A: LayerNorm1 + transpose ----------------
        # xn_t: [dm-part, k-block, t] bf16, consumed by in_proj
        xn_t = big.tile([128, DM // 128, L], BF16, name="xn_t")
        # normalize sweep (stats were computed up top, before weight loads)
        for i in range(NTT):
            xt = lnp.tile([128, DM], F32, tag="xt", bufs=4)
            nc.sync.dma_start(xt, x_d[i * 128:(i + 1) * 128, :])
            nc.scalar.activation(xt, xt, AF.Identity,
                                 scale=mvall[:, 1, i:i + 1],
                                 bias=nmr[:, i:i + 1])
            for kb in range(DM // 128):
                ps_t = pp.tile([128, 512], F32, tag="ps")
                nc.tensor.transpose(ps_t[:, 0:128], xt[:, kb * 128:(kb + 1) * 128],
                                    eye_sb)
                nc.vector.tensor_copy(xn_t[:, kb, i * 128:(i + 1) * 128],
                                      ps_t[:, 0:128])

        # ---------------- phase B: in_proj ----------------
        # xc half into conv staging (3-col zero halo at front)
        xzf = [big.tile([128, L + 3], BF16, name=f"xzf{g}") for g in range(NG)]
        for g in range(NG):
            nc.vector.memset(xzf[g][:, 0:3], 0.0)
            for s in range(NSL):
                pz = pp.tile([128, 512], F32, tag="ps")
                for k in range(DM // 128):
                    nc.tensor.matmul(pz, w_in_sb[:, k, g * 128:(g + 1) * 128],
                                     xn_t[:, k, s * 512:(s + 1) * 512],
                                     start=(k == 0), stop=(k == DM // 128 - 1))
                nc.vector.tensor_copy(xzf[g][:, 3 + s * 512:3 + (s + 1) * 512], pz)

        # z half + SiLU gate -> sg (spilled to DRAM, reloaded at gate)
        for g in range(NG):
            for s in range(NSL):
                pz = pp.tile([128, 512], F32, tag="ps")
                for k in range(DM // 128):
                    nc.tensor.matmul(
                        pz, w_in_sb[:, k, DI + g * 128: DI + (g + 1) * 128],
                        xn_t[:, k, s * 512:(s + 1) * 512],
                        start=(k == 0), stop=(k == DM // 128 - 1))
                sgt = snp.tile([128, 512], BF16, tag="sgt", bufs=2)
                _silu(nc, snp, sgt, pz, silub_sb[:, g:g + 1])
                nc.sync.dma_start(sg_d[g][:, s * 512:(s + 1) * 512], sgt)

        # ---------------- phase C: depthwise causal conv + SiLU ----------
        xc = [big.tile([128, L], BF16, name=f"xc{g}") for g in range(NG)]
        for g in range(NG):
            for s in range(NSL):
                pc = pp.tile([128, 512], F32, tag="ps")
                for k in range(D_CONV):
                    nc.tensor.matmul(pc, convd_sb[:, k, g, :],
                                     xzf[g][:, k + s * 512: k + (s + 1) * 512],
                                     start=(k == 0), stop=(k == D_CONV - 1))
                _silu(nc, snp, xc[g][:, s * 512:(s + 1) * 512], pc,
                      convb_sb[:, g:g + 1])

        # ---------------- phase D: x_proj -> (dt_r, B, C) -----------------
        xdbl = big.tile([64, L], BF16, name="xdbl")
        for s in range(NSL):
            pxd = pp.tile([64, 512], F32, tag="pxd", bufs=1)
            for g in range(NG):
                nc.tensor.matmul(pxd, w_xp_sb[:, g, :],
                                 xc[g][:, s * 512:(s + 1) * 512],
                                 start=(g == 0), stop=(g == NG - 1))
            nc.vector.tensor_copy(xdbl[:, s * 512:(s + 1) * 512], pxd)
        nc.sync.dma_start(bc_d, xdbl[DR:DR + 2 * DS, :])

        # dt = softplus(dt_proj @ dt_r + bias); p = dt * xc
        # dt_c[0..3] reuse xn_t's space, dt_c[4..7] reuse w_in_sb (both dead
        # after in_proj); p_c reuses the conv staging xzf (dead after conv)
        dt_c = [xn_t[:, g, :] if g < 4 else w_in_sb[:, g - 4, :]
                for g in range(NG)]
        p_c = [xzf[g][:, 0:L] for g in range(NG)]
        for g in range(NG):
            for s in range(NSL):
                sl = slice(s * 512, (s + 1) * 512)
                pdt = pp.tile([128, 512], F32, tag="ps")
                nc.tensor.matmul(pdt, w_dt_sb[:, g * 128:(g + 1) * 128],
                                 xdbl[0:DR, sl],
                                 start=True, stop=True)
                nc.scalar.activation(dt_c[g][:, sl], pdt, AF.Exp,
                                     bias=dtb_sb[:, g:g + 1])
        for g in range(NG):
            nc.scalar.activation(dt_c[g], dt_c[g], AF.Ln, bias=1.0)
            nc.vector.tensor_mul(p_c[g], dt_c[g], xc[g])

        # ---------------- phase E: selective scan ------------------------
        # waves of 3 groups share each B/C broadcast; time in halves of 1024.
        # py uses 6 PSUM banks, leaving 2 for out_proj so each half's
        # out_proj + collective piece overlaps the next half's compute.
        pp_ctx.__exit__(None, None, None)
        HW = L // 2
        WAVES = [(0, 1, 2), (3, 4, 5), (6, 7)]
        yf = [xc[g] for g in range(NG)]
        ypart_h = [dramp.tile([HW, DM], BF16, bufs=1, name=f"yph{h}")
                   for h in range(2)]
        gbuf_h = [dramp.tile([2, HW, DM], BF16, bufs=1, name=f"gbh{h}")
                  for h in range(2)]
        ppy_ctx = tc.tile_pool(name="ppy", space="PSUM", bufs=1)
        ppy = ppy_ctx.__enter__()
        pp2_ctx = tc.tile_pool(name="pp2", space="PSUM", bufs=2)
        pp = pp2_ctx.__enter__()
        hcars = [snp.tile([128, 4, DS], BF16, tag="hcar", bufs=3,
                          name=f"hcar{w}") for w in range(3)]

        def out_proj_half(half):
            hs = slice(half * HW, (half + 1) * HW)
            for tb in range(8):
                t0 = half * HW + tb * 128
                po = pp.tile([128, DM], F32, tag="ps")
                for g in range(NG):
                    nc.tensor.matmul(po, yf[g][:, t0:t0 + 128],
                                     w_out_sb[:, g, :],
                                     start=(g == 0), stop=(g == NG - 1))
                ot = lnp.tile([128, DM], BF16, tag="ot")
                nc.scalar.copy(ot, po)
                nc.sync.dma_start(ypart_h[half][tb * 128:(tb + 1) * 128, :], ot)
            nc.gpsimd.collective_compute(
                "AllGather", OP.bypass, replica_groups=REPLICA_GROUPS,
                ins=[ypart_h[half].opt()], outs=[gbuf_h[half].opt()],
            )

        for half in (0, 1):
            hs = slice(half * HW, (half + 1) * HW)
            for wv, wave in enumerate(WAVES):
                hcar = hcars[wv]
                py = [ppy.tile([128, HW], F32, tag=f"py{j}", bufs=1,
                               name=f"py{j}") for j in range(len(wave))]
                for j, g in enumerate(wave):
                    yd = snp.tile([128, HW], BF16, tag="yd", bufs=2)
                    nc.vector.tensor_scalar_mul(yd, xc[g][:, hs],
                                                d_sb[:, g:g + 1])
                    for s in range(2):
                        nc.tensor.matmul(py[j][:, s * 512:(s + 1) * 512],
                                         eyen_sb,
                                         yd[:, s * 512:(s + 1) * 512],
                                         start=True, stop=False)
                for n in range(DS):
                    brep = repp.tile([128, HW], BF16, tag="brep", bufs=3)
                    nc.sync.dma_start(
                        brep, bc_d[n:n + 1, hs].to_broadcast([128, HW]))
                    crep = repp.tile([128, HW], BF16, tag="crep", bufs=3)
                    nc.sync.dma_start(
                        crep,
                        bc_d[DS + n:DS + n + 1, hs].to_broadcast([128, HW]))
                    for j, g in enumerate(wave):
                        da = snp.tile([128, HW], BF16, tag="da", bufs=3)
                        nc.scalar.activation(da, dt_c[g][:, hs], AF.Exp,
                                             scale=a_sb[:, g, n:n + 1])
                        bt = snp.tile([128, HW], BF16, tag="bt",
                                      name="btd", bufs=3)
                        nc.vector.tensor_mul(bt, p_c[g][:, hs], brep)
                        h = snp.tile([128, HW], BF16, tag="h", bufs=2)
                        init = 0.0 if half == 0 else hcar[:, j, n:n + 1]
                        nc.vector.tensor_tensor_scan(h, da, bt,
                                                     initial=init,
                                                     op0=OP.mult,
                                                     op1=OP.add)
                        if half == 0:
                            nc.vector.tensor_copy(hcar[:, j, n:n + 1],
                                                  h[:, HW - 1:HW])
                        yn = snp.tile([128, HW], BF16, tag="yn", bufs=2)
                        nc.vector.tensor_mul(yn, h, crep)
                        for s in range(2):
                            nc.tensor.matmul(
                                py[j][:, s * 512:(s + 1) * 512], eyen_sb,
                                yn[:, s * 512:(s + 1) * 512],
                                start=False, stop=(n == DS - 1))
                # gate with silu(z); write into xc[g]'s tile (xc dead now)
                for j, g in enumerate(wave):
                    sgl = repp.tile([128, HW], BF16, tag="sgl", bufs=2)
                    nc.sync.dma_start(sgl, sg_d[g][:, hs])
                    nc.vector.tensor_mul(yf[g][:, hs], py[j], sgl)
            out_proj_half(half)

        # ---------------- final residual + LayerNorm ----------------------
        mvall2 = wp.tile([128, 2, NTT], F32, name="mvall2")
        nmr2 = wp.tile([128, NTT], F32, name="nmr2")

        def sum3(i):
            # natural-time tile i: dir0 rows i*128.. (gbuf piece by row range)
            r0 = i * 128
            ph0 = 0 if r0 < HW else 1
            s0 = lnp.tile([128, DM], BF16, tag="s0")
            nc.sync.dma_start(
                s0, gbuf_h[ph0][0, r0 - ph0 * HW: r0 - ph0 * HW + 128, :])
            # direction-1 partial is in flipped local order
            lr = L - (i + 1) * 128
            ph1 = 0 if lr < HW else 1
            s1 = lnp.tile([128, DM], BF16, tag="s1")
            nc.sync.dma_start(
                s1, gbuf_h[ph1][1, lr - ph1 * HW: lr - ph1 * HW + 128, :])
            xt2 = lnp.tile([128, DM], BF16, tag="xt2")
            nc.sync.dma_start(xt2, x_nat[i * 128:(i + 1) * 128, :])
            pj = pp.tile([128, DM], F32, tag="ps")
            nc.tensor.matmul(pj, eyej_sb, s1, start=True, stop=False)
            nc.tensor.matmul(pj, eyen_sb, s0, start=False, stop=False)
            nc.tensor.matmul(pj, eyen_sb, xt2, start=False, stop=True)
            return pj

        # sweep 1: stats for every tile (batched rstd afterwards)
        for i in range(NTT):
            pj = sum3(i)
            st = lnp.tile([128, 6], F32, tag="st2")
            nc.vector.bn_stats(st, pj)
            nc.vector.bn_aggr(mvall2[:, :, i], st)
        nc.scalar.activation(mvall2[:, 1, :], mvall2[:, 1, :], AF.Ln,
                             bias=eps_sb[:, 0:1])
        nc.scalar.activation(mvall2[:, 1, :], mvall2[:, 1, :], AF.Exp,
                             scale=-0.5)
        nc.vector.tensor_mul(nmr2, mvall2[:, 0, :], mvall2[:, 1, :])
        nc.vector.tensor_scalar_mul(nmr2, nmr2, -1.0)
        # sweep 2: rebuild the sum (3 cheap matmuls) and normalize from PSUM
        for i in range(NTT):
            pj = sum3(i)
            s = lnp.tile([128, DM], F32, tag="ssum")
            nc.scalar.activation(s, pj, AF.Identity,
                                 scale=mvall2[:, 1, i:i + 1],
                                 bias=nmr2[:, i:i + 1])
            nc.vector.tensor_mul(s, s, ln2w_sb)
            nc.vector.tensor_add(s, s, ln2b_sb)
            nc.sync.dma_start(out[i * 128:(i + 1) * 128, :], s)
        pp2_ctx.__exit__(None, None, None)
        ppy_ctx.__exit__(None, None, None)


# ---------------- host side ----------------

def make_core_inputs(inputs):
    """Build the 8 per-core input dicts from the full problem inputs."""
    x = np.ascontiguousarray(inputs["x"], dtype=np.float32)        # [B, L, DM]
    ln1_w = inputs["ln1_w"].astype(np.float32)
    ln1_b = inputs["ln1_b"].astype(np.float32)
    in_w = inputs["in_proj_w"].astype(np.float32)                  # [2, 2DI, DM]
    conv_w = inputs["conv_w"].astype(np.float32)                   # [2, DI, 4]
    conv_b = inputs["conv_b"].astype(np.float32)                   # [2, DI]
    xp_w = inputs["x_proj_w"].astype(np.float32)                   # [2, 64, DI]
    dt_w = inputs["dt_proj_w"].astype(np.float32)                  # [2, DI, DR]
    dt_b = inputs["dt_proj_b"].astype(np.float32)                  # [2, DI]
    a_log = inputs["A_log"].astype(np.float32)                     # [2, DI, DS]
    d_par = inputs["D_param"].astype(np.float32)                   # [2, DI]
    out_w = inputs["out_proj_w"].astype(np.float32)                # [2, DM, DI]
    ln2_w = inputs["ln2_w"].astype(np.float32)
    ln2_b = inputs["ln2_b"].astype(np.float32)

    bf = ml_dtypes.bfloat16
    eye = np.eye(128, dtype=np.float32)
    per_dir = []
    for d in range(2):
        w = in_w[d]                                   # [2DI, DM]
        w_in_T = np.ascontiguousarray((w * ln1_w[None, :]).T)      # [DM, 2DI]
        v = w @ ln1_b                                  # [2DI]
        csum = conv_w[d].sum(axis=1)                   # [DI]
        convb_adj = conv_b[d] + csum * v[:DI]
        silub_adj = v[DI:]
        convd = np.zeros((D_CONV, NG, 128, 128), np.float32)
        for k in range(D_CONV):
            for g in range(NG):
                np.fill_diagonal(convd[k, g], conv_w[d, g * 128:(g + 1) * 128, k])
        a_neg = -np.exp(a_log[d])                      # [DI, DS]
        per_dir.append(dict(
            w_in=w_in_T.astype(bf),
            convd=convd.astype(bf),
            convb=convb_adj.reshape(NG, 128),
            silub=silub_adj.reshape(NG, 128),
            w_xp=np.ascontiguousarray(xp_w[d].T).astype(bf),       # [DI, 64]
            w_dt=np.ascontiguousarray(dt_w[d].T).astype(bf),       # [DR, DI]
            dtb=dt_b[d].reshape(NG, 128),
            a_sc=np.ascontiguousarray(a_neg.reshape(NG, 128, DS)),
            d_vec=d_par[d].reshape(NG, 128),
            w_out=np.ascontiguousarray(out_w[d].T).astype(bf),     # [DI, DM]
        ))

    in_maps = []
    for core in range(N_CORES):
        d, b = core // 4, core % 4
        xb = x[b]
        m = dict(per_dir[d])
        m["x_d"] = xb if d == 0 else np.ascontiguousarray(xb[::-1])
        m["x_nat"] = xb.astype(bf)
        m["eye"] = eye
        m["eyen"] = eye.astype(bf)
        m["eyej"] = eye[::-1].copy().astype(bf)
        m["ln2w"] = ln2_w.reshape(1, DM)
        m["ln2b"] = ln2_b.reshape(1, DM)
        in_maps.append(m)
    return in_maps


_NC = None


def _get_module():
    global _NC
    if _NC is None:
        _NC = build_module()
    return _NC


def kernel(**inputs) -> np.ndarray:
    nc = _get_module()
    in_maps = make_core_inputs(inputs)
    res = run_bass_kernel_spmd(nc, in_maps, core_ids=list(range(N_CORES)))
    outs = [res.results[b]["out"] for b in range(B_SZ)]
    return np.stack(outs, axis=0)


if __name__ == "__main__":
    nc = build_module()
    print("module built ok")


# revision 21
# speedup vs baseline: 1.2812x; 1.0334x over previous
"""Bidirectional Mamba layer on 8 Trainium2 NeuronCores (Bass/Tile).

Sharding: 8 cores = 2 directions x 4 batch samples. Each core runs the full
Mamba block for its (direction, sample) pair; a pairwise AllGather combines
the two directions, and every core (redundantly, SPMD-symmetric) applies the
final residual + LayerNorm.

Layout: channels on partitions, time on the free axis, full L=2048 per op
(monolith — no time chunking, so the selective scan needs no state carry).
All matmuls run bf16 (single PE pass); elementwise work is bf16 for the DVE
2x mode. The per-(group, state) inner loop is:
  Act:  da = exp(dt * A[g,n])          (scalar engine)
  DVE/GPS: bt = p * B_n                (broadcast multiply)
  DVE:  h = tensor_tensor_scan(da, bt) (the irreducible 2 cyc/elem scan)
  DVE:  yn = h * C_n
  PE:   py += I @ yn                   (PSUM accumulation over states)
"""

import ml_dtypes
import numpy as np

import concourse.bass as bass
import concourse.bacc as bacc
import concourse.tile as tile
from concourse import mybir
from concourse.bass_utils import run_bass_kernel_spmd

# ---- problem shapes (hardcoded per contract) ----
B_SZ, L, DM = 4, 2048, 512
D_CONV, DS, DR = 4, 16, 32
DI = 1024                  # d_inner
E2 = 2 * DI                # in_proj rows
NG = DI // 128             # 8 channel blocks
NTT = L // 128             # 16 time tiles of 128
NSL = L // 512             # 4 psum-width slices
EPS = 1e-5
F32 = mybir.dt.float32
BF16 = mybir.dt.bfloat16
AF = mybir.ActivationFunctionType
OP = mybir.AluOpType

N_CORES = 8

# CoreSim does not implement Silu; tests flip this to use sigmoid+mult
USE_SILU = True
REPLICA_GROUPS = [[0, 4], [1, 5], [2, 6], [3, 7]]

# groups whose bt-/yn-multiply runs on GPSIMD instead of DVE.
# Empirically GPSIMD elementwise is net-negative here: it contends for the
# DVE's SBUF port and slows 2x-mode DVE ops more than it absorbs.
GPS_GS = ()
GPS_YS = ()


def _silu(nc, pool, out_tile, psum, bias=None):
    """out = silu(psum [+ bias]); Silu on HW, sigmoid+mult fallback for sim."""
    kw = {} if bias is None else {"bias": bias}
    if USE_SILU:
        nc.scalar.activation(out_tile, psum, AF.Silu, **kw)
    else:
        zb = pool.tile(list(out_tile.shape), F32, tag="_silu_zb", name="zb",
                       bufs=1)
        nc.scalar.activation(zb, psum, AF.Identity, **kw)
        nc.scalar.activation(out_tile, psum, AF.Sigmoid, **kw)
        nc.vector.tensor_mul(out_tile, zb, out_tile)


def build_module():
    nc = bacc.Bacc(
        "TRN2", target_bir_lowering=False, debug=False, num_devices=N_CORES
    )

    # ---------------- I/O ----------------
    x_d = nc.dram_tensor("x_d", [L, DM], BF16, kind="ExternalInput")
    x_nat = nc.dram_tensor("x_nat", [L, DM], BF16, kind="ExternalInput")
    w_in = nc.dram_tensor("w_in", [DM, E2], BF16, kind="ExternalInput")
    convd = nc.dram_tensor("convd", [D_CONV, NG, 128, 128], BF16, kind="ExternalInput")
    convb = nc.dram_tensor("convb", [NG, 128], F32, kind="ExternalInput")
    silub = nc.dram_tensor("silub", [NG, 128], F32, kind="ExternalInput")
    w_xp = nc.dram_tensor("w_xp", [DI, DR + 2 * DS], BF16, kind="ExternalInput")
    w_dt = nc.dram_tensor("w_dt", [DR, DI], BF16, kind="ExternalInput")
    dtb = nc.dram_tensor("dtb", [NG, 128], F32, kind="ExternalInput")
    a_sc = nc.dram_tensor("a_sc", [NG, 128, DS], F32, kind="ExternalInput")
    d_vec = nc.dram_tensor("d_vec", [NG, 128], F32, kind="ExternalInput")
    w_out = nc.dram_tensor("w_out", [DI, DM], BF16, kind="ExternalInput")
    eye = nc.dram_tensor("eye", [128, 128], F32, kind="ExternalInput")
    eyen = nc.dram_tensor("eyen", [128, 128], BF16, kind="ExternalInput")
    eyej = nc.dram_tensor("eyej", [128, 128], BF16, kind="ExternalInput")
    ln2w = nc.dram_tensor("ln2w", [1, DM], F32, kind="ExternalInput")
    ln2b = nc.dram_tensor("ln2b", [1, DM], F32, kind="ExternalInput")
    out = nc.dram_tensor("out", [L, DM], F32, kind="ExternalOutput")

    with tile.TileContext(nc) as tc:
        build_program(
            tc, x_d, x_nat, w_in, convd, convb, silub, w_xp, w_dt, dtb,
            a_sc, d_vec, w_out, eye, eyen, eyej, ln2w, ln2b, out,
        )
    nc.compile()
    return nc


def build_program(tc, x_d, x_nat, w_in, convd, convb, silub, w_xp, w_dt, dtb,
                  a_sc, d_vec, w_out, eye, eyen, eyej, ln2w, ln2b, out):
    nc = tc.nc
    from contextlib import ExitStack

    ctx = ExitStack()
    with ctx:
        wp = ctx.enter_context(tc.tile_pool(name="wp", bufs=1))
        lnp = ctx.enter_context(tc.tile_pool(name="lnp", bufs=2))
        big = ctx.enter_context(tc.tile_pool(name="big", bufs=1))
        snp = ctx.enter_context(tc.tile_pool(name="snp", bufs=3))
        repp = ctx.enter_context(tc.tile_pool(name="repp", bufs=3))
        dramp = ctx.enter_context(tc.tile_pool(name="dramp", bufs=2, space="DRAM"))
        pp_ctx = tc.tile_pool(name="pp", space="PSUM", bufs=2)
        pp = pp_ctx.__enter__()

        # x-load + LN1 stats first so its DMAs aren't queued behind weights
        eps_sb = wp.tile([128, 1], F32)
        nc.vector.memset(eps_sb, EPS)
        mvall = wp.tile([128, 2, NTT], F32, name="mvall")
        nmr = wp.tile([128, NTT], F32, name="nmr")
        xts = []
        for i in range(NTT):
            xt = lnp.tile([128, DM], BF16, tag="xt", bufs=NTT)
            nc.sync.dma_start(xt, x_d[i * 128:(i + 1) * 128, :])
            xts.append(xt)
            st = lnp.tile([128, 6], F32, tag="st")
            nc.vector.bn_stats(st, xt)
            nc.vector.bn_aggr(mvall[:, :, i], st)
        nc.scalar.activation(mvall[:, 1, :], mvall[:, 1, :], AF.Ln,
                             bias=eps_sb[:, 0:1])
        nc.scalar.activation(mvall[:, 1, :], mvall[:, 1, :], AF.Exp,
                             scale=-0.5)
        nc.vector.tensor_mul(nmr, mvall[:, 0, :], mvall[:, 1, :])
        nc.vector.tensor_scalar_mul(nmr, nmr, -1.0)

        w_in_sb = wp.tile([128, DM // 128, E2], BF16)
        for k in range(DM // 128):
            nc.sync.dma_start(w_in_sb[:, k, :], w_in[k * 128:(k + 1) * 128, :])
        convd_sb = wp.tile([128, D_CONV, NG, 128], BF16)
        for k in range(D_CONV):
            for g in range(NG):
                nc.sync.dma_start(convd_sb[:, k, g, :], convd[k, g])
        w_xp_sb = wp.tile([128, NG, DR + 2 * DS], BF16)
        for g in range(NG):
            nc.sync.dma_start(w_xp_sb[:, g, :], w_xp[g * 128:(g + 1) * 128, :])
        w_dt_sb = wp.tile([DR, DI], BF16)
        nc.sync.dma_start(w_dt_sb, w_dt[:, :])
        w_out_sb = wp.tile([128, NG, DM], BF16)
        for g in range(NG):
            nc.sync.dma_start(w_out_sb[:, g, :], w_out[g * 128:(g + 1) * 128, :])
        a_sb = wp.tile([128, NG, DS], F32)
        for g in range(NG):
            nc.sync.dma_start(a_sb[:, g, :], a_sc[g])
        eye_sb = wp.tile([128, 128], F32)
        nc.sync.dma_start(eye_sb, eye[:, :])
        eyen_sb = wp.tile([128, 128], BF16)
        nc.sync.dma_start(eyen_sb, eyen[:, :])
        eyej_sb = wp.tile([128, 128], BF16)
        nc.sync.dma_start(eyej_sb, eyej[:, :])

        def col_load(name, src):
            t = wp.tile([128, NG], F32, name=name)
            for g in range(NG):
                nc.sync.dma_start(t[:, g:g + 1], src[g:g + 1, :].rearrange("a b -> b a"))
            return t

        convb_sb = col_load("convb_sb", convb)
        silub_sb = col_load("silub_sb", silub)
        dtb_sb = col_load("dtb_sb", dtb)
        d_sb = col_load("d_sb", d_vec)

        ln2w_sb = wp.tile([128, DM], F32)
        nc.sync.dma_start(ln2w_sb, ln2w[0:1, :].to_broadcast([128, DM]))
        ln2b_sb = wp.tile([128, DM], F32)
        nc.sync.dma_start(ln2b_sb, ln2b[0:1, :].to_broadcast([128, DM]))

        sg_d = dramp.tile([NG, 128, L], BF16, bufs=1)
        bc_d = dramp.tile([2 * DS, L], BF16, bufs=1)

        # ---------------- phase A: LayerNorm1 + transpose ----------------
        # xn_t: [dm-part, k-block, t] bf16, consumed by in_proj
        xn_t = big.tile([128, DM // 128, L], BF16, name="xn_t")
        # normalize sweep on the resident bf16 tiles (no reload)
        for i in range(NTT):
            xt = xts[i]
            nc.scalar.activation(xt, xt, AF.Identity,
                                 scale=mvall[:, 1, i:i + 1],
                                 bias=nmr[:, i:i + 1])
            for kb in range(DM // 128):
                ps_t = pp.tile([128, 128], BF16, tag="pst")
                nc.tensor.transpose(ps_t, xt[:, kb * 128:(kb + 1) * 128],
                                    eyen_sb)
                nc.vector.tensor_copy(xn_t[:, kb, i * 128:(i + 1) * 128],
                                      ps_t)

        # ---------------- phase B: in_proj ----------------
        # xc half into conv staging (3-col zero halo at front)
        xzf = [big.tile([128, L + 3], BF16, name=f"xzf{g}") for g in range(NG)]
        for g in range(NG):
            nc.vector.memset(xzf[g][:, 0:3], 0.0)
            for s in range(NSL):
                pz = pp.tile([128, 512], F32, tag="ps")
                for k in range(DM // 128):
                    nc.tensor.matmul(pz, w_in_sb[:, k, g * 128:(g + 1) * 128],
                                     xn_t[:, k, s * 512:(s + 1) * 512],
                                     start=(k == 0), stop=(k == DM // 128 - 1))
                nc.vector.tensor_copy(xzf[g][:, 3 + s * 512:3 + (s + 1) * 512], pz)

        # z half + SiLU gate -> sg (spilled to DRAM, reloaded at gate)
        for g in range(NG):
            for s in range(NSL):
                pz = pp.tile([128, 512], F32, tag="ps")
                for k in range(DM // 128):
                    nc.tensor.matmul(
                        pz, w_in_sb[:, k, DI + g * 128: DI + (g + 1) * 128],
                        xn_t[:, k, s * 512:(s + 1) * 512],
                        start=(k == 0), stop=(k == DM // 128 - 1))
                sgt = snp.tile([128, 512], BF16, tag="sgt", bufs=2)
                _silu(nc, snp, sgt, pz, silub_sb[:, g:g + 1])
                nc.sync.dma_start(sg_d[g][:, s * 512:(s + 1) * 512], sgt)

        # ---------------- phase C: depthwise causal conv + SiLU ----------
        xc = [big.tile([128, L], BF16, name=f"xc{g}") for g in range(NG)]
        for g in range(NG):
            for s in range(NSL):
                pc = pp.tile([128, 512], F32, tag="ps")
                for k in range(D_CONV):
                    nc.tensor.matmul(pc, convd_sb[:, k, g, :],
                                     xzf[g][:, k + s * 512: k + (s + 1) * 512],
                                     start=(k == 0), stop=(k == D_CONV - 1))
                _silu(nc, snp, xc[g][:, s * 512:(s + 1) * 512], pc,
                      convb_sb[:, g:g + 1])

        # ---------------- phase D: x_proj -> (dt_r, B, C) -----------------
        xdbl = big.tile([64, L], BF16, name="xdbl")
        for s in range(NSL):
            pxd = pp.tile([64, 512], F32, tag="pxd", bufs=1)
            for g in range(NG):
                nc.tensor.matmul(pxd, w_xp_sb[:, g, :],
                                 xc[g][:, s * 512:(s + 1) * 512],
                                 start=(g == 0), stop=(g == NG - 1))
            nc.vector.tensor_copy(xdbl[:, s * 512:(s + 1) * 512], pxd)
        nc.sync.dma_start(bc_d, xdbl[DR:DR + 2 * DS, :])

        # dt = softplus(dt_proj @ dt_r + bias); p = dt * xc
        # dt_c[0..3] reuse xn_t's space, dt_c[4..7] reuse w_in_sb (both dead
        # after in_proj); p_c reuses the conv staging xzf (dead after conv)
        dt_c = [xn_t[:, g, :] if g < 4 else w_in_sb[:, g - 4, :]
                for g in range(NG)]
        p_c = [xzf[g][:, 0:L] for g in range(NG)]
        for g in range(NG):
            for s in range(NSL):
                sl = slice(s * 512, (s + 1) * 512)
                pdt = pp.tile([128, 512], F32, tag="ps")
                nc.tensor.matmul(pdt, w_dt_sb[:, g * 128:(g + 1) * 128],
                                 xdbl[0:DR, sl],
                                 start=True, stop=True)
                nc.scalar.activation(dt_c[g][:, sl], pdt, AF.Exp,
                                     bias=dtb_sb[:, g:g + 1])
        for g in range(NG):
            nc.scalar.activation(dt_c[g], dt_c[g], AF.Ln, bias=1.0)
            nc.vector.tensor_mul(p_c[g], dt_c[g], xc[g])

        # ---------------- phase E: selective scan ------------------------
        # waves of 3 groups share each B/C broadcast; time in halves of 1024.
        # py uses 6 PSUM banks, leaving 2 for out_proj so each half's
        # out_proj + collective piece overlaps the next half's compute.
        pp_ctx.__exit__(None, None, None)
        HW = L // 2
        WAVES = [(0, 1, 2), (3, 4, 5), (6, 7)]
        yf = [xc[g] for g in range(NG)]
        ypart_h = [dramp.tile([HW, DM], BF16, bufs=1, name=f"yph{h}")
                   for h in range(2)]
        gbuf_h = [dramp.tile([2, HW, DM], BF16, bufs=1, name=f"gbh{h}")
                  for h in range(2)]
        ppy_ctx = tc.tile_pool(name="ppy", space="PSUM", bufs=1)
        ppy = ppy_ctx.__enter__()
        pp2_ctx = tc.tile_pool(name="pp2", space="PSUM", bufs=2)
        pp = pp2_ctx.__enter__()
        hcars = [snp.tile([128, 4, DS], BF16, tag="hcar", bufs=3,
                          name=f"hcar{w}") for w in range(3)]

        def out_proj_half(half):
            hs = slice(half * HW, (half + 1) * HW)
            for tb in range(8):
                t0 = half * HW + tb * 128
                po = pp.tile([128, DM], F32, tag="ps")
                for g in range(NG):
                    nc.tensor.matmul(po, yf[g][:, t0:t0 + 128],
                                     w_out_sb[:, g, :],
                                     start=(g == 0), stop=(g == NG - 1))
                ot = lnp.tile([128, DM], BF16, tag="ot")
                nc.scalar.copy(ot, po)
                nc.sync.dma_start(ypart_h[half][tb * 128:(tb + 1) * 128, :], ot)
            nc.gpsimd.collective_compute(
                "AllGather", OP.bypass, replica_groups=REPLICA_GROUPS,
                ins=[ypart_h[half].opt()], outs=[gbuf_h[half].opt()],
            )

        for half in (0, 1):
            hs = slice(half * HW, (half + 1) * HW)
            for wv, wave in enumerate(WAVES):
                hcar = hcars[wv]
                py = [ppy.tile([128, HW], F32, tag=f"py{j}", bufs=1,
                               name=f"py{j}") for j in range(len(wave))]
                for j, g in enumerate(wave):
                    yd = snp.tile([128, HW], BF16, tag="yd", bufs=2)
                    nc.vector.tensor_scalar_mul(yd, xc[g][:, hs],
                                                d_sb[:, g:g + 1])
                    for s in range(2):
                        nc.tensor.matmul(py[j][:, s * 512:(s + 1) * 512],
                                         eyen_sb,
                                         yd[:, s * 512:(s + 1) * 512],
                                         start=True, stop=False)
                for n in range(DS):
                    brep = repp.tile([128, HW], BF16, tag="brep", bufs=3)
                    nc.sync.dma_start(
                        brep, bc_d[n:n + 1, hs].to_broadcast([128, HW]))
                    crep = repp.tile([128, HW], BF16, tag="crep", bufs=3)
                    nc.sync.dma_start(
                        crep,
                        bc_d[DS + n:DS + n + 1, hs].to_broadcast([128, HW]))
                    for j, g in enumerate(wave):
                        da = snp.tile([128, HW], BF16, tag="da", bufs=3)
                        nc.scalar.activation(da, dt_c[g][:, hs], AF.Exp,
                                             scale=a_sb[:, g, n:n + 1])
                        bt = snp.tile([128, HW], BF16, tag="bt",
                                      name="btd", bufs=3)
                        nc.vector.tensor_mul(bt, p_c[g][:, hs], brep)
                        h = snp.tile([128, HW], BF16, tag="h", bufs=2)
                        init = 0.0 if half == 0 else hcar[:, j, n:n + 1]
                        nc.vector.tensor_tensor_scan(h, da, bt,
                                                     initial=init,
                                                     op0=OP.mult,
                                                     op1=OP.add)
                        if half == 0:
                            nc.vector.tensor_copy(hcar[:, j, n:n + 1],
                                                  h[:, HW - 1:HW])
                        yn = snp.tile([128, HW], BF16, tag="yn", bufs=2)
                        nc.vector.tensor_mul(yn, h, crep)
                        for s in range(2):
                            nc.tensor.matmul(
                                py[j][:, s * 512:(s + 1) * 512], eyen_sb,
                                yn[:, s * 512:(s + 1) * 512],
                                start=False, stop=(n == DS - 1))
                # gate with silu(z); write into xc[g]'s tile (xc dead now)
                for j, g in enumerate(wave):
                    sgl = repp.tile([128, HW], BF16, tag="sgl", bufs=2)
                    nc.sync.dma_start(sgl, sg_d[g][:, hs])
                    nc.vector.tensor_mul(yf[g][:, hs], py[j], sgl)
            out_proj_half(half)

        # ---------------- final residual + LayerNorm ----------------------
        mvall2 = wp.tile([128, 2, NTT], F32, name="mvall2")
        nmr2 = wp.tile([128, NTT], F32, name="nmr2")

        def sum3(i):
            # natural-time tile i: dir0 rows i*128.. (gbuf piece by row range)
            r0 = i * 128
            ph0 = 0 if r0 < HW else 1
            s0 = lnp.tile([128, DM], BF16, tag="s0")
            nc.sync.dma_start(
                s0, gbuf_h[ph0][0, r0 - ph0 * HW: r0 - ph0 * HW + 128, :])
            # direction-1 partial is in flipped local order
            lr = L - (i + 1) * 128
            ph1 = 0 if lr < HW else 1
            s1 = lnp.tile([128, DM], BF16, tag="s1")
            nc.sync.dma_start(
                s1, gbuf_h[ph1][1, lr - ph1 * HW: lr - ph1 * HW + 128, :])
            xt2 = lnp.tile([128, DM], BF16, tag="xt2")
            nc.sync.dma_start(xt2, x_nat[i * 128:(i + 1) * 128, :])
            pj = pp.tile([128, DM], F32, tag="ps")
            nc.tensor.matmul(pj, eyej_sb, s1, start=True, stop=False)
            nc.tensor.matmul(pj, eyen_sb, s0, start=False, stop=False)
            nc.tensor.matmul(pj, eyen_sb, xt2, start=False, stop=True)
            return pj

        # sweep 1: stats for every tile (batched rstd afterwards)
        for i in range(NTT):
            pj = sum3(i)
            st = lnp.tile([128, 6], F32, tag="st2")
            nc.vector.bn_stats(st, pj)
            nc.vector.bn_aggr(mvall2[:, :, i], st)
        nc.scalar.activation(mvall2[:, 1, :], mvall2[:, 1, :], AF.Ln,
                             bias=eps_sb[:, 0:1])
        nc.scalar.activation(mvall2[:, 1, :], mvall2[:, 1, :], AF.Exp,
                             scale=-0.5)
        nc.vector.tensor_mul(nmr2, mvall2[:, 0, :], mvall2[:, 1, :])
        nc.vector.tensor_scalar_mul(nmr2, nmr2, -1.0)
        # sweep 2: rebuild the sum (3 cheap matmuls) and normalize from PSUM
        for i in range(NTT):
            pj = sum3(i)
            s = lnp.tile([128, DM], F32, tag="ssum")
            nc.scalar.activation(s, pj, AF.Identity,
                                 scale=mvall2[:, 1, i:i + 1],
                                 bias=nmr2[:, i:i + 1])
            nc.vector.tensor_mul(s, s, ln2w_sb)
            nc.vector.tensor_add(s, s, ln2b_sb)
            nc.sync.dma_start(out[i * 128:(i + 1) * 128, :], s)
        pp2_ctx.__exit__(None, None, None)
        ppy_ctx.__exit__(None, None, None)


# ---------------- host side ----------------

def make_core_inputs(inputs):
    """Build the 8 per-core input dicts from the full problem inputs."""
    x = np.ascontiguousarray(inputs["x"], dtype=np.float32)        # [B, L, DM]
    ln1_w = inputs["ln1_w"].astype(np.float32)
    ln1_b = inputs["ln1_b"].astype(np.float32)
    in_w = inputs["in_proj_w"].astype(np.float32)                  # [2, 2DI, DM]
    conv_w = inputs["conv_w"].astype(np.float32)                   # [2, DI, 4]
    conv_b = inputs["conv_b"].astype(np.float32)                   # [2, DI]
    xp_w = inputs["x_proj_w"].astype(np.float32)                   # [2, 64, DI]
    dt_w = inputs["dt_proj_w"].astype(np.float32)                  # [2, DI, DR]
    dt_b = inputs["dt_proj_b"].astype(np.float32)                  # [2, DI]
    a_log = inputs["A_log"].astype(np.float32)                     # [2, DI, DS]
    d_par = inputs["D_param"].astype(np.float32)                   # [2, DI]
    out_w = inputs["out_proj_w"].astype(np.float32)                # [2, DM, DI]
    ln2_w = inputs["ln2_w"].astype(np.float32)
    ln2_b = inputs["ln2_b"].astype(np.float32)

    bf = ml_dtypes.bfloat16
    eye = np.eye(128, dtype=np.float32)
    per_dir = []
    for d in range(2):
        w = in_w[d]                                   # [2DI, DM]
        w_in_T = np.ascontiguousarray((w * ln1_w[None, :]).T)      # [DM, 2DI]
        v = w @ ln1_b                                  # [2DI]
        csum = conv_w[d].sum(axis=1)                   # [DI]
        convb_adj = conv_b[d] + csum * v[:DI]
        silub_adj = v[DI:]
        convd = np.zeros((D_CONV, NG, 128, 128), np.float32)
        for k in range(D_CONV):
            for g in range(NG):
                np.fill_diagonal(convd[k, g], conv_w[d, g * 128:(g + 1) * 128, k])
        a_neg = -np.exp(a_log[d])                      # [DI, DS]
        per_dir.append(dict(
            w_in=w_in_T.astype(bf),
            convd=convd.astype(bf),
            convb=convb_adj.reshape(NG, 128),
            silub=silub_adj.reshape(NG, 128),
            w_xp=np.ascontiguousarray(xp_w[d].T).astype(bf),       # [DI, 64]
            w_dt=np.ascontiguousarray(dt_w[d].T).astype(bf),       # [DR, DI]
            dtb=dt_b[d].reshape(NG, 128),
            a_sc=np.ascontiguousarray(a_neg.reshape(NG, 128, DS)),
            d_vec=d_par[d].reshape(NG, 128),
            w_out=np.ascontiguousarray(out_w[d].T).astype(bf),     # [DI, DM]
        ))

    in_maps = []
    for core in range(N_CORES):
        d, b = core // 4, core % 4
        xb = x[b]
        m = dict(per_dir[d])
        m["x_d"] = (xb if d == 0 else np.ascontiguousarray(xb[::-1])).astype(bf)
        m["x_nat"] = xb.astype(bf)
        m["eye"] = eye
        m["eyen"] = eye.astype(bf)
        m["eyej"] = eye[::-1].copy().astype(bf)
        m["ln2w"] = ln2_w.reshape(1, DM)
        m["ln2b"] = ln2_b.reshape(1, DM)
        in_maps.append(m)
    return in_maps


_NC = None


def _get_module():
    global _NC
    if _NC is None:
        _NC = build_module()
    return _NC


def kernel(**inputs) -> np.ndarray:
    nc = _get_module()
    in_maps = make_core_inputs(inputs)
    res = run_bass_kernel_spmd(nc, in_maps, core_ids=list(range(N_CORES)))
    outs = [res.results[b]["out"] for b in range(B_SZ)]
    return np.stack(outs, axis=0)


if __name__ == "__main__":
    nc = build_module()
    print("module built ok")


# revision 22
# speedup vs baseline: 1.2853x; 1.0032x over previous
"""Bidirectional Mamba layer on 8 Trainium2 NeuronCores (Bass/Tile).

Sharding: 8 cores = 2 directions x 4 batch samples. Each core runs the full
Mamba block for its (direction, sample) pair; a pairwise AllGather combines
the two directions, and every core (redundantly, SPMD-symmetric) applies the
final residual + LayerNorm.

Layout: channels on partitions, time on the free axis, full L=2048 per op
(monolith — no time chunking, so the selective scan needs no state carry).
All matmuls run bf16 (single PE pass); elementwise work is bf16 for the DVE
2x mode. The per-(group, state) inner loop is:
  Act:  da = exp(dt * A[g,n])          (scalar engine)
  DVE/GPS: bt = p * B_n                (broadcast multiply)
  DVE:  h = tensor_tensor_scan(da, bt) (the irreducible 2 cyc/elem scan)
  DVE:  yn = h * C_n
  PE:   py += I @ yn                   (PSUM accumulation over states)
"""

import ml_dtypes
import numpy as np

import concourse.bass as bass
import concourse.bacc as bacc
import concourse.tile as tile
from concourse import mybir
from concourse.bass_utils import run_bass_kernel_spmd

# ---- problem shapes (hardcoded per contract) ----
B_SZ, L, DM = 4, 2048, 512
D_CONV, DS, DR = 4, 16, 32
DI = 1024                  # d_inner
E2 = 2 * DI                # in_proj rows
NG = DI // 128             # 8 channel blocks
NTT = L // 128             # 16 time tiles of 128
NSL = L // 512             # 4 psum-width slices
EPS = 1e-5
F32 = mybir.dt.float32
BF16 = mybir.dt.bfloat16
AF = mybir.ActivationFunctionType
OP = mybir.AluOpType

N_CORES = 8

# CoreSim does not implement Silu; tests flip this to use sigmoid+mult
USE_SILU = True
REPLICA_GROUPS = [[0, 4], [1, 5], [2, 6], [3, 7]]

# groups whose bt-/yn-multiply runs on GPSIMD instead of DVE.
# Empirically GPSIMD elementwise is net-negative here: it contends for the
# DVE's SBUF port and slows 2x-mode DVE ops more than it absorbs.
GPS_GS = ()
GPS_YS = ()


def _silu(nc, pool, out_tile, psum, bias=None):
    """out = silu(psum [+ bias]); Silu on HW, sigmoid+mult fallback for sim."""
    kw = {} if bias is None else {"bias": bias}
    if USE_SILU:
        nc.scalar.activation(out_tile, psum, AF.Silu, **kw)
    else:
        zb = pool.tile(list(out_tile.shape), F32, tag="_silu_zb", name="zb",
                       bufs=1)
        nc.scalar.activation(zb, psum, AF.Identity, **kw)
        nc.scalar.activation(out_tile, psum, AF.Sigmoid, **kw)
        nc.vector.tensor_mul(out_tile, zb, out_tile)


def build_module():
    nc = bacc.Bacc(
        "TRN2", target_bir_lowering=False, debug=False, num_devices=N_CORES
    )

    # ---------------- I/O ----------------
    x_d = nc.dram_tensor("x_d", [L, DM], BF16, kind="ExternalInput")
    x_nat = nc.dram_tensor("x_nat", [L, DM], BF16, kind="ExternalInput")
    w_in = nc.dram_tensor("w_in", [DM, E2], BF16, kind="ExternalInput")
    convd = nc.dram_tensor("convd", [D_CONV, NG, 128, 128], BF16, kind="ExternalInput")
    convb = nc.dram_tensor("convb", [NG, 128], F32, kind="ExternalInput")
    silub = nc.dram_tensor("silub", [NG, 128], F32, kind="ExternalInput")
    w_xp = nc.dram_tensor("w_xp", [DI, DR + 2 * DS], BF16, kind="ExternalInput")
    w_dt = nc.dram_tensor("w_dt", [DR, DI], BF16, kind="ExternalInput")
    dtb = nc.dram_tensor("dtb", [NG, 128], F32, kind="ExternalInput")
    a_sc = nc.dram_tensor("a_sc", [NG, 128, DS], F32, kind="ExternalInput")
    d_vec = nc.dram_tensor("d_vec", [NG, 128], F32, kind="ExternalInput")
    w_out = nc.dram_tensor("w_out", [DI, DM], BF16, kind="ExternalInput")
    eye = nc.dram_tensor("eye", [128, 128], F32, kind="ExternalInput")
    eyen = nc.dram_tensor("eyen", [128, 128], BF16, kind="ExternalInput")
    eyej = nc.dram_tensor("eyej", [128, 128], BF16, kind="ExternalInput")
    ln2w = nc.dram_tensor("ln2w", [1, DM], F32, kind="ExternalInput")
    ln2b = nc.dram_tensor("ln2b", [1, DM], F32, kind="ExternalInput")
    out = nc.dram_tensor("out", [L, DM], F32, kind="ExternalOutput")

    with tile.TileContext(nc) as tc:
        build_program(
            tc, x_d, x_nat, w_in, convd, convb, silub, w_xp, w_dt, dtb,
            a_sc, d_vec, w_out, eye, eyen, eyej, ln2w, ln2b, out,
        )
    nc.compile()
    return nc


def build_program(tc, x_d, x_nat, w_in, convd, convb, silub, w_xp, w_dt, dtb,
                  a_sc, d_vec, w_out, eye, eyen, eyej, ln2w, ln2b, out):
    nc = tc.nc
    from contextlib import ExitStack

    ctx = ExitStack()
    with ctx:
        wp = ctx.enter_context(tc.tile_pool(name="wp", bufs=1))
        lnp = ctx.enter_context(tc.tile_pool(name="lnp", bufs=2))
        big = ctx.enter_context(tc.tile_pool(name="big", bufs=1))
        snp = ctx.enter_context(tc.tile_pool(name="snp", bufs=3))
        repp = ctx.enter_context(tc.tile_pool(name="repp", bufs=3))
        dramp = ctx.enter_context(tc.tile_pool(name="dramp", bufs=2, space="DRAM"))
        pp_ctx = tc.tile_pool(name="pp", space="PSUM", bufs=2)
        pp = pp_ctx.__enter__()

        # x-load + LN1 stats first so its DMAs aren't queued behind weights
        eps_sb = wp.tile([128, 1], F32)
        nc.vector.memset(eps_sb, EPS)
        mvall = wp.tile([128, 2, NTT], F32, name="mvall")
        nmr = wp.tile([128, NTT], F32, name="nmr")
        xts = []
        for i in range(NTT):
            xt = lnp.tile([128, DM], BF16, tag="xt", bufs=NTT)
            nc.sync.dma_start(xt, x_d[i * 128:(i + 1) * 128, :])
            xts.append(xt)
            st = lnp.tile([128, 6], F32, tag="st")
            nc.vector.bn_stats(st, xt)
            nc.vector.bn_aggr(mvall[:, :, i], st)
        # rstd batches split 4/12 so the first tiles (which gate in_proj)
        # unblock as early as possible
        for lo, hi in ((0, 4), (4, NTT)):
            nc.scalar.activation(mvall[:, 1, lo:hi], mvall[:, 1, lo:hi],
                                 AF.Ln, bias=eps_sb[:, 0:1])
            nc.scalar.activation(mvall[:, 1, lo:hi], mvall[:, 1, lo:hi],
                                 AF.Exp, scale=-0.5)
            nc.vector.tensor_mul(nmr[:, lo:hi], mvall[:, 0, lo:hi],
                                 mvall[:, 1, lo:hi])
            nc.vector.tensor_scalar_mul(nmr[:, lo:hi], nmr[:, lo:hi], -1.0)

        w_in_sb = wp.tile([128, DM // 128, E2], BF16)
        for k in range(DM // 128):
            nc.sync.dma_start(w_in_sb[:, k, :], w_in[k * 128:(k + 1) * 128, :])
        convd_sb = wp.tile([128, D_CONV, NG, 128], BF16)
        for k in range(D_CONV):
            for g in range(NG):
                nc.sync.dma_start(convd_sb[:, k, g, :], convd[k, g])
        w_xp_sb = wp.tile([128, NG, DR + 2 * DS], BF16)
        for g in range(NG):
            nc.sync.dma_start(w_xp_sb[:, g, :], w_xp[g * 128:(g + 1) * 128, :])
        w_dt_sb = wp.tile([DR, DI], BF16)
        nc.sync.dma_start(w_dt_sb, w_dt[:, :])
        w_out_sb = wp.tile([128, NG, DM], BF16)
        for g in range(NG):
            nc.sync.dma_start(w_out_sb[:, g, :], w_out[g * 128:(g + 1) * 128, :])
        a_sb = wp.tile([128, NG, DS], F32)
        for g in range(NG):
            nc.sync.dma_start(a_sb[:, g, :], a_sc[g])
        eye_sb = wp.tile([128, 128], F32)
        nc.sync.dma_start(eye_sb, eye[:, :])
        eyen_sb = wp.tile([128, 128], BF16)
        nc.sync.dma_start(eyen_sb, eyen[:, :])
        eyej_sb = wp.tile([128, 128], BF16)
        nc.sync.dma_start(eyej_sb, eyej[:, :])

        def col_load(name, src):
            t = wp.tile([128, NG], F32, name=name)
            for g in range(NG):
                nc.sync.dma_start(t[:, g:g + 1], src[g:g + 1, :].rearrange("a b -> b a"))
            return t

        convb_sb = col_load("convb_sb", convb)
        silub_sb = col_load("silub_sb", silub)
        dtb_sb = col_load("dtb_sb", dtb)
        d_sb = col_load("d_sb", d_vec)

        ln2w_sb = wp.tile([128, DM], F32)
        nc.sync.dma_start(ln2w_sb, ln2w[0:1, :].to_broadcast([128, DM]))
        ln2b_sb = wp.tile([128, DM], F32)
        nc.sync.dma_start(ln2b_sb, ln2b[0:1, :].to_broadcast([128, DM]))

        sg_d = dramp.tile([NG, 128, L], BF16, bufs=1)
        bc_d = dramp.tile([2 * DS, L], BF16, bufs=1)

        # ---------------- phase A: LayerNorm1 + transpose ----------------
        # xn_t: [dm-part, k-block, t] bf16, consumed by in_proj
        xn_t = big.tile([128, DM // 128, L], BF16, name="xn_t")
        # normalize sweep on the resident bf16 tiles (no reload)
        for i in range(NTT):
            xt = xts[i]
            nc.scalar.activation(xt, xt, AF.Identity,
                                 scale=mvall[:, 1, i:i + 1],
                                 bias=nmr[:, i:i + 1])
            for kb in range(DM // 128):
                ps_t = pp.tile([128, 128], BF16, tag="pst")
                nc.tensor.transpose(ps_t, xt[:, kb * 128:(kb + 1) * 128],
                                    eyen_sb)
                nc.vector.tensor_copy(xn_t[:, kb, i * 128:(i + 1) * 128],
                                      ps_t)

        # ---------------- phase B: in_proj ----------------
        # xc half into conv staging (3-col zero halo at front)
        xzf = [big.tile([128, L + 3], BF16, name=f"xzf{g}") for g in range(NG)]
        for g in range(NG):
            nc.vector.memset(xzf[g][:, 0:3], 0.0)
            for s in range(NSL):
                pz = pp.tile([128, 512], F32, tag="ps", bufs=3)
                for k in range(DM // 128):
                    nc.tensor.matmul(pz, w_in_sb[:, k, g * 128:(g + 1) * 128],
                                     xn_t[:, k, s * 512:(s + 1) * 512],
                                     start=(k == 0), stop=(k == DM // 128 - 1))
                nc.vector.tensor_copy(xzf[g][:, 3 + s * 512:3 + (s + 1) * 512], pz)

        # z half + SiLU gate -> sg (spilled to DRAM, reloaded at gate)
        for g in range(NG):
            for s in range(NSL):
                pz = pp.tile([128, 512], F32, tag="ps", bufs=3)
                for k in range(DM // 128):
                    nc.tensor.matmul(
                        pz, w_in_sb[:, k, DI + g * 128: DI + (g + 1) * 128],
                        xn_t[:, k, s * 512:(s + 1) * 512],
                        start=(k == 0), stop=(k == DM // 128 - 1))
                sgt = snp.tile([128, 512], BF16, tag="sgt", bufs=2)
                _silu(nc, snp, sgt, pz, silub_sb[:, g:g + 1])
                nc.sync.dma_start(sg_d[g][:, s * 512:(s + 1) * 512], sgt)

        # ---------------- phase C: depthwise causal conv + SiLU ----------
        xc = [big.tile([128, L], BF16, name=f"xc{g}") for g in range(NG)]
        for g in range(NG):
            for s in range(NSL):
                pc = pp.tile([128, 512], F32, tag="ps", bufs=3)
                for k in range(D_CONV):
                    nc.tensor.matmul(pc, convd_sb[:, k, g, :],
                                     xzf[g][:, k + s * 512: k + (s + 1) * 512],
                                     start=(k == 0), stop=(k == D_CONV - 1))
                _silu(nc, snp, xc[g][:, s * 512:(s + 1) * 512], pc,
                      convb_sb[:, g:g + 1])

        # ---------------- phase D: x_proj -> (dt_r, B, C) -----------------
        xdbl = big.tile([64, L], BF16, name="xdbl")
        for s in range(NSL):
            pxd = pp.tile([64, 512], F32, tag="pxd", bufs=1)
            for g in range(NG):
                nc.tensor.matmul(pxd, w_xp_sb[:, g, :],
                                 xc[g][:, s * 512:(s + 1) * 512],
                                 start=(g == 0), stop=(g == NG - 1))
            nc.vector.tensor_copy(xdbl[:, s * 512:(s + 1) * 512], pxd)
        nc.sync.dma_start(bc_d, xdbl[DR:DR + 2 * DS, :])

        # dt = softplus(dt_proj @ dt_r + bias); p = dt * xc
        # dt_c[0..3] reuse xn_t's space, dt_c[4..7] reuse w_in_sb (both dead
        # after in_proj); p_c reuses the conv staging xzf (dead after conv)
        dt_c = [xn_t[:, g, :] if g < 4 else w_in_sb[:, g - 4, :]
                for g in range(NG)]
        p_c = [xzf[g][:, 0:L] for g in range(NG)]
        for g in range(NG):
            for s in range(NSL):
                sl = slice(s * 512, (s + 1) * 512)
                pdt = pp.tile([128, 512], F32, tag="ps", bufs=3)
                nc.tensor.matmul(pdt, w_dt_sb[:, g * 128:(g + 1) * 128],
                                 xdbl[0:DR, sl],
                                 start=True, stop=True)
                nc.scalar.activation(dt_c[g][:, sl], pdt, AF.Exp,
                                     bias=dtb_sb[:, g:g + 1])
        for g in range(NG):
            nc.scalar.activation(dt_c[g], dt_c[g], AF.Ln, bias=1.0)
            nc.vector.tensor_mul(p_c[g], dt_c[g], xc[g])

        # ---------------- phase E: selective scan ------------------------
        # waves of 3 groups share each B/C broadcast; time in halves of 1024.
        # py uses 6 PSUM banks, leaving 2 for out_proj so each half's
        # out_proj + collective piece overlaps the next half's compute.
        pp_ctx.__exit__(None, None, None)
        HW = L // 2
        WAVES = [(0, 1, 2), (3, 4, 5), (6, 7)]
        yf = [xc[g] for g in range(NG)]
        ypart_h = [dramp.tile([HW, DM], BF16, bufs=1, name=f"yph{h}")
                   for h in range(2)]
        gbuf_h = [dramp.tile([2, HW, DM], BF16, bufs=1, name=f"gbh{h}")
                  for h in range(2)]
        ppy_ctx = tc.tile_pool(name="ppy", space="PSUM", bufs=1)
        ppy = ppy_ctx.__enter__()
        pp2_ctx = tc.tile_pool(name="pp2", space="PSUM", bufs=2)
        pp = pp2_ctx.__enter__()
        hcars = [snp.tile([128, 4, DS], BF16, tag="hcar", bufs=3,
                          name=f"hcar{w}") for w in range(3)]

        def out_proj_half(half):
            hs = slice(half * HW, (half + 1) * HW)
            for tb in range(8):
                t0 = half * HW + tb * 128
                po = pp.tile([128, DM], F32, tag="ps")
                for g in range(NG):
                    nc.tensor.matmul(po, yf[g][:, t0:t0 + 128],
                                     w_out_sb[:, g, :],
                                     start=(g == 0), stop=(g == NG - 1))
                ot = lnp.tile([128, DM], BF16, tag="ot")
                nc.scalar.copy(ot, po)
                nc.sync.dma_start(ypart_h[half][tb * 128:(tb + 1) * 128, :], ot)
            nc.gpsimd.collective_compute(
                "AllGather", OP.bypass, replica_groups=REPLICA_GROUPS,
                ins=[ypart_h[half].opt()], outs=[gbuf_h[half].opt()],
            )

        for half in (0, 1):
            hs = slice(half * HW, (half + 1) * HW)
            for wv, wave in enumerate(WAVES):
                hcar = hcars[wv]
                py = [ppy.tile([128, HW], F32, tag=f"py{j}", bufs=1,
                               name=f"py{j}") for j in range(len(wave))]
                for j, g in enumerate(wave):
                    yd = snp.tile([128, HW], BF16, tag="yd", bufs=2)
                    nc.vector.tensor_scalar_mul(yd, xc[g][:, hs],
                                                d_sb[:, g:g + 1])
                    for s in range(2):
                        nc.tensor.matmul(py[j][:, s * 512:(s + 1) * 512],
                                         eyen_sb,
                                         yd[:, s * 512:(s + 1) * 512],
                                         start=True, stop=False)
                for n in range(DS):
                    brep = repp.tile([128, HW], BF16, tag="brep", bufs=3)
                    nc.sync.dma_start(
                        brep, bc_d[n:n + 1, hs].to_broadcast([128, HW]))
                    crep = repp.tile([128, HW], BF16, tag="crep", bufs=3)
                    nc.sync.dma_start(
                        crep,
                        bc_d[DS + n:DS + n + 1, hs].to_broadcast([128, HW]))
                    for j, g in enumerate(wave):
                        da = snp.tile([128, HW], BF16, tag="da", bufs=3)
                        nc.scalar.activation(da, dt_c[g][:, hs], AF.Exp,
                                             scale=a_sb[:, g, n:n + 1])
                        bt = snp.tile([128, HW], BF16, tag="bt",
                                      name="btd", bufs=3)
                        nc.vector.tensor_mul(bt, p_c[g][:, hs], brep)
                        h = snp.tile([128, HW], BF16, tag="h", bufs=2)
                        init = 0.0 if half == 0 else hcar[:, j, n:n + 1]
                        nc.vector.tensor_tensor_scan(h, da, bt,
                                                     initial=init,
                                                     op0=OP.mult,
                                                     op1=OP.add)
                        if half == 0:
                            nc.vector.tensor_copy(hcar[:, j, n:n + 1],
                                                  h[:, HW - 1:HW])
                        yn = snp.tile([128, HW], BF16, tag="yn", bufs=2)
                        nc.vector.tensor_mul(yn, h, crep)
                        for s in range(2):
                            nc.tensor.matmul(
                                py[j][:, s * 512:(s + 1) * 512], eyen_sb,
                                yn[:, s * 512:(s + 1) * 512],
                                start=False, stop=(n == DS - 1))
                # gate with silu(z); write into xc[g]'s tile (xc dead now)
                for j, g in enumerate(wave):
                    sgl = repp.tile([128, HW], BF16, tag="sgl", bufs=2)
                    nc.sync.dma_start(sgl, sg_d[g][:, hs])
                    nc.vector.tensor_mul(yf[g][:, hs], py[j], sgl)
            out_proj_half(half)

        # ---------------- final residual + LayerNorm ----------------------
        mvall2 = wp.tile([128, 2, NTT], F32, name="mvall2")
        nmr2 = wp.tile([128, NTT], F32, name="nmr2")

        def sum3(i):
            # natural-time tile i: dir0 rows i*128.. (gbuf piece by row range)
            r0 = i * 128
            ph0 = 0 if r0 < HW else 1
            s0 = lnp.tile([128, DM], BF16, tag="s0")
            nc.sync.dma_start(
                s0, gbuf_h[ph0][0, r0 - ph0 * HW: r0 - ph0 * HW + 128, :])
            # direction-1 partial is in flipped local order
            lr = L - (i + 1) * 128
            ph1 = 0 if lr < HW else 1
            s1 = lnp.tile([128, DM], BF16, tag="s1")
            nc.sync.dma_start(
                s1, gbuf_h[ph1][1, lr - ph1 * HW: lr - ph1 * HW + 128, :])
            xt2 = lnp.tile([128, DM], BF16, tag="xt2")
            nc.sync.dma_start(xt2, x_nat[i * 128:(i + 1) * 128, :])
            pj = pp.tile([128, DM], F32, tag="ps")
            nc.tensor.matmul(pj, eyej_sb, s1, start=True, stop=False)
            nc.tensor.matmul(pj, eyen_sb, s0, start=False, stop=False)
            nc.tensor.matmul(pj, eyen_sb, xt2, start=False, stop=True)
            return pj

        # sweep 1: stats for every tile (batched rstd afterwards)
        for i in range(NTT):
            pj = sum3(i)
            st = lnp.tile([128, 6], F32, tag="st2")
            nc.vector.bn_stats(st, pj)
            nc.vector.bn_aggr(mvall2[:, :, i], st)
        nc.scalar.activation(mvall2[:, 1, :], mvall2[:, 1, :], AF.Ln,
                             bias=eps_sb[:, 0:1])
        nc.scalar.activation(mvall2[:, 1, :], mvall2[:, 1, :], AF.Exp,
                             scale=-0.5)
        nc.vector.tensor_mul(nmr2, mvall2[:, 0, :], mvall2[:, 1, :])
        nc.vector.tensor_scalar_mul(nmr2, nmr2, -1.0)
        # sweep 2: rebuild the sum (3 cheap matmuls) and normalize from PSUM
        for i in range(NTT):
            pj = sum3(i)
            s = lnp.tile([128, DM], F32, tag="ssum")
            nc.scalar.activation(s, pj, AF.Identity,
                                 scale=mvall2[:, 1, i:i + 1],
                                 bias=nmr2[:, i:i + 1])
            nc.vector.tensor_mul(s, s, ln2w_sb)
            nc.vector.tensor_add(s, s, ln2b_sb)
            nc.sync.dma_start(out[i * 128:(i + 1) * 128, :], s)
        pp2_ctx.__exit__(None, None, None)
        ppy_ctx.__exit__(None, None, None)


# ---------------- host side ----------------

def make_core_inputs(inputs):
    """Build the 8 per-core input dicts from the full problem inputs."""
    x = np.ascontiguousarray(inputs["x"], dtype=np.float32)        # [B, L, DM]
    ln1_w = inputs["ln1_w"].astype(np.float32)
    ln1_b = inputs["ln1_b"].astype(np.float32)
    in_w = inputs["in_proj_w"].astype(np.float32)                  # [2, 2DI, DM]
    conv_w = inputs["conv_w"].astype(np.float32)                   # [2, DI, 4]
    conv_b = inputs["conv_b"].astype(np.float32)                   # [2, DI]
    xp_w = inputs["x_proj_w"].astype(np.float32)                   # [2, 64, DI]
    dt_w = inputs["dt_proj_w"].astype(np.float32)                  # [2, DI, DR]
    dt_b = inputs["dt_proj_b"].astype(np.float32)                  # [2, DI]
    a_log = inputs["A_log"].astype(np.float32)                     # [2, DI, DS]
    d_par = inputs["D_param"].astype(np.float32)                   # [2, DI]
    out_w = inputs["out_proj_w"].astype(np.float32)                # [2, DM, DI]
    ln2_w = inputs["ln2_w"].astype(np.float32)
    ln2_b = inputs["ln2_b"].astype(np.float32)

    bf = ml_dtypes.bfloat16
    eye = np.eye(128, dtype=np.float32)
    per_dir = []
    for d in range(2):
        w = in_w[d]                                   # [2DI, DM]
        w_in_T = np.ascontiguousarray((w * ln1_w[None, :]).T)      # [DM, 2DI]
        v = w @ ln1_b                                  # [2DI]
        csum = conv_w[d].sum(axis=1)                   # [DI]
        convb_adj = conv_b[d] + csum * v[:DI]
        silub_adj = v[DI:]
        convd = np.zeros((D_CONV, NG, 128, 128), np.float32)
        for k in range(D_CONV):
            for g in range(NG):
                np.fill_diagonal(convd[k, g], conv_w[d, g * 128:(g + 1) * 128, k])
        a_neg = -np.exp(a_log[d])                      # [DI, DS]
        per_dir.append(dict(
            w_in=w_in_T.astype(bf),
            convd=convd.astype(bf),
            convb=convb_adj.reshape(NG, 128),
            silub=silub_adj.reshape(NG, 128),
            w_xp=np.ascontiguousarray(xp_w[d].T).astype(bf),       # [DI, 64]
            w_dt=np.ascontiguousarray(dt_w[d].T).astype(bf),       # [DR, DI]
            dtb=dt_b[d].reshape(NG, 128),
            a_sc=np.ascontiguousarray(a_neg.reshape(NG, 128, DS)),
            d_vec=d_par[d].reshape(NG, 128),
            w_out=np.ascontiguousarray(out_w[d].T).astype(bf),     # [DI, DM]
        ))

    in_maps = []
    for core in range(N_CORES):
        d, b = core // 4, core % 4
        xb = x[b]
        m = dict(per_dir[d])
        m["x_d"] = (xb if d == 0 else np.ascontiguousarray(xb[::-1])).astype(bf)
        m["x_nat"] = xb.astype(bf)
        m["eye"] = eye
        m["eyen"] = eye.astype(bf)
        m["eyej"] = eye[::-1].copy().astype(bf)
        m["ln2w"] = ln2_w.reshape(1, DM)
        m["ln2b"] = ln2_b.reshape(1, DM)
        in_maps.append(m)
    return in_maps


_NC = None


def _get_module():
    global _NC
    if _NC is None:
        _NC = build_module()
    return _NC


def kernel(**inputs) -> np.ndarray:
    nc = _get_module()
    in_maps = make_core_inputs(inputs)
    res = run_bass_kernel_spmd(nc, in_maps, core_ids=list(range(N_CORES)))
    outs = [res.results[b]["out"] for b in range(B_SZ)]
    return np.stack(outs, axis=0)


if __name__ == "__main__":
    nc = build_module()
    print("module built ok")


# revision 23
# speedup vs baseline: 1.3188x; 1.0261x over previous
"""Bidirectional Mamba layer on 8 Trainium2 NeuronCores (Bass/Tile).

Sharding: 8 cores = 2 directions x 4 batch samples. Each core runs the full
Mamba block for its (direction, sample) pair; a pairwise AllGather combines
the two directions, and every core (redundantly, SPMD-symmetric) applies the
final residual + LayerNorm.

Layout: channels on partitions, time on the free axis, full L=2048 per op
(monolith — no time chunking, so the selective scan needs no state carry).
All matmuls run bf16 (single PE pass); elementwise work is bf16 for the DVE
2x mode. The per-(group, state) inner loop is:
  Act:  da = exp(dt * A[g,n])          (scalar engine)
  DVE/GPS: bt = p * B_n                (broadcast multiply)
  DVE:  h = tensor_tensor_scan(da, bt) (the irreducible 2 cyc/elem scan)
  DVE:  yn = h * C_n
  PE:   py += I @ yn                   (PSUM accumulation over states)
"""

import ml_dtypes
import numpy as np

import concourse.bass as bass
import concourse.bacc as bacc
import concourse.tile as tile
from concourse import mybir
from concourse.bass_utils import run_bass_kernel_spmd

# ---- problem shapes (hardcoded per contract) ----
B_SZ, L, DM = 4, 2048, 512
D_CONV, DS, DR = 4, 16, 32
DI = 1024                  # d_inner
E2 = 2 * DI                # in_proj rows
NG = DI // 128             # 8 channel blocks
NTT = L // 128             # 16 time tiles of 128
NSL = L // 512             # 4 psum-width slices
EPS = 1e-5
F32 = mybir.dt.float32
BF16 = mybir.dt.bfloat16
AF = mybir.ActivationFunctionType
OP = mybir.AluOpType

N_CORES = 8

# CoreSim does not implement Silu; tests flip this to use sigmoid+mult
USE_SILU = True
REPLICA_GROUPS = [[0, 4], [1, 5], [2, 6], [3, 7]]

# groups whose bt-/yn-multiply runs on GPSIMD instead of DVE.
# Empirically GPSIMD elementwise is net-negative here: it contends for the
# DVE's SBUF port and slows 2x-mode DVE ops more than it absorbs.
GPS_GS = ()
GPS_YS = ()


def _silu(nc, pool, out_tile, psum, bias=None):
    """out = silu(psum [+ bias]); Silu on HW, sigmoid+mult fallback for sim."""
    kw = {} if bias is None else {"bias": bias}
    if USE_SILU:
        nc.scalar.activation(out_tile, psum, AF.Silu, **kw)
    else:
        zb = pool.tile(list(out_tile.shape), F32, tag="_silu_zb", name="zb",
                       bufs=1)
        nc.scalar.activation(zb, psum, AF.Identity, **kw)
        nc.scalar.activation(out_tile, psum, AF.Sigmoid, **kw)
        nc.vector.tensor_mul(out_tile, zb, out_tile)


def build_module():
    nc = bacc.Bacc(
        "TRN2", target_bir_lowering=False, debug=False, num_devices=N_CORES
    )

    # ---------------- I/O ----------------
    x_d = nc.dram_tensor("x_d", [L, DM], BF16, kind="ExternalInput")
    x_nat = nc.dram_tensor("x_nat", [L, DM], BF16, kind="ExternalInput")
    w_in = nc.dram_tensor("w_in", [DM, E2], BF16, kind="ExternalInput")
    convw = nc.dram_tensor("convw", [NG, 128, D_CONV], F32, kind="ExternalInput")
    convb = nc.dram_tensor("convb", [NG, 128], F32, kind="ExternalInput")
    silub = nc.dram_tensor("silub", [NG, 128], F32, kind="ExternalInput")
    w_xp = nc.dram_tensor("w_xp", [DI, DR + 2 * DS], BF16, kind="ExternalInput")
    w_dt = nc.dram_tensor("w_dt", [DR, DI], BF16, kind="ExternalInput")
    dtb = nc.dram_tensor("dtb", [NG, 128], F32, kind="ExternalInput")
    a_sc = nc.dram_tensor("a_sc", [NG, 128, DS], F32, kind="ExternalInput")
    d_vec = nc.dram_tensor("d_vec", [NG, 128], F32, kind="ExternalInput")
    w_out = nc.dram_tensor("w_out", [DI, DM], BF16, kind="ExternalInput")
    eye = nc.dram_tensor("eye", [128, 128], F32, kind="ExternalInput")
    eyen = nc.dram_tensor("eyen", [128, 128], BF16, kind="ExternalInput")
    eyej = nc.dram_tensor("eyej", [128, 128], BF16, kind="ExternalInput")
    ln2w = nc.dram_tensor("ln2w", [1, DM], F32, kind="ExternalInput")
    ln2b = nc.dram_tensor("ln2b", [1, DM], F32, kind="ExternalInput")
    out = nc.dram_tensor("out", [L, DM], F32, kind="ExternalOutput")

    with tile.TileContext(nc) as tc:
        build_program(
            tc, x_d, x_nat, w_in, convw, convb, silub, w_xp, w_dt, dtb,
            a_sc, d_vec, w_out, eye, eyen, eyej, ln2w, ln2b, out,
        )
    nc.compile()
    return nc


def build_program(tc, x_d, x_nat, w_in, convw, convb, silub, w_xp, w_dt, dtb,
                  a_sc, d_vec, w_out, eye, eyen, eyej, ln2w, ln2b, out):
    nc = tc.nc
    from contextlib import ExitStack

    ctx = ExitStack()
    with ctx:
        wp = ctx.enter_context(tc.tile_pool(name="wp", bufs=1))
        lnp = ctx.enter_context(tc.tile_pool(name="lnp", bufs=2))
        big = ctx.enter_context(tc.tile_pool(name="big", bufs=1))
        snp = ctx.enter_context(tc.tile_pool(name="snp", bufs=3))
        repp = ctx.enter_context(tc.tile_pool(name="repp", bufs=3))
        dramp = ctx.enter_context(tc.tile_pool(name="dramp", bufs=2, space="DRAM"))
        pp_ctx = tc.tile_pool(name="pp", space="PSUM", bufs=2)
        pp = pp_ctx.__enter__()

        # x-load + LN1 stats first so its DMAs aren't queued behind weights
        eps_sb = wp.tile([128, 1], F32)
        nc.vector.memset(eps_sb, EPS)
        mvall = wp.tile([128, 2, NTT], F32, name="mvall")
        nmr = wp.tile([128, NTT], F32, name="nmr")
        xts = []
        for i in range(NTT):
            xt = lnp.tile([128, DM], BF16, tag="xt", bufs=NTT)
            nc.sync.dma_start(xt, x_d[i * 128:(i + 1) * 128, :])
            xts.append(xt)
            st = lnp.tile([128, 6], F32, tag="st")
            nc.vector.bn_stats(st, xt)
            nc.vector.bn_aggr(mvall[:, :, i], st)
        # rstd batches split 4/12 so the first tiles (which gate in_proj)
        # unblock as early as possible
        for lo, hi in ((0, 4), (4, NTT)):
            nc.scalar.activation(mvall[:, 1, lo:hi], mvall[:, 1, lo:hi],
                                 AF.Ln, bias=eps_sb[:, 0:1])
            nc.scalar.activation(mvall[:, 1, lo:hi], mvall[:, 1, lo:hi],
                                 AF.Exp, scale=-0.5)
            nc.vector.tensor_mul(nmr[:, lo:hi], mvall[:, 0, lo:hi],
                                 mvall[:, 1, lo:hi])
            nc.vector.tensor_scalar_mul(nmr[:, lo:hi], nmr[:, lo:hi], -1.0)

        w_in_sb = wp.tile([128, DM // 128, E2], BF16)
        for k in range(DM // 128):
            nc.sync.dma_start(w_in_sb[:, k, :], w_in[k * 128:(k + 1) * 128, :])
        convw_sb = wp.tile([128, NG, D_CONV], F32)
        for g in range(NG):
            nc.sync.dma_start(convw_sb[:, g, :], convw[g])
        w_xp_sb = wp.tile([128, NG, DR + 2 * DS], BF16)
        for g in range(NG):
            nc.sync.dma_start(w_xp_sb[:, g, :], w_xp[g * 128:(g + 1) * 128, :])
        w_dt_sb = wp.tile([DR, DI], BF16)
        nc.sync.dma_start(w_dt_sb, w_dt[:, :])
        w_out_sb = wp.tile([128, NG, DM], BF16)
        for g in range(NG):
            nc.sync.dma_start(w_out_sb[:, g, :], w_out[g * 128:(g + 1) * 128, :])
        a_sb = wp.tile([128, NG, DS], F32)
        for g in range(NG):
            nc.sync.dma_start(a_sb[:, g, :], a_sc[g])
        eye_sb = wp.tile([128, 128], F32)
        nc.sync.dma_start(eye_sb, eye[:, :])
        eyen_sb = wp.tile([128, 128], BF16)
        nc.sync.dma_start(eyen_sb, eyen[:, :])
        eyej_sb = wp.tile([128, 128], BF16)
        nc.sync.dma_start(eyej_sb, eyej[:, :])

        def col_load(name, src):
            t = wp.tile([128, NG], F32, name=name)
            for g in range(NG):
                nc.sync.dma_start(t[:, g:g + 1], src[g:g + 1, :].rearrange("a b -> b a"))
            return t

        convb_sb = col_load("convb_sb", convb)
        silub_sb = col_load("silub_sb", silub)
        dtb_sb = col_load("dtb_sb", dtb)
        d_sb = col_load("d_sb", d_vec)

        ln2w_sb = wp.tile([128, DM], F32)
        nc.sync.dma_start(ln2w_sb, ln2w[0:1, :].to_broadcast([128, DM]))
        ln2b_sb = wp.tile([128, DM], F32)
        nc.sync.dma_start(ln2b_sb, ln2b[0:1, :].to_broadcast([128, DM]))

        sg_d = dramp.tile([NG, 128, L], BF16, bufs=1)
        bc_d = dramp.tile([2 * DS, L], BF16, bufs=1)

        # ---------------- phase A: LayerNorm1 + transpose ----------------
        # xn_t: [dm-part, k-block, t] bf16, consumed by in_proj
        xn_t = big.tile([128, DM // 128, L], BF16, name="xn_t")
        # normalize sweep on the resident bf16 tiles (no reload)
        for i in range(NTT):
            xt = xts[i]
            nc.scalar.activation(xt, xt, AF.Identity,
                                 scale=mvall[:, 1, i:i + 1],
                                 bias=nmr[:, i:i + 1])
            for kb in range(DM // 128):
                ps_t = pp.tile([128, 128], BF16, tag="pst")
                nc.tensor.transpose(ps_t, xt[:, kb * 128:(kb + 1) * 128],
                                    eyen_sb)
                nc.vector.tensor_copy(xn_t[:, kb, i * 128:(i + 1) * 128],
                                      ps_t)

        # ---------------- phase B: in_proj ----------------
        # xc half into conv staging (3-col zero halo at front)
        xzf = [big.tile([128, L + 3], BF16, name=f"xzf{g}") for g in range(NG)]
        for g in range(NG):
            nc.vector.memset(xzf[g][:, 0:3], 0.0)
            for s in range(NSL):
                pz = pp.tile([128, 512], F32, tag="ps", bufs=3)
                for k in range(DM // 128):
                    nc.tensor.matmul(pz, w_in_sb[:, k, g * 128:(g + 1) * 128],
                                     xn_t[:, k, s * 512:(s + 1) * 512],
                                     start=(k == 0), stop=(k == DM // 128 - 1))
                nc.vector.tensor_copy(xzf[g][:, 3 + s * 512:3 + (s + 1) * 512], pz)

        # z half + SiLU gate -> sg (spilled to DRAM, reloaded at gate)
        for g in range(NG):
            for s in range(NSL):
                pz = pp.tile([128, 512], F32, tag="ps", bufs=3)
                for k in range(DM // 128):
                    nc.tensor.matmul(
                        pz, w_in_sb[:, k, DI + g * 128: DI + (g + 1) * 128],
                        xn_t[:, k, s * 512:(s + 1) * 512],
                        start=(k == 0), stop=(k == DM // 128 - 1))
                sgt = snp.tile([128, 512], BF16, tag="sgt", bufs=2)
                _silu(nc, snp, sgt, pz, silub_sb[:, g:g + 1])
                nc.sync.dma_start(sg_d[g][:, s * 512:(s + 1) * 512], sgt)

        # ---------------- phase C: depthwise causal conv + SiLU ----------
        # 4-tap conv as a chain of fused (x*w_k)+acc DVE ops (the DVE is idle
        # during this phase; this keeps the 128 diag-matmuls off the PE)
        xc = [big.tile([128, L], BF16, name=f"xc{g}") for g in range(NG)]
        for g in range(NG):
            acc = snp.tile([128, L], BF16, tag="cacc", bufs=2)
            nc.vector.tensor_scalar_mul(acc, xzf[g][:, 0:L],
                                        convw_sb[:, g, 0:1])
            for k in range(1, D_CONV):
                nc.vector.scalar_tensor_tensor(
                    acc, xzf[g][:, k:k + L], convw_sb[:, g, k:k + 1], acc,
                    op0=OP.mult, op1=OP.add)
            _silu(nc, snp, xc[g], acc, convb_sb[:, g:g + 1])

        # ---------------- phase D: x_proj -> (dt_r, B, C) -----------------
        xdbl = big.tile([64, L], BF16, name="xdbl")
        for s in range(NSL):
            pxd = pp.tile([64, 512], F32, tag="pxd", bufs=1)
            for g in range(NG):
                nc.tensor.matmul(pxd, w_xp_sb[:, g, :],
                                 xc[g][:, s * 512:(s + 1) * 512],
                                 start=(g == 0), stop=(g == NG - 1))
            nc.vector.tensor_copy(xdbl[:, s * 512:(s + 1) * 512], pxd)
            nc.sync.dma_start(bc_d[:, s * 512:(s + 1) * 512],
                              xdbl[DR:DR + 2 * DS, s * 512:(s + 1) * 512])

        # dt = softplus(dt_proj @ dt_r + bias); p = dt * xc
        # dt_c[0..3] reuse xn_t's space, dt_c[4..7] reuse w_in_sb (both dead
        # after in_proj); p_c reuses the conv staging xzf (dead after conv)
        dt_c = [xn_t[:, g, :] if g < 4 else w_in_sb[:, g - 4, :]
                for g in range(NG)]
        p_c = [xzf[g][:, 0:L] for g in range(NG)]
        def dt_exp(g):
            for s in range(NSL):
                sl = slice(s * 512, (s + 1) * 512)
                pdt = pp.tile([128, 512], F32, tag="ps", bufs=3)
                nc.tensor.matmul(pdt, w_dt_sb[:, g * 128:(g + 1) * 128],
                                 xdbl[0:DR, sl],
                                 start=True, stop=True)
                nc.scalar.activation(dt_c[g][:, sl], pdt, AF.Exp,
                                     bias=dtb_sb[:, g:g + 1])

        def dt_ln_p(g):
            nc.scalar.activation(dt_c[g], dt_c[g], AF.Ln, bias=1.0)
            nc.vector.tensor_mul(p_c[g], dt_c[g], xc[g])

        # group 0 fully first so phase E's first wave unblocks early
        dt_exp(0)
        dt_ln_p(0)
        for g in range(1, NG):
            dt_exp(g)
        for g in range(1, NG):
            dt_ln_p(g)

        # ---------------- phase E: selective scan ------------------------
        # waves of 3 groups share each B/C broadcast; time in halves of 1024.
        # py uses 6 PSUM banks, leaving 2 for out_proj so each half's
        # out_proj + collective piece overlaps the next half's compute.
        pp_ctx.__exit__(None, None, None)
        HW = L // 2
        WAVES = [(0, 1, 2), (3, 4, 5), (6, 7)]
        yf = [xc[g] for g in range(NG)]
        ypart_h = [dramp.tile([HW, DM], BF16, bufs=1, name=f"yph{h}")
                   for h in range(2)]
        gbuf_h = [dramp.tile([2, HW, DM], BF16, bufs=1, name=f"gbh{h}")
                  for h in range(2)]
        ppy_ctx = tc.tile_pool(name="ppy", space="PSUM", bufs=1)
        ppy = ppy_ctx.__enter__()
        pp2_ctx = tc.tile_pool(name="pp2", space="PSUM", bufs=2)
        pp = pp2_ctx.__enter__()
        hcars = [snp.tile([128, 4, DS], BF16, tag="hcar", bufs=3,
                          name=f"hcar{w}") for w in range(3)]

        def out_proj_half(half):
            hs = slice(half * HW, (half + 1) * HW)
            for tb in range(8):
                t0 = half * HW + tb * 128
                po = pp.tile([128, DM], F32, tag="ps")
                for g in range(NG):
                    nc.tensor.matmul(po, yf[g][:, t0:t0 + 128],
                                     w_out_sb[:, g, :],
                                     start=(g == 0), stop=(g == NG - 1))
                ot = lnp.tile([128, DM], BF16, tag="ot")
                nc.scalar.copy(ot, po)
                nc.sync.dma_start(ypart_h[half][tb * 128:(tb + 1) * 128, :], ot)
            nc.gpsimd.collective_compute(
                "AllGather", OP.bypass, replica_groups=REPLICA_GROUPS,
                ins=[ypart_h[half].opt()], outs=[gbuf_h[half].opt()],
            )

        for half in (0, 1):
            hs = slice(half * HW, (half + 1) * HW)
            for wv, wave in enumerate(WAVES):
                hcar = hcars[wv]
                py = [ppy.tile([128, HW], F32, tag=f"py{j}", bufs=1,
                               name=f"py{j}") for j in range(len(wave))]
                for j, g in enumerate(wave):
                    yd = snp.tile([128, HW], BF16, tag="yd", bufs=2)
                    nc.vector.tensor_scalar_mul(yd, xc[g][:, hs],
                                                d_sb[:, g:g + 1])
                    for s in range(2):
                        nc.tensor.matmul(py[j][:, s * 512:(s + 1) * 512],
                                         eyen_sb,
                                         yd[:, s * 512:(s + 1) * 512],
                                         start=True, stop=False)
                for n in range(DS):
                    brep = repp.tile([128, HW], BF16, tag="brep", bufs=3)
                    nc.sync.dma_start(
                        brep, bc_d[n:n + 1, hs].to_broadcast([128, HW]))
                    crep = repp.tile([128, HW], BF16, tag="crep", bufs=3)
                    nc.sync.dma_start(
                        crep,
                        bc_d[DS + n:DS + n + 1, hs].to_broadcast([128, HW]))
                    for j, g in enumerate(wave):
                        da = snp.tile([128, HW], BF16, tag="da", bufs=3)
                        nc.scalar.activation(da, dt_c[g][:, hs], AF.Exp,
                                             scale=a_sb[:, g, n:n + 1])
                        bt = snp.tile([128, HW], BF16, tag="bt",
                                      name="btd", bufs=3)
                        nc.vector.tensor_mul(bt, p_c[g][:, hs], brep)
                        h = snp.tile([128, HW], BF16, tag="h", bufs=2)
                        init = 0.0 if half == 0 else hcar[:, j, n:n + 1]
                        nc.vector.tensor_tensor_scan(h, da, bt,
                                                     initial=init,
                                                     op0=OP.mult,
                                                     op1=OP.add)
                        if half == 0:
                            nc.vector.tensor_copy(hcar[:, j, n:n + 1],
                                                  h[:, HW - 1:HW])
                        yn = snp.tile([128, HW], BF16, tag="yn", bufs=2)
                        nc.vector.tensor_mul(yn, h, crep)
                        for s in range(2):
                            nc.tensor.matmul(
                                py[j][:, s * 512:(s + 1) * 512], eyen_sb,
                                yn[:, s * 512:(s + 1) * 512],
                                start=False, stop=(n == DS - 1))
                # gate with silu(z); write into xc[g]'s tile (xc dead now)
                for j, g in enumerate(wave):
                    sgl = repp.tile([128, HW], BF16, tag="sgl", bufs=2)
                    nc.sync.dma_start(sgl, sg_d[g][:, hs])
                    nc.vector.tensor_mul(yf[g][:, hs], py[j], sgl)
            out_proj_half(half)

        # ---------------- final residual + LayerNorm ----------------------
        mvall2 = wp.tile([128, 2, NTT], F32, name="mvall2")
        nmr2 = wp.tile([128, NTT], F32, name="nmr2")

        def sum3(i):
            # natural-time tile i: dir0 rows i*128.. (gbuf piece by row range)
            r0 = i * 128
            ph0 = 0 if r0 < HW else 1
            s0 = lnp.tile([128, DM], BF16, tag="s0")
            nc.sync.dma_start(
                s0, gbuf_h[ph0][0, r0 - ph0 * HW: r0 - ph0 * HW + 128, :])
            # direction-1 partial is in flipped local order
            lr = L - (i + 1) * 128
            ph1 = 0 if lr < HW else 1
            s1 = lnp.tile([128, DM], BF16, tag="s1")
            nc.sync.dma_start(
                s1, gbuf_h[ph1][1, lr - ph1 * HW: lr - ph1 * HW + 128, :])
            xt2 = lnp.tile([128, DM], BF16, tag="xt2")
            nc.sync.dma_start(xt2, x_nat[i * 128:(i + 1) * 128, :])
            pj = pp.tile([128, DM], F32, tag="ps")
            nc.tensor.matmul(pj, eyej_sb, s1, start=True, stop=False)
            nc.tensor.matmul(pj, eyen_sb, s0, start=False, stop=False)
            nc.tensor.matmul(pj, eyen_sb, xt2, start=False, stop=True)
            return pj

        # sweep 1: stats for every tile (batched rstd afterwards)
        for i in range(NTT):
            pj = sum3(i)
            st = lnp.tile([128, 6], F32, tag="st2")
            nc.vector.bn_stats(st, pj)
            nc.vector.bn_aggr(mvall2[:, :, i], st)
        nc.scalar.activation(mvall2[:, 1, :], mvall2[:, 1, :], AF.Ln,
                             bias=eps_sb[:, 0:1])
        nc.scalar.activation(mvall2[:, 1, :], mvall2[:, 1, :], AF.Exp,
                             scale=-0.5)
        nc.vector.tensor_mul(nmr2, mvall2[:, 0, :], mvall2[:, 1, :])
        nc.vector.tensor_scalar_mul(nmr2, nmr2, -1.0)
        # sweep 2: rebuild the sum (3 cheap matmuls) and normalize from PSUM
        for i in range(NTT):
            pj = sum3(i)
            s = lnp.tile([128, DM], F32, tag="ssum")
            nc.scalar.activation(s, pj, AF.Identity,
                                 scale=mvall2[:, 1, i:i + 1],
                                 bias=nmr2[:, i:i + 1])
            nc.vector.tensor_mul(s, s, ln2w_sb)
            nc.vector.tensor_add(s, s, ln2b_sb)
            nc.sync.dma_start(out[i * 128:(i + 1) * 128, :], s)
        pp2_ctx.__exit__(None, None, None)
        ppy_ctx.__exit__(None, None, None)


# ---------------- host side ----------------

def make_core_inputs(inputs):
    """Build the 8 per-core input dicts from the full problem inputs."""
    x = np.ascontiguousarray(inputs["x"], dtype=np.float32)        # [B, L, DM]
    ln1_w = inputs["ln1_w"].astype(np.float32)
    ln1_b = inputs["ln1_b"].astype(np.float32)
    in_w = inputs["in_proj_w"].astype(np.float32)                  # [2, 2DI, DM]
    conv_w = inputs["conv_w"].astype(np.float32)                   # [2, DI, 4]
    conv_b = inputs["conv_b"].astype(np.float32)                   # [2, DI]
    xp_w = inputs["x_proj_w"].astype(np.float32)                   # [2, 64, DI]
    dt_w = inputs["dt_proj_w"].astype(np.float32)                  # [2, DI, DR]
    dt_b = inputs["dt_proj_b"].astype(np.float32)                  # [2, DI]
    a_log = inputs["A_log"].astype(np.float32)                     # [2, DI, DS]
    d_par = inputs["D_param"].astype(np.float32)                   # [2, DI]
    out_w = inputs["out_proj_w"].astype(np.float32)                # [2, DM, DI]
    ln2_w = inputs["ln2_w"].astype(np.float32)
    ln2_b = inputs["ln2_b"].astype(np.float32)

    bf = ml_dtypes.bfloat16
    eye = np.eye(128, dtype=np.float32)
    per_dir = []
    for d in range(2):
        w = in_w[d]                                   # [2DI, DM]
        w_in_T = np.ascontiguousarray((w * ln1_w[None, :]).T)      # [DM, 2DI]
        v = w @ ln1_b                                  # [2DI]
        csum = conv_w[d].sum(axis=1)                   # [DI]
        convb_adj = conv_b[d] + csum * v[:DI]
        silub_adj = v[DI:]
        a_neg = -np.exp(a_log[d])                      # [DI, DS]
        per_dir.append(dict(
            w_in=w_in_T.astype(bf),
            convw=np.ascontiguousarray(conv_w[d].reshape(NG, 128, D_CONV)),
            convb=convb_adj.reshape(NG, 128),
            silub=silub_adj.reshape(NG, 128),
            w_xp=np.ascontiguousarray(xp_w[d].T).astype(bf),       # [DI, 64]
            w_dt=np.ascontiguousarray(dt_w[d].T).astype(bf),       # [DR, DI]
            dtb=dt_b[d].reshape(NG, 128),
            a_sc=np.ascontiguousarray(a_neg.reshape(NG, 128, DS)),
            d_vec=d_par[d].reshape(NG, 128),
            w_out=np.ascontiguousarray(out_w[d].T).astype(bf),     # [DI, DM]
        ))

    in_maps = []
    for core in range(N_CORES):
        d, b = core // 4, core % 4
        xb = x[b]
        m = dict(per_dir[d])
        m["x_d"] = (xb if d == 0 else np.ascontiguousarray(xb[::-1])).astype(bf)
        m["x_nat"] = xb.astype(bf)
        m["eye"] = eye
        m["eyen"] = eye.astype(bf)
        m["eyej"] = eye[::-1].copy().astype(bf)
        m["ln2w"] = ln2_w.reshape(1, DM)
        m["ln2b"] = ln2_b.reshape(1, DM)
        in_maps.append(m)
    return in_maps


_NC = None


def _get_module():
    global _NC
    if _NC is None:
        _NC = build_module()
    return _NC


def kernel(**inputs) -> np.ndarray:
    nc = _get_module()
    in_maps = make_core_inputs(inputs)
    res = run_bass_kernel_spmd(nc, in_maps, core_ids=list(range(N_CORES)))
    outs = [res.results[b]["out"] for b in range(B_SZ)]
    return np.stack(outs, axis=0)


if __name__ == "__main__":
    nc = build_module()
    print("module built ok")


# revision 31
# speedup vs baseline: 1.3418x; 1.0174x over previous
"""Bidirectional Mamba layer on 8 Trainium2 NeuronCores (Bass/Tile).

Sharding: 8 cores = 2 directions x 4 batch samples. Each core runs the full
Mamba block for its (direction, sample) pair; a pairwise AllGather combines
the two directions, and every core (redundantly, SPMD-symmetric) applies the
final residual + LayerNorm.

Layout: channels on partitions, time on the free axis, full L=2048 per op
(monolith — no time chunking, so the selective scan needs no state carry).
All matmuls run bf16 (single PE pass); elementwise work is bf16 for the DVE
2x mode. The per-(group, state) inner loop is:
  Act:  da = exp(dt * A[g,n])          (scalar engine)
  DVE/GPS: bt = p * B_n                (broadcast multiply)
  DVE:  h = tensor_tensor_scan(da, bt) (the irreducible 2 cyc/elem scan)
  DVE:  yn = h * C_n
  PE:   py += I @ yn                   (PSUM accumulation over states)
"""

import ml_dtypes
import numpy as np

import concourse.bass as bass
import concourse.bacc as bacc
import concourse.tile as tile
from concourse import mybir
from concourse.bass_utils import run_bass_kernel_spmd

# ---- problem shapes (hardcoded per contract) ----
B_SZ, L, DM = 4, 2048, 512
D_CONV, DS, DR = 4, 16, 32
DI = 1024                  # d_inner
E2 = 2 * DI                # in_proj rows
NG = DI // 128             # 8 channel blocks
NTT = L // 128             # 16 time tiles of 128
NSL = L // 512             # 4 psum-width slices
EPS = 1e-5
F32 = mybir.dt.float32
BF16 = mybir.dt.bfloat16
AF = mybir.ActivationFunctionType
OP = mybir.AluOpType

N_CORES = 8

# CoreSim does not implement Silu; tests flip this to use sigmoid+mult
USE_SILU = True
REPLICA_GROUPS = [[0, 4], [1, 5], [2, 6], [3, 7]]

# groups whose bt-/yn-multiply runs on GPSIMD instead of DVE.
# Empirically GPSIMD elementwise is net-negative here: it contends for the
# DVE's SBUF port and slows 2x-mode DVE ops more than it absorbs.
GPS_GS = ()
GPS_YS = ()


def _silu(nc, pool, out_tile, psum, bias=None):
    """out = silu(psum [+ bias]); Silu on HW, sigmoid+mult fallback for sim."""
    kw = {} if bias is None else {"bias": bias}
    if USE_SILU:
        nc.scalar.activation(out_tile, psum, AF.Silu, **kw)
    else:
        zb = pool.tile(list(out_tile.shape), F32, tag="_silu_zb", name="zb",
                       bufs=1)
        nc.scalar.activation(zb, psum, AF.Identity, **kw)
        nc.scalar.activation(out_tile, psum, AF.Sigmoid, **kw)
        nc.vector.tensor_mul(out_tile, zb, out_tile)


def build_module():
    nc = bacc.Bacc(
        "TRN2", target_bir_lowering=False, debug=False, num_devices=N_CORES
    )

    # ---------------- I/O ----------------
    x_d = nc.dram_tensor("x_d", [L, DM], BF16, kind="ExternalInput")
    x_nat = nc.dram_tensor("x_nat", [L, DM], BF16, kind="ExternalInput")
    w_in = nc.dram_tensor("w_in", [DM, E2], BF16, kind="ExternalInput")
    convw = nc.dram_tensor("convw", [NG, 128, D_CONV], F32, kind="ExternalInput")
    convb = nc.dram_tensor("convb", [NG, 128], F32, kind="ExternalInput")
    silub = nc.dram_tensor("silub", [NG, 128], F32, kind="ExternalInput")
    w_xp = nc.dram_tensor("w_xp", [DI, DR + 2 * DS], BF16, kind="ExternalInput")
    w_dt = nc.dram_tensor("w_dt", [DR, DI], BF16, kind="ExternalInput")
    dtb = nc.dram_tensor("dtb", [NG, 128], F32, kind="ExternalInput")
    a_sc = nc.dram_tensor("a_sc", [NG, 128, DS], F32, kind="ExternalInput")
    d_vec = nc.dram_tensor("d_vec", [NG, 128], F32, kind="ExternalInput")
    w_out = nc.dram_tensor("w_out", [DI, DM], BF16, kind="ExternalInput")
    eye = nc.dram_tensor("eye", [128, 128], F32, kind="ExternalInput")
    eyen = nc.dram_tensor("eyen", [128, 128], BF16, kind="ExternalInput")
    eyej = nc.dram_tensor("eyej", [128, 128], BF16, kind="ExternalInput")
    ln2w = nc.dram_tensor("ln2w", [1, DM], F32, kind="ExternalInput")
    ln2b = nc.dram_tensor("ln2b", [1, DM], F32, kind="ExternalInput")
    out = nc.dram_tensor("out", [L, DM], F32, kind="ExternalOutput")

    with tile.TileContext(nc) as tc:
        build_program(
            tc, x_d, x_nat, w_in, convw, convb, silub, w_xp, w_dt, dtb,
            a_sc, d_vec, w_out, eye, eyen, eyej, ln2w, ln2b, out,
        )
    nc.compile()
    return nc


def build_program(tc, x_d, x_nat, w_in, convw, convb, silub, w_xp, w_dt, dtb,
                  a_sc, d_vec, w_out, eye, eyen, eyej, ln2w, ln2b, out):
    nc = tc.nc
    HW = L // 2
    from contextlib import ExitStack

    ctx = ExitStack()
    with ctx:
        wp = ctx.enter_context(tc.tile_pool(name="wp", bufs=1))
        lnp = ctx.enter_context(tc.tile_pool(name="lnp", bufs=2))
        big = ctx.enter_context(tc.tile_pool(name="big", bufs=1))
        snp = ctx.enter_context(tc.tile_pool(name="snp", bufs=3))
        repp = ctx.enter_context(tc.tile_pool(name="repp", bufs=3))
        dramp = ctx.enter_context(tc.tile_pool(name="dramp", bufs=2, space="DRAM"))
        pp_ctx = tc.tile_pool(name="pp", space="PSUM", bufs=2)
        pp = pp_ctx.__enter__()

        # x-load + LN1 stats first so its DMAs aren't queued behind weights
        eps_sb = wp.tile([128, 1], F32)
        nc.vector.memset(eps_sb, EPS)
        mvall = wp.tile([128, 2, NTT], F32, name="mvall")
        nmr = wp.tile([128, NTT], F32, name="nmr")
        xts = []
        for i in range(NTT):
            xt = lnp.tile([128, DM], BF16, tag="xt", bufs=NTT)
            nc.sync.dma_start(xt, x_d[i * 128:(i + 1) * 128, :])
            xts.append(xt)
            st = lnp.tile([128, 6], F32, tag="st")
            nc.vector.bn_stats(st, xt)
            nc.vector.bn_aggr(mvall[:, :, i], st)
        # rstd batches split 4/12 so the first tiles (which gate in_proj)
        # unblock as early as possible
        for lo, hi in ((0, 4), (4, NTT)):
            nc.scalar.activation(mvall[:, 1, lo:hi], mvall[:, 1, lo:hi],
                                 AF.Ln, bias=eps_sb[:, 0:1])
            nc.scalar.activation(mvall[:, 1, lo:hi], mvall[:, 1, lo:hi],
                                 AF.Exp, scale=-0.5)
            nc.vector.tensor_mul(nmr[:, lo:hi], mvall[:, 0, lo:hi],
                                 mvall[:, 1, lo:hi])
            nc.vector.tensor_scalar_mul(nmr[:, lo:hi], nmr[:, lo:hi], -1.0)

        w_in_sb = wp.tile([128, DM // 128, E2], BF16)
        for k in range(DM // 128):
            nc.sync.dma_start(w_in_sb[:, k, :], w_in[k * 128:(k + 1) * 128, :])
        convw_sb = wp.tile([128, NG, D_CONV], F32)
        for g in range(NG):
            nc.sync.dma_start(convw_sb[:, g, :], convw[g])
        w_xp_sb = wp.tile([128, NG, DR + 2 * DS], BF16)
        for g in range(NG):
            nc.sync.dma_start(w_xp_sb[:, g, :], w_xp[g * 128:(g + 1) * 128, :])
        w_dt_sb = wp.tile([DR, DI], BF16)
        nc.sync.dma_start(w_dt_sb, w_dt[:, :])
        w_out_sb = wp.tile([128, NG, DM], BF16)
        for g in range(NG):
            nc.sync.dma_start(w_out_sb[:, g, :], w_out[g * 128:(g + 1) * 128, :])
        a_sb = wp.tile([128, NG, DS], F32)
        for g in range(NG):
            nc.sync.dma_start(a_sb[:, g, :], a_sc[g])
        eye_sb = wp.tile([128, 128], F32)
        nc.sync.dma_start(eye_sb, eye[:, :])
        eyen_sb = wp.tile([128, 128], BF16)
        nc.sync.dma_start(eyen_sb, eyen[:, :])
        eyej_sb = wp.tile([128, 128], BF16)
        nc.sync.dma_start(eyej_sb, eyej[:, :])

        def col_load(name, src):
            t = wp.tile([128, NG], F32, name=name)
            for g in range(NG):
                nc.sync.dma_start(t[:, g:g + 1], src[g:g + 1, :].rearrange("a b -> b a"))
            return t

        convb_sb = col_load("convb_sb", convb)
        silub_sb = col_load("silub_sb", silub)
        dtb_sb = col_load("dtb_sb", dtb)
        d_sb = col_load("d_sb", d_vec)

        ln2w_sb = wp.tile([128, DM], F32)
        nc.sync.dma_start(ln2w_sb, ln2w[0:1, :].to_broadcast([128, DM]))
        ln2b_sb = wp.tile([128, DM], F32)
        nc.sync.dma_start(ln2b_sb, ln2b[0:1, :].to_broadcast([128, DM]))

        sg_d = dramp.tile([NG, 128, L], BF16, bufs=1)
        bc_d = dramp.tile([2 * DS, L], BF16, bufs=1)

        # ---------------- phase A: LayerNorm1 + transpose ----------------
        # xn_t: [dm-part, k-block, t] bf16, consumed by in_proj
        xn_t = big.tile([128, DM // 128, L], BF16, name="xn_t")
        # normalize sweep on the resident bf16 tiles (no reload)
        for i in range(NTT):
            xt = xts[i]
            nc.scalar.activation(xt, xt, AF.Identity,
                                 scale=mvall[:, 1, i:i + 1],
                                 bias=nmr[:, i:i + 1])
            for kb in range(DM // 128):
                ps_t = pp.tile([128, 128], BF16, tag="pst")
                nc.tensor.transpose(ps_t, xt[:, kb * 128:(kb + 1) * 128],
                                    eyen_sb)
                nc.vector.tensor_copy(xn_t[:, kb, i * 128:(i + 1) * 128],
                                      ps_t)

        # ---------------- phase B: in_proj ----------------
        # xc half into conv staging (3-col zero halo at front)
        xzf = [big.tile([128, L + 3], BF16, name=f"xzf{g}") for g in range(NG)]
        for g in range(NG):
            nc.vector.memset(xzf[g][:, 0:3], 0.0)
            for s in range(NSL):
                pz = pp.tile([128, 512], F32, tag="ps", bufs=3)
                for k in range(DM // 128):
                    nc.tensor.matmul(pz, w_in_sb[:, k, g * 128:(g + 1) * 128],
                                     xn_t[:, k, s * 512:(s + 1) * 512],
                                     start=(k == 0), stop=(k == DM // 128 - 1))
                nc.vector.tensor_copy(xzf[g][:, 3 + s * 512:3 + (s + 1) * 512], pz)

        # z half + SiLU gate -> sg (spilled to DRAM, reloaded at gate)
        for g in range(NG):
            for s in range(NSL):
                pz = pp.tile([128, 512], F32, tag="ps", bufs=3)
                for k in range(DM // 128):
                    nc.tensor.matmul(
                        pz, w_in_sb[:, k, DI + g * 128: DI + (g + 1) * 128],
                        xn_t[:, k, s * 512:(s + 1) * 512],
                        start=(k == 0), stop=(k == DM // 128 - 1))
                sgt = snp.tile([128, 512], BF16, tag="sgt", bufs=2)
                _silu(nc, snp, sgt, pz, silub_sb[:, g:g + 1])
                nc.sync.dma_start(sg_d[g][:, s * 512:(s + 1) * 512], sgt)

        # ---------------- phase C: depthwise causal conv + SiLU ----------
        # 4-tap conv as a chain of fused (x*w_k)+acc DVE ops (the DVE is idle
        # during this phase; this keeps the 128 diag-matmuls off the PE)
        xc = [big.tile([128, L], BF16, name=f"xc{g}") for g in range(NG)]
        for g in range(NG):
            # conv accumulators borrow the phase-E da/bt tag space
            accs = [snp.tile([128, 2, HW], BF16, tag=t, bufs=2,
                             name=f"cacc_{t}").rearrange("p a b -> p (a b)")
                    for t in ("da", "bt")]
            nc.vector.tensor_scalar_mul(accs[0], xzf[g][:, 0:L],
                                        convw_sb[:, g, 0:1])
            for k in range(1, D_CONV):
                nc.vector.scalar_tensor_tensor(
                    accs[k % 2], xzf[g][:, k:k + L], convw_sb[:, g, k:k + 1],
                    accs[(k + 1) % 2], op0=OP.mult, op1=OP.add)
            _silu(nc, snp, xc[g], accs[(D_CONV - 1) % 2],
                  convb_sb[:, g:g + 1])

        # ---------------- phase D: x_proj -> (dt_r, B, C) -----------------
        xdbl = big.tile([64, L], BF16, name="xdbl")
        for s in range(NSL):
            pxd = pp.tile([64, 512], F32, tag="pxd", bufs=1)
            for g in range(NG):
                nc.tensor.matmul(pxd, w_xp_sb[:, g, :],
                                 xc[g][:, s * 512:(s + 1) * 512],
                                 start=(g == 0), stop=(g == NG - 1))
            nc.vector.tensor_copy(xdbl[:, s * 512:(s + 1) * 512], pxd)
            nc.sync.dma_start(bc_d[:, s * 512:(s + 1) * 512],
                              xdbl[DR:DR + 2 * DS, s * 512:(s + 1) * 512])

        # dt = softplus(dt_proj @ dt_r + bias); p = dt * xc
        # dt_c[0..3] reuse xn_t's space, dt_c[4..7] reuse w_in_sb (both dead
        # after in_proj); p_c reuses the conv staging xzf (dead after conv)
        dt_c = [xn_t[:, g, :] if g < 4 else w_in_sb[:, g - 4, :]
                for g in range(NG)]
        p_c = [xzf[g][:, 0:L] for g in range(NG)]
        def dt_exp(g):
            for s in range(NSL):
                sl = slice(s * 512, (s + 1) * 512)
                pdt = pp.tile([128, 512], F32, tag="ps", bufs=3)
                nc.tensor.matmul(pdt, w_dt_sb[:, g * 128:(g + 1) * 128],
                                 xdbl[0:DR, sl],
                                 start=True, stop=True)
                nc.scalar.activation(dt_c[g][:, sl], pdt, AF.Exp,
                                     bias=dtb_sb[:, g:g + 1])

        def dt_ln_p(g):
            nc.scalar.activation(dt_c[g], dt_c[g], AF.Ln, bias=1.0)
            nc.vector.tensor_mul(p_c[g], dt_c[g], xc[g])

        # group 0 fully first so phase E's first wave unblocks early
        dt_exp(0)
        dt_ln_p(0)
        for g in range(1, NG):
            dt_exp(g)
        for g in range(1, NG):
            dt_ln_p(g)

        # ---------------- phase E: selective scan ------------------------
        # waves of 3 groups share each B/C broadcast; time in halves of 1024.
        # py uses 6 PSUM banks, leaving 2 for out_proj so each half's
        # out_proj + collective piece overlaps the next half's compute.
        pp_ctx.__exit__(None, None, None)
        WAVES = [(0, 1, 2), (3, 4, 5), (6, 7)]
        yf = [xc[g] for g in range(NG)]
        ypart_h = [dramp.tile([HW, DM], BF16, bufs=1, name=f"yph{h}")
                   for h in range(2)]
        gbuf_h = [dramp.tile([2, HW, DM], BF16, bufs=1, name=f"gbh{h}")
                  for h in range(2)]
        ppy_ctx = tc.tile_pool(name="ppy", space="PSUM", bufs=1)
        ppy = ppy_ctx.__enter__()
        pp2_ctx = tc.tile_pool(name="pp2", space="PSUM", bufs=2)
        pp = pp2_ctx.__enter__()
        hcars = [snp.tile([128, 4, DS], BF16, tag="hcar", bufs=3,
                          name=f"hcar{w}") for w in range(3)]

        def out_proj_half(half):
            hs = slice(half * HW, (half + 1) * HW)
            for tb in range(8):
                t0 = half * HW + tb * 128
                po = pp.tile([128, DM], F32, tag="ps")
                for g in range(NG):
                    nc.tensor.matmul(po, yf[g][:, t0:t0 + 128],
                                     w_out_sb[:, g, :],
                                     start=(g == 0), stop=(g == NG - 1))
                ot = lnp.tile([128, DM], BF16, tag="ot")
                nc.scalar.copy(ot, po)
                nc.sync.dma_start(ypart_h[half][tb * 128:(tb + 1) * 128, :], ot)
            nc.gpsimd.collective_compute(
                "AllGather", OP.bypass, replica_groups=REPLICA_GROUPS,
                ins=[ypart_h[half].opt()], outs=[gbuf_h[half].opt()],
            )

        for half in (0, 1):
            hs = slice(half * HW, (half + 1) * HW)
            for wv, wave in enumerate(WAVES):
                hcar = hcars[wv]
                py = [ppy.tile([128, HW], F32, tag=f"py{j}", bufs=1,
                               name=f"py{j}") for j in range(len(wave))]
                for j, g in enumerate(wave):
                    yd = snp.tile([128, HW], BF16, tag="yd", bufs=2)
                    nc.vector.tensor_scalar_mul(yd, xc[g][:, hs],
                                                d_sb[:, g:g + 1])
                    for s in range(2):
                        nc.tensor.matmul(py[j][:, s * 512:(s + 1) * 512],
                                         eyen_sb,
                                         yd[:, s * 512:(s + 1) * 512],
                                         start=True, stop=False)
                for n0 in range(0, DS, 2):
                    # paired states: one broadcast + one mul covers n0, n0+1
                    brep = repp.tile([128, 2, HW], BF16, tag="brep", bufs=2)
                    nc.sync.dma_start(
                        brep, bc_d[n0:n0 + 2, hs].rearrange("a b -> () a b").to_broadcast([128, 2, HW]))
                    crep = repp.tile([128, 2, HW], BF16, tag="crep", bufs=2)
                    nc.sync.dma_start(
                        crep,
                        bc_d[DS + n0:DS + n0 + 2, hs].rearrange("a b -> () a b").to_broadcast([128, 2, HW]))
                    for j, g in enumerate(wave):
                        da = snp.tile([128, 2, HW], BF16, tag="da", bufs=2)
                        for q in range(2):
                            nc.scalar.activation(da[:, q, :], dt_c[g][:, hs],
                                                 AF.Exp,
                                                 scale=a_sb[:, g, n0 + q:n0 + q + 1])
                        bt = snp.tile([128, 2, HW], BF16, tag="bt",
                                      name="btd", bufs=2)
                        nc.vector.tensor_mul(
                            bt,
                            p_c[g][:, hs].rearrange("p b -> p () b")
                            .to_broadcast([128, 2, HW]),
                            brep)
                        h = snp.tile([128, 2, HW], BF16, tag="h", bufs=2)
                        for q in range(2):
                            n = n0 + q
                            init = 0.0 if half == 0 else hcar[:, j, n:n + 1]
                            nc.vector.tensor_tensor_scan(h[:, q, :],
                                                         da[:, q, :],
                                                         bt[:, q, :],
                                                         initial=init,
                                                         op0=OP.mult,
                                                         op1=OP.add)
                        if half == 0:
                            nc.vector.tensor_copy(hcar[:, j, n0:n0 + 2],
                                                      h[:, :, HW - 1])
                        yn = snp.tile([128, 2, HW], BF16, tag="yn", bufs=2)
                        nc.vector.tensor_mul(yn, h, crep)
                        for q in range(2):
                            for s in range(2):
                                nc.tensor.matmul(
                                    py[j][:, s * 512:(s + 1) * 512], eyen_sb,
                                    yn[:, q, s * 512:(s + 1) * 512],
                                    start=False,
                                    stop=(n0 + q == DS - 1))
                # gate with silu(z); write into xc[g]'s tile (xc dead now)
                for j, g in enumerate(wave):
                    sgl = repp.tile([128, HW], BF16, tag="sgl", bufs=1)
                    nc.sync.dma_start(sgl, sg_d[g][:, hs])
                    nc.vector.tensor_mul(yf[g][:, hs], py[j], sgl)
            out_proj_half(half)

        # ---------------- final residual + LayerNorm ----------------------
        mvall2 = wp.tile([128, 2, NTT], F32, name="mvall2")
        nmr2 = wp.tile([128, NTT], F32, name="nmr2")

        def sum3(i):
            # natural-time tile i: dir0 rows i*128.. (gbuf piece by row range)
            r0 = i * 128
            ph0 = 0 if r0 < HW else 1
            s0 = lnp.tile([128, DM], BF16, tag="s0")
            nc.sync.dma_start(
                s0, gbuf_h[ph0][0, r0 - ph0 * HW: r0 - ph0 * HW + 128, :])
            # direction-1 partial is in flipped local order
            lr = L - (i + 1) * 128
            ph1 = 0 if lr < HW else 1
            s1 = lnp.tile([128, DM], BF16, tag="s1")
            nc.sync.dma_start(
                s1, gbuf_h[ph1][1, lr - ph1 * HW: lr - ph1 * HW + 128, :])
            xt2 = lnp.tile([128, DM], BF16, tag="xt2")
            nc.sync.dma_start(xt2, x_nat[i * 128:(i + 1) * 128, :])
            pj = pp.tile([128, DM], F32, tag="ps")
            nc.tensor.matmul(pj, eyej_sb, s1, start=True, stop=False)
            nc.tensor.matmul(pj, eyen_sb, s0, start=False, stop=False)
            nc.tensor.matmul(pj, eyen_sb, xt2, start=False, stop=True)
            return pj

        # sweep 1: stats for every tile (batched rstd afterwards)
        for i in range(NTT):
            pj = sum3(i)
            st = lnp.tile([128, 6], F32, tag="st2")
            nc.vector.bn_stats(st, pj)
            nc.vector.bn_aggr(mvall2[:, :, i], st)
        nc.scalar.activation(mvall2[:, 1, :], mvall2[:, 1, :], AF.Ln,
                             bias=eps_sb[:, 0:1])
        nc.scalar.activation(mvall2[:, 1, :], mvall2[:, 1, :], AF.Exp,
                             scale=-0.5)
        nc.vector.tensor_mul(nmr2, mvall2[:, 0, :], mvall2[:, 1, :])
        nc.vector.tensor_scalar_mul(nmr2, nmr2, -1.0)
        # sweep 2: rebuild the sum (3 cheap matmuls) and normalize from PSUM
        for i in range(NTT):
            pj = sum3(i)
            s = lnp.tile([128, DM], F32, tag="ssum")
            nc.scalar.activation(s, pj, AF.Identity,
                                 scale=mvall2[:, 1, i:i + 1],
                                 bias=nmr2[:, i:i + 1])
            nc.vector.tensor_mul(s, s, ln2w_sb)
            nc.vector.tensor_add(s, s, ln2b_sb)
            nc.sync.dma_start(out[i * 128:(i + 1) * 128, :], s)
        pp2_ctx.__exit__(None, None, None)
        ppy_ctx.__exit__(None, None, None)


# ---------------- host side ----------------

def make_core_inputs(inputs):
    """Build the 8 per-core input dicts from the full problem inputs."""
    x = np.ascontiguousarray(inputs["x"], dtype=np.float32)        # [B, L, DM]
    ln1_w = inputs["ln1_w"].astype(np.float32)
    ln1_b = inputs["ln1_b"].astype(np.float32)
    in_w = inputs["in_proj_w"].astype(np.float32)                  # [2, 2DI, DM]
    conv_w = inputs["conv_w"].astype(np.float32)                   # [2, DI, 4]
    conv_b = inputs["conv_b"].astype(np.float32)                   # [2, DI]
    xp_w = inputs["x_proj_w"].astype(np.float32)                   # [2, 64, DI]
    dt_w = inputs["dt_proj_w"].astype(np.float32)                  # [2, DI, DR]
    dt_b = inputs["dt_proj_b"].astype(np.float32)                  # [2, DI]
    a_log = inputs["A_log"].astype(np.float32)                     # [2, DI, DS]
    d_par = inputs["D_param"].astype(np.float32)                   # [2, DI]
    out_w = inputs["out_proj_w"].astype(np.float32)                # [2, DM, DI]
    ln2_w = inputs["ln2_w"].astype(np.float32)
    ln2_b = inputs["ln2_b"].astype(np.float32)

    bf = ml_dtypes.bfloat16
    eye = np.eye(128, dtype=np.float32)
    per_dir = []
    for d in range(2):
        w = in_w[d]                                   # [2DI, DM]
        w_in_T = np.ascontiguousarray((w * ln1_w[None, :]).T)      # [DM, 2DI]
        v = w @ ln1_b                                  # [2DI]
        csum = conv_w[d].sum(axis=1)                   # [DI]
        convb_adj = conv_b[d] + csum * v[:DI]
        silub_adj = v[DI:]
        a_neg = -np.exp(a_log[d])                      # [DI, DS]
        per_dir.append(dict(
            w_in=w_in_T.astype(bf),
            convw=np.ascontiguousarray(conv_w[d].reshape(NG, 128, D_CONV)),
            convb=convb_adj.reshape(NG, 128),
            silub=silub_adj.reshape(NG, 128),
            w_xp=np.ascontiguousarray(xp_w[d].T).astype(bf),       # [DI, 64]
            w_dt=np.ascontiguousarray(dt_w[d].T).astype(bf),       # [DR, DI]
            dtb=dt_b[d].reshape(NG, 128),
            a_sc=np.ascontiguousarray(a_neg.reshape(NG, 128, DS)),
            d_vec=d_par[d].reshape(NG, 128),
            w_out=np.ascontiguousarray(out_w[d].T).astype(bf),     # [DI, DM]
        ))

    in_maps = []
    for core in range(N_CORES):
        d, b = core // 4, core % 4
        xb = x[b]
        m = dict(per_dir[d])
        m["x_d"] = (xb if d == 0 else np.ascontiguousarray(xb[::-1])).astype(bf)
        m["x_nat"] = xb.astype(bf)
        m["eye"] = eye
        m["eyen"] = eye.astype(bf)
        m["eyej"] = eye[::-1].copy().astype(bf)
        m["ln2w"] = ln2_w.reshape(1, DM)
        m["ln2b"] = ln2_b.reshape(1, DM)
        in_maps.append(m)
    return in_maps


_NC = None


def _get_module():
    global _NC
    if _NC is None:
        _NC = build_module()
    return _NC


def kernel(**inputs) -> np.ndarray:
    nc = _get_module()
    in_maps = make_core_inputs(inputs)
    res = run_bass_kernel_spmd(nc, in_maps, core_ids=list(range(N_CORES)))
    outs = [res.results[b]["out"] for b in range(B_SZ)]
    return np.stack(outs, axis=0)


if __name__ == "__main__":
    nc = build_module()
    print("module built ok")


# revision 32
# speedup vs baseline: 1.3732x; 1.0234x over previous
"""Bidirectional Mamba layer on 8 Trainium2 NeuronCores (Bass/Tile).

Sharding: 8 cores = 2 directions x 4 batch samples. Each core runs the full
Mamba block for its (direction, sample) pair; a pairwise AllGather combines
the two directions, and every core (redundantly, SPMD-symmetric) applies the
final residual + LayerNorm.

Layout: channels on partitions, time on the free axis, full L=2048 per op
(monolith — no time chunking, so the selective scan needs no state carry).
All matmuls run bf16 (single PE pass); elementwise work is bf16 for the DVE
2x mode. The per-(group, state) inner loop is:
  Act:  da = exp(dt * A[g,n])          (scalar engine)
  DVE/GPS: bt = p * B_n                (broadcast multiply)
  DVE:  h = tensor_tensor_scan(da, bt) (the irreducible 2 cyc/elem scan)
  DVE:  yn = h * C_n
  PE:   py += I @ yn                   (PSUM accumulation over states)
"""

import ml_dtypes
import numpy as np

import concourse.bass as bass
import concourse.bacc as bacc
import concourse.tile as tile
from concourse import mybir
from concourse.bass_utils import run_bass_kernel_spmd

# ---- problem shapes (hardcoded per contract) ----
B_SZ, L, DM = 4, 2048, 512
D_CONV, DS, DR = 4, 16, 32
DI = 1024                  # d_inner
E2 = 2 * DI                # in_proj rows
NG = DI // 128             # 8 channel blocks
NTT = L // 128             # 16 time tiles of 128
NSL = L // 512             # 4 psum-width slices
EPS = 1e-5
F32 = mybir.dt.float32
BF16 = mybir.dt.bfloat16
AF = mybir.ActivationFunctionType
OP = mybir.AluOpType

N_CORES = 8

# CoreSim does not implement Silu; tests flip this to use sigmoid+mult
USE_SILU = True
REPLICA_GROUPS = [[0, 4], [1, 5], [2, 6], [3, 7]]

# groups whose bt-/yn-multiply runs on GPSIMD instead of DVE.
# Empirically GPSIMD elementwise is net-negative here: it contends for the
# DVE's SBUF port and slows 2x-mode DVE ops more than it absorbs.
GPS_GS = ()
GPS_YS = ()


def _silu(nc, pool, out_tile, psum, bias=None):
    """out = silu(psum [+ bias]); Silu on HW, sigmoid+mult fallback for sim."""
    kw = {} if bias is None else {"bias": bias}
    if USE_SILU:
        nc.scalar.activation(out_tile, psum, AF.Silu, **kw)
    else:
        zb = pool.tile(list(out_tile.shape), F32, tag="_silu_zb", name="zb",
                       bufs=1)
        nc.scalar.activation(zb, psum, AF.Identity, **kw)
        nc.scalar.activation(out_tile, psum, AF.Sigmoid, **kw)
        nc.vector.tensor_mul(out_tile, zb, out_tile)


def build_module():
    nc = bacc.Bacc(
        "TRN2", target_bir_lowering=False, debug=False, num_devices=N_CORES
    )

    # ---------------- I/O ----------------
    x_d = nc.dram_tensor("x_d", [L, DM], BF16, kind="ExternalInput")
    x_nat = nc.dram_tensor("x_nat", [L, DM], BF16, kind="ExternalInput")
    w_in = nc.dram_tensor("w_in", [DM, E2], BF16, kind="ExternalInput")
    convw = nc.dram_tensor("convw", [NG, 128, D_CONV], F32, kind="ExternalInput")
    convb = nc.dram_tensor("convb", [NG, 128], F32, kind="ExternalInput")
    silub = nc.dram_tensor("silub", [NG, 128], F32, kind="ExternalInput")
    w_xp = nc.dram_tensor("w_xp", [DI, DR + 2 * DS], BF16, kind="ExternalInput")
    w_dt = nc.dram_tensor("w_dt", [DR, DI], BF16, kind="ExternalInput")
    dtb = nc.dram_tensor("dtb", [NG, 128], F32, kind="ExternalInput")
    a_sc = nc.dram_tensor("a_sc", [NG, 128, DS], F32, kind="ExternalInput")
    d_vec = nc.dram_tensor("d_vec", [NG, 128], F32, kind="ExternalInput")
    w_out = nc.dram_tensor("w_out", [DI, DM], BF16, kind="ExternalInput")
    eye = nc.dram_tensor("eye", [128, 128], F32, kind="ExternalInput")
    eyen = nc.dram_tensor("eyen", [128, 128], BF16, kind="ExternalInput")
    eyej = nc.dram_tensor("eyej", [128, 128], BF16, kind="ExternalInput")
    ln2w = nc.dram_tensor("ln2w", [1, DM], F32, kind="ExternalInput")
    ln2b = nc.dram_tensor("ln2b", [1, DM], F32, kind="ExternalInput")
    out = nc.dram_tensor("out", [L, DM], F32, kind="ExternalOutput")

    with tile.TileContext(nc) as tc:
        build_program(
            tc, x_d, x_nat, w_in, convw, convb, silub, w_xp, w_dt, dtb,
            a_sc, d_vec, w_out, eye, eyen, eyej, ln2w, ln2b, out,
        )
    nc.compile()
    return nc


def build_program(tc, x_d, x_nat, w_in, convw, convb, silub, w_xp, w_dt, dtb,
                  a_sc, d_vec, w_out, eye, eyen, eyej, ln2w, ln2b, out):
    nc = tc.nc
    HW = L // 2
    from contextlib import ExitStack

    ctx = ExitStack()
    with ctx:
        wp = ctx.enter_context(tc.tile_pool(name="wp", bufs=1))
        lnp = ctx.enter_context(tc.tile_pool(name="lnp", bufs=2))
        big = ctx.enter_context(tc.tile_pool(name="big", bufs=1))
        snp = ctx.enter_context(tc.tile_pool(name="snp", bufs=3))
        repp = ctx.enter_context(tc.tile_pool(name="repp", bufs=3))
        dramp = ctx.enter_context(tc.tile_pool(name="dramp", bufs=2, space="DRAM"))
        pp_ctx = tc.tile_pool(name="pp", space="PSUM", bufs=2)
        pp = pp_ctx.__enter__()

        # x-load + LN1 stats first so its DMAs aren't queued behind weights
        eps_sb = wp.tile([128, 1], F32)
        nc.vector.memset(eps_sb, EPS)
        mvall = wp.tile([128, 2, NTT], F32, name="mvall")
        nmr = wp.tile([128, NTT], F32, name="nmr")
        xts = []
        for i in range(NTT):
            xt = lnp.tile([128, DM], BF16, tag="xt", bufs=NTT)
            nc.sync.dma_start(xt, x_d[i * 128:(i + 1) * 128, :])
            xts.append(xt)
            st = lnp.tile([128, 6], F32, tag="st")
            nc.vector.bn_stats(st, xt)
            nc.vector.bn_aggr(mvall[:, :, i], st)
        # rstd batches split 4/12 so the first tiles (which gate in_proj)
        # unblock as early as possible
        for lo, hi in ((0, 4), (4, NTT)):
            nc.scalar.activation(mvall[:, 1, lo:hi], mvall[:, 1, lo:hi],
                                 AF.Ln, bias=eps_sb[:, 0:1])
            nc.scalar.activation(mvall[:, 1, lo:hi], mvall[:, 1, lo:hi],
                                 AF.Exp, scale=-0.5)
            nc.vector.tensor_mul(nmr[:, lo:hi], mvall[:, 0, lo:hi],
                                 mvall[:, 1, lo:hi])
            nc.vector.tensor_scalar_mul(nmr[:, lo:hi], nmr[:, lo:hi], -1.0)

        w_in_sb = wp.tile([128, DM // 128, E2], BF16)
        for k in range(DM // 128):
            nc.sync.dma_start(w_in_sb[:, k, :], w_in[k * 128:(k + 1) * 128, :])
        convw_sb = wp.tile([128, NG, D_CONV], F32)
        for g in range(NG):
            nc.sync.dma_start(convw_sb[:, g, :], convw[g])
        w_xp_sb = wp.tile([128, NG, DR + 2 * DS], BF16)
        for g in range(NG):
            nc.sync.dma_start(w_xp_sb[:, g, :], w_xp[g * 128:(g + 1) * 128, :])
        w_dt_sb = wp.tile([DR, DI], BF16)
        nc.sync.dma_start(w_dt_sb, w_dt[:, :])
        w_out_sb = wp.tile([128, NG, DM], BF16)
        for g in range(NG):
            nc.sync.dma_start(w_out_sb[:, g, :], w_out[g * 128:(g + 1) * 128, :])
        a_sb = wp.tile([128, NG, DS], F32)
        for g in range(NG):
            nc.sync.dma_start(a_sb[:, g, :], a_sc[g])
        eye_sb = wp.tile([128, 128], F32)
        nc.sync.dma_start(eye_sb, eye[:, :])
        eyen_sb = wp.tile([128, 128], BF16)
        nc.sync.dma_start(eyen_sb, eyen[:, :])
        eyej_sb = wp.tile([128, 128], BF16)
        nc.sync.dma_start(eyej_sb, eyej[:, :])

        def col_load(name, src):
            t = wp.tile([128, NG], F32, name=name)
            for g in range(NG):
                nc.sync.dma_start(t[:, g:g + 1], src[g:g + 1, :].rearrange("a b -> b a"))
            return t

        convb_sb = col_load("convb_sb", convb)
        silub_sb = col_load("silub_sb", silub)
        dtb_sb = col_load("dtb_sb", dtb)
        d_sb = col_load("d_sb", d_vec)

        ln2w_sb = wp.tile([128, DM], F32)
        nc.sync.dma_start(ln2w_sb, ln2w[0:1, :].to_broadcast([128, DM]))
        ln2b_sb = wp.tile([128, DM], F32)
        nc.sync.dma_start(ln2b_sb, ln2b[0:1, :].to_broadcast([128, DM]))

        sg_d = dramp.tile([NG, 128, L], BF16, bufs=1)
        bc_d = dramp.tile([2 * DS, L], BF16, bufs=1)

        # ---------------- phase A: LayerNorm1 + transpose ----------------
        # xn_t: [dm-part, k-block, t] bf16, consumed by in_proj
        xn_t = big.tile([128, DM // 128, L], BF16, name="xn_t")
        # normalize sweep on the resident bf16 tiles (no reload)
        for i in range(NTT):
            xt = xts[i]
            nc.scalar.activation(xt, xt, AF.Identity,
                                 scale=mvall[:, 1, i:i + 1],
                                 bias=nmr[:, i:i + 1])
            for kb in range(DM // 128):
                ps_t = pp.tile([128, 128], BF16, tag="pst")
                nc.tensor.transpose(ps_t, xt[:, kb * 128:(kb + 1) * 128],
                                    eyen_sb)
                nc.vector.tensor_copy(xn_t[:, kb, i * 128:(i + 1) * 128],
                                      ps_t)

        # ---------------- phase B: in_proj ----------------
        # xc half into conv staging (3-col zero halo at front)
        xzf = [big.tile([128, L + 3], BF16, name=f"xzf{g}") for g in range(NG)]
        for g in range(NG):
            nc.vector.memset(xzf[g][:, 0:3], 0.0)
            for s in range(NSL):
                pz = pp.tile([128, 512], F32, tag="ps", bufs=3)
                for k in range(DM // 128):
                    nc.tensor.matmul(pz, w_in_sb[:, k, g * 128:(g + 1) * 128],
                                     xn_t[:, k, s * 512:(s + 1) * 512],
                                     start=(k == 0), stop=(k == DM // 128 - 1))
                nc.vector.tensor_copy(xzf[g][:, 3 + s * 512:3 + (s + 1) * 512], pz)

        # z half + SiLU gate -> sg (spilled to DRAM, reloaded at gate)
        for g in range(NG):
            for s in range(NSL):
                pz = pp.tile([128, 512], F32, tag="ps", bufs=3)
                for k in range(DM // 128):
                    nc.tensor.matmul(
                        pz, w_in_sb[:, k, DI + g * 128: DI + (g + 1) * 128],
                        xn_t[:, k, s * 512:(s + 1) * 512],
                        start=(k == 0), stop=(k == DM // 128 - 1))
                sgt = snp.tile([128, 512], BF16, tag="sgt", bufs=2)
                _silu(nc, snp, sgt, pz, silub_sb[:, g:g + 1])
                nc.sync.dma_start(sg_d[g][:, s * 512:(s + 1) * 512], sgt)

        # ---------------- phase C: depthwise causal conv + SiLU ----------
        # 4-tap conv as a chain of fused (x*w_k)+acc DVE ops (the DVE is idle
        # during this phase; this keeps the 128 diag-matmuls off the PE)
        xc = [big.tile([128, L], BF16, name=f"xc{g}") for g in range(NG)]
        for g in range(NG):
            # conv accumulators borrow the phase-E da/bt tag space
            accs = [snp.tile([128, 2, HW], BF16, tag=t, bufs=2,
                             name=f"cacc_{t}").rearrange("p a b -> p (a b)")
                    for t in ("da", "bt")]
            nc.vector.tensor_scalar_mul(accs[0], xzf[g][:, 0:L],
                                        convw_sb[:, g, 0:1])
            for k in range(1, D_CONV):
                nc.vector.scalar_tensor_tensor(
                    accs[k % 2], xzf[g][:, k:k + L], convw_sb[:, g, k:k + 1],
                    accs[(k + 1) % 2], op0=OP.mult, op1=OP.add)
            _silu(nc, snp, xc[g], accs[(D_CONV - 1) % 2],
                  convb_sb[:, g:g + 1])

        # ---------------- phase D: x_proj -> (dt_r, B, C) -----------------
        xdbl = big.tile([64, L], BF16, name="xdbl")
        for s in range(NSL):
            pxd = pp.tile([64, 512], F32, tag="pxd", bufs=1)
            for g in range(NG):
                nc.tensor.matmul(pxd, w_xp_sb[:, g, :],
                                 xc[g][:, s * 512:(s + 1) * 512],
                                 start=(g == 0), stop=(g == NG - 1))
            nc.vector.tensor_copy(xdbl[:, s * 512:(s + 1) * 512], pxd)
            nc.sync.dma_start(bc_d[:, s * 512:(s + 1) * 512],
                              xdbl[DR:DR + 2 * DS, s * 512:(s + 1) * 512])

        # dt = softplus(dt_proj @ dt_r + bias); p = dt * xc
        # dt_c[0..3] reuse xn_t's space, dt_c[4..7] reuse w_in_sb (both dead
        # after in_proj); p_c reuses the conv staging xzf (dead after conv)
        dt_c = [xn_t[:, g, :] if g < 4 else w_in_sb[:, g - 4, :]
                for g in range(NG)]
        p_c = [xzf[g][:, 0:L] for g in range(NG)]
        def dt_exp(g):
            for s in range(NSL):
                sl = slice(s * 512, (s + 1) * 512)
                pdt = pp.tile([128, 512], F32, tag="ps", bufs=3)
                nc.tensor.matmul(pdt, w_dt_sb[:, g * 128:(g + 1) * 128],
                                 xdbl[0:DR, sl],
                                 start=True, stop=True)
                nc.scalar.activation(dt_c[g][:, sl], pdt, AF.Exp,
                                     bias=dtb_sb[:, g:g + 1])

        def dt_ln_p(g):
            nc.scalar.activation(dt_c[g], dt_c[g], AF.Ln, bias=1.0)
            nc.vector.tensor_mul(p_c[g], dt_c[g], xc[g])

        # group 0 fully first so phase E's first wave unblocks early
        dt_exp(0)
        dt_ln_p(0)
        for g in range(1, NG):
            dt_exp(g)
        for g in range(1, NG):
            dt_ln_p(g)

        # ---------------- phase E: selective scan ------------------------
        # waves of 3 groups share each B/C broadcast; time in halves of 1024.
        # py uses 6 PSUM banks, leaving 2 for out_proj so each half's
        # out_proj + collective piece overlaps the next half's compute.
        pp_ctx.__exit__(None, None, None)
        WAVES = [(0, 1, 2), (3, 4, 5), (6, 7)]
        yf = [xc[g] for g in range(NG)]
        ypart_h = [dramp.tile([HW, DM], BF16, bufs=1, name=f"yph{h}")
                   for h in range(2)]
        gbuf_h = [dramp.tile([2, HW, DM], BF16, bufs=1, name=f"gbh{h}")
                  for h in range(2)]
        ppy_ctx = tc.tile_pool(name="ppy", space="PSUM", bufs=1)
        ppy = ppy_ctx.__enter__()
        pp2_ctx = tc.tile_pool(name="pp2", space="PSUM", bufs=2)
        pp = pp2_ctx.__enter__()
        hcars = [snp.tile([128, 4, DS], BF16, tag="hcar", bufs=3,
                          name=f"hcar{w}") for w in range(3)]

        def out_proj_half(half):
            hs = slice(half * HW, (half + 1) * HW)
            for tb in range(8):
                t0 = half * HW + tb * 128
                po = pp.tile([128, DM], F32, tag="ps")
                for g in range(NG):
                    nc.tensor.matmul(po, yf[g][:, t0:t0 + 128],
                                     w_out_sb[:, g, :],
                                     start=(g == 0), stop=(g == NG - 1))
                ot = lnp.tile([128, DM], BF16, tag="ot")
                nc.scalar.copy(ot, po)
                nc.sync.dma_start(ypart_h[half][tb * 128:(tb + 1) * 128, :], ot)
            nc.gpsimd.collective_compute(
                "AllGather", OP.bypass, replica_groups=REPLICA_GROUPS,
                ins=[ypart_h[half].opt()], outs=[gbuf_h[half].opt()],
            )

        for half in (0, 1):
            hs = slice(half * HW, (half + 1) * HW)
            for wv, wave in enumerate(WAVES):
                hcar = hcars[wv]
                py = [ppy.tile([128, HW], F32, tag=f"py{j}", bufs=1,
                               name=f"py{j}") for j in range(len(wave))]
                for j, g in enumerate(wave):
                    yd = snp.tile([128, HW], BF16, tag="yd", bufs=2)
                    nc.vector.tensor_scalar_mul(yd, xc[g][:, hs],
                                                d_sb[:, g:g + 1])
                    for s in range(2):
                        nc.tensor.matmul(py[j][:, s * 512:(s + 1) * 512],
                                         eyen_sb,
                                         yd[:, s * 512:(s + 1) * 512],
                                         start=True, stop=False)
                for n0 in range(0, DS, 2):
                    # paired states: one broadcast + one mul covers n0, n0+1
                    brep = repp.tile([128, 2, HW], BF16, tag="brep", bufs=2)
                    nc.sync.dma_start(
                        brep, bc_d[n0:n0 + 2, hs].rearrange("a b -> () a b").to_broadcast([128, 2, HW]))
                    crep = repp.tile([128, 2, HW], BF16, tag="crep", bufs=2)
                    nc.sync.dma_start(
                        crep,
                        bc_d[DS + n0:DS + n0 + 2, hs].rearrange("a b -> () a b").to_broadcast([128, 2, HW]))
                    for j, g in enumerate(wave):
                        da = snp.tile([128, 2, HW], BF16, tag="da", bufs=2)
                        for q in range(2):
                            nc.scalar.activation(da[:, q, :], dt_c[g][:, hs],
                                                 AF.Exp,
                                                 scale=a_sb[:, g, n0 + q:n0 + q + 1])
                        bt = snp.tile([128, 2, HW], BF16, tag="bt",
                                      name="btd", bufs=2)
                        nc.vector.tensor_mul(
                            bt,
                            p_c[g][:, hs].rearrange("p b -> p () b")
                            .to_broadcast([128, 2, HW]),
                            brep)
                        h = snp.tile([128, 2, HW], BF16, tag="h", bufs=2)
                        for q in range(2):
                            n = n0 + q
                            init = 0.0 if half == 0 else hcar[:, j, n:n + 1]
                            nc.vector.tensor_tensor_scan(h[:, q, :],
                                                         da[:, q, :],
                                                         bt[:, q, :],
                                                         initial=init,
                                                         op0=OP.mult,
                                                         op1=OP.add)
                        if half == 0:
                            nc.vector.tensor_copy(hcar[:, j, n0:n0 + 2],
                                                      h[:, :, HW - 1])
                        yn = snp.tile([128, 2, HW], BF16, tag="yn", bufs=2)
                        nc.vector.tensor_mul(yn, h, crep)
                        for q in range(2):
                            for s in range(2):
                                nc.tensor.matmul(
                                    py[j][:, s * 512:(s + 1) * 512], eyen_sb,
                                    yn[:, q, s * 512:(s + 1) * 512],
                                    start=False,
                                    stop=(n0 + q == DS - 1))
                # gate with silu(z); write into xc[g]'s tile (xc dead now)
                for j, g in enumerate(wave):
                    sgl = repp.tile([128, HW], BF16, tag="sgl", bufs=1)
                    nc.sync.dma_start(sgl, sg_d[g][:, hs])
                    nc.vector.tensor_mul(yf[g][:, hs], py[j], sgl)
            out_proj_half(half)

        # ---------------- final residual + LayerNorm ----------------------
        def sum3(i):
            # natural-time tile i: dir0 rows i*128.. (gbuf piece by row range)
            r0 = i * 128
            ph0 = 0 if r0 < HW else 1
            s0 = lnp.tile([128, DM], BF16, tag="s0")
            nc.sync.dma_start(
                s0, gbuf_h[ph0][0, r0 - ph0 * HW: r0 - ph0 * HW + 128, :])
            # direction-1 partial is in flipped local order
            lr = L - (i + 1) * 128
            ph1 = 0 if lr < HW else 1
            s1 = lnp.tile([128, DM], BF16, tag="s1")
            nc.sync.dma_start(
                s1, gbuf_h[ph1][1, lr - ph1 * HW: lr - ph1 * HW + 128, :])
            xt2 = lnp.tile([128, DM], BF16, tag="xt2")
            nc.sync.dma_start(xt2, x_nat[i * 128:(i + 1) * 128, :])
            pj = pp.tile([128, DM], F32, tag="ps")
            nc.tensor.matmul(pj, eyej_sb, s1, start=True, stop=False)
            nc.tensor.matmul(pj, eyen_sb, s0, start=False, stop=False)
            nc.tensor.matmul(pj, eyen_sb, xt2, start=False, stop=True)
            return pj

        # single sweep: per-tile rstd via Sqrt table + DVE reciprocal
        # (no Ln/Exp table ping-pong, no second load+matmul pass)
        for i in range(NTT):
            pj = sum3(i)
            st = lnp.tile([128, 6], F32, tag="st2")
            nc.vector.bn_stats(st, pj)
            mv = lnp.tile([128, 2], F32, tag="mv2")
            nc.vector.bn_aggr(mv, st)
            sd = lnp.tile([128, 2], F32, tag="sd2")
            nc.scalar.activation(sd[:, 1:2], mv[:, 1:2], AF.Sqrt,
                                 bias=eps_sb[:, 0:1])
            nc.vector.reciprocal(sd[:, 1:2], sd[:, 1:2])
            nc.vector.tensor_scalar(out=sd[:, 0:1], in0=mv[:, 0:1],
                                    scalar1=sd[:, 1:2], scalar2=-1.0,
                                    op0=OP.mult, op1=OP.mult)
            s = lnp.tile([128, DM], F32, tag="ssum")
            nc.scalar.activation(s, pj, AF.Identity,
                                 scale=sd[:, 1:2],
                                 bias=sd[:, 0:1])
            nc.vector.tensor_mul(s, s, ln2w_sb)
            nc.vector.tensor_add(s, s, ln2b_sb)
            nc.sync.dma_start(out[i * 128:(i + 1) * 128, :], s)
        pp2_ctx.__exit__(None, None, None)
        ppy_ctx.__exit__(None, None, None)


# ---------------- host side ----------------

def make_core_inputs(inputs):
    """Build the 8 per-core input dicts from the full problem inputs."""
    x = np.ascontiguousarray(inputs["x"], dtype=np.float32)        # [B, L, DM]
    ln1_w = inputs["ln1_w"].astype(np.float32)
    ln1_b = inputs["ln1_b"].astype(np.float32)
    in_w = inputs["in_proj_w"].astype(np.float32)                  # [2, 2DI, DM]
    conv_w = inputs["conv_w"].astype(np.float32)                   # [2, DI, 4]
    conv_b = inputs["conv_b"].astype(np.float32)                   # [2, DI]
    xp_w = inputs["x_proj_w"].astype(np.float32)                   # [2, 64, DI]
    dt_w = inputs["dt_proj_w"].astype(np.float32)                  # [2, DI, DR]
    dt_b = inputs["dt_proj_b"].astype(np.float32)                  # [2, DI]
    a_log = inputs["A_log"].astype(np.float32)                     # [2, DI, DS]
    d_par = inputs["D_param"].astype(np.float32)                   # [2, DI]
    out_w = inputs["out_proj_w"].astype(np.float32)                # [2, DM, DI]
    ln2_w = inputs["ln2_w"].astype(np.float32)
    ln2_b = inputs["ln2_b"].astype(np.float32)

    bf = ml_dtypes.bfloat16
    eye = np.eye(128, dtype=np.float32)
    per_dir = []
    for d in range(2):
        w = in_w[d]                                   # [2DI, DM]
        w_in_T = np.ascontiguousarray((w * ln1_w[None, :]).T)      # [DM, 2DI]
        v = w @ ln1_b                                  # [2DI]
        csum = conv_w[d].sum(axis=1)                   # [DI]
        convb_adj = conv_b[d] + csum * v[:DI]
        silub_adj = v[DI:]
        a_neg = -np.exp(a_log[d])                      # [DI, DS]
        per_dir.append(dict(
            w_in=w_in_T.astype(bf),
            convw=np.ascontiguousarray(conv_w[d].reshape(NG, 128, D_CONV)),
            convb=convb_adj.reshape(NG, 128),
            silub=silub_adj.reshape(NG, 128),
            w_xp=np.ascontiguousarray(xp_w[d].T).astype(bf),       # [DI, 64]
            w_dt=np.ascontiguousarray(dt_w[d].T).astype(bf),       # [DR, DI]
            dtb=dt_b[d].reshape(NG, 128),
            a_sc=np.ascontiguousarray(a_neg.reshape(NG, 128, DS)),
            d_vec=d_par[d].reshape(NG, 128),
            w_out=np.ascontiguousarray(out_w[d].T).astype(bf),     # [DI, DM]
        ))

    in_maps = []
    for core in range(N_CORES):
        d, b = core // 4, core % 4
        xb = x[b]
        m = dict(per_dir[d])
        m["x_d"] = (xb if d == 0 else np.ascontiguousarray(xb[::-1])).astype(bf)
        m["x_nat"] = xb.astype(bf)
        m["eye"] = eye
        m["eyen"] = eye.astype(bf)
        m["eyej"] = eye[::-1].copy().astype(bf)
        m["ln2w"] = ln2_w.reshape(1, DM)
        m["ln2b"] = ln2_b.reshape(1, DM)
        in_maps.append(m)
    return in_maps


_NC = None


def _get_module():
    global _NC
    if _NC is None:
        _NC = build_module()
    return _NC


def kernel(**inputs) -> np.ndarray:
    nc = _get_module()
    in_maps = make_core_inputs(inputs)
    res = run_bass_kernel_spmd(nc, in_maps, core_ids=list(range(N_CORES)))
    outs = [res.results[b]["out"] for b in range(B_SZ)]
    return np.stack(outs, axis=0)


if __name__ == "__main__":
    nc = build_module()
    print("module built ok")
